# revision 1
# baseline (speedup 1.0000x reference)
"""Trainium2 Bass kernel for nn_EnhancedDRKANTreeNet (KAN layer + LayerNorm + SE gate).

Strategy: data-parallel over the 8192 tokens across 8 NeuronCores (1024 tokens
per core — exactly one batch row each). Per core, everything is computed in
feature-major ("orientation A") layout: tiles are [feature_partition, token].

  out^T[o, n] = sum_i x^T[i, n]·Wb[o, i] + sum_{i,g} bn_g[i, n]·Ws[o, i, g]

The i-contraction (1024) and (i,g)-contraction (3072) are both mapped to
128-deep PE matmul accumulations with the pre-transposed weights stationary
(lhsT) and the x / normalized-basis tiles moving (rhs), in float32r (full-rate
fp32 PE mode for moving-dim >= 256).

LayerNorm stats (reduction over features = partitions) are computed with
ones-vector matmuls on the PE; the normalization apply is restructured as
  y = out^T * (ln_w (x) r) + (ln_w (x) (-mu*r) + ln_b (x) 1)
where both broadcast factors are produced by tiny K=1/K=2 matmuls
(outer-products on the PE), so the DVE only does 2 elementwise ops per tile.
rsqrt is computed on the DVE via the int32 bit-hack seed + 2 Newton steps
(ACT Rsqrt/Reciprocal are banned; avoids an ACT table swap for Sqrt).

SE: h^T = relu(W1·y^T + b1) via K=128 accumulation, se^T = sigmoid(W2·h^T+b2)
via a K=32 matmul; biases ride the ACT activations as per-partition bias APs.

All ACT functions used (Relu, Square, Sigmoid, Copy) live in the single
`sigmoid_and_others` table set: no table thrashing.
"""

import os
from contextlib import ExitStack

import numpy as np

P = 128
T = 512            # tokens per tile (= max fp32 moving dim = one PSUM bank)
NT = 2             # token tiles per core
NTOK = NT * T      # 1024 tokens per core
NC_I = 8           # contraction chunks of 128 over D_IN
NCH = 4            # rhs channels per i-chunk: x, bn[-1], bn[0], bn[1]
NO = 8             # output-feature chunks of 128
D = 1024
N_CORES = 8
GRID = [-1.0, 0.0, 1.0]
EPS_BASIS = 1e-6
LN_EPS = 1e-5
RSQRT_MAGIC = 0x5F3759DF

_cache = {}


def _build_nc(reps: int = 1):
    import concourse.bass as bass
    import concourse.mybir as mybir
    import concourse.tile as tile
    from concourse import bacc

    f32 = mybir.dt.float32
    f32r = mybir.dt.float32r
    i32 = mybir.dt.int32
    AF = mybir.ActivationFunctionType
    OP = mybir.AluOpType
    ts = bass.ts

    nc = bacc.Bacc(
        "TRN2",
        target_bir_lowering=False,
        debug=False,
        enable_asserts=False,
        num_devices=N_CORES,
    )

    xt_d = nc.dram_tensor("xt", [NC_I, P, NTOK], f32r, kind="ExternalInput")
    w_d = nc.dram_tensor("w", [NC_I, P, NCH * D], f32r, kind="ExternalInput")
    w1t_d = nc.dram_tensor("w1t", [NO, P, 32], f32r, kind="ExternalInput")
    w2t_d = nc.dram_tensor("w2t", [32, D], f32r, kind="ExternalInput")
    lnw1p_d = nc.dram_tensor("lnw1p", [1, NO * P], f32r, kind="ExternalInput")
    lnb_d = nc.dram_tensor("lnb", [P, NO], f32, kind="ExternalInput")
    ones_d = nc.dram_tensor("ones", [P, 1], f32r, kind="ExternalInput")
    b1_d = nc.dram_tensor("b1", [32, 1], f32, kind="ExternalInput")
    b2_d = nc.dram_tensor("b2", [P, NO], f32, kind="ExternalInput")
    out_d = nc.dram_tensor("outT", [NO, P, NTOK], f32, kind="ExternalOutput")

    with tile.TileContext(nc) as tc, ExitStack() as ctx:
        wp = ctx.enter_context(tc.tile_pool(name="wp", bufs=3))
        xp = ctx.enter_context(tc.tile_pool(name="xp", bufs=3))
        bp = ctx.enter_context(tc.tile_pool(name="bp", bufs=2))
        bnp = ctx.enter_context(tc.tile_pool(name="bnp", bufs=2))
        op_pool = ctx.enter_context(tc.tile_pool(name="op", bufs=2))
        sqp = ctx.enter_context(tc.tile_pool(name="sqp", bufs=2))
        sep = ctx.enter_context(tc.tile_pool(name="sep", bufs=2))
        stp = ctx.enter_context(tc.tile_pool(name="stp", bufs=2))
        cp = ctx.enter_context(tc.tile_pool(name="cp", bufs=1))
        pp = ctx.enter_context(tc.tile_pool(name="pp", bufs=8, space="PSUM"))

        # warm the sigmoid_and_others ACT table set at t=0 so the ~2.7us
        # table load overlaps the initial weight/x DMAs instead of gating the
        # first basis activation
        warm_t = cp.tile([P, 1], f32, tag="warm")
        nc.scalar.activation(
            warm_t[:], nc.const_aps.tensor(1.0, (P, 1)), AF.Relu
        )

        # ---- constants, loaded once ----
        w1t_t = cp.tile([P, NO, 32], f32r, tag="w1t")
        nc.gpsimd.dma_start(w1t_t[:], w1t_d.ap().rearrange("c p j -> p c j"))
        w2t_t = cp.tile([32, D], f32r, tag="w2t")
        nc.gpsimd.dma_start(w2t_t[:], w2t_d.ap())
        lnw1p_t = cp.tile([1, NO, P], f32r, tag="lnw1p")
        nc.gpsimd.dma_start(
            lnw1p_t[:], lnw1p_d.ap().rearrange("a (c p) -> a c p", c=NO)
        )
        lnb_t = cp.tile([P, NO], f32, tag="lnb")
        nc.gpsimd.dma_start(lnb_t[:], lnb_d.ap())
        b1_t = cp.tile([32, 1], f32, tag="b1")
        nc.gpsimd.dma_start(b1_t[:], b1_d.ap())
        b2_t = cp.tile([P, NO], f32, tag="b2")
        nc.gpsimd.dma_start(b2_t[:], b2_d.ap())
        ones_t = cp.tile([P, 1], f32r, tag="ones")
        nc.gpsimd.dma_start(ones_t[:], ones_d.ap())

        def emit_body():
            outs_all, sA_all, sB_all = [], [], []
            for m in range(NT):
                # ---- main matmul accumulation over (i-chunk, channel) ----
                ps = [pp.tile([P, T], f32, tag="ps", name=f"ps_{m}_{o}") for o in range(NO)]
                for c in range(NC_I):
                    x_t = xp.tile([P, T], f32r, tag="x")
                    nc.sync.dma_start(x_t[:], xt_d.ap()[c, :, ts(m, T)])

                    # basis: r_g = relu(1-|x-g|) on ACT, squares on gpsimd,
                    # normalization on DVE. The sigma-trick folds bn_0 into the
                    # host-combined weights: channels are [x, bn_-1, bn_+1, sigma]
                    # with sigma = sum_g bn_g = 1 - eps/(S+eps).
                    b = []
                    for gi, g in enumerate(GRID):
                        r_t = bp.tile([P, T], f32, tag=f"r{gi}")
                        # |x - g| = Abs(s*x + b) with s=+-1 so b stays in {0.0, 1.0}
                        # (only those float consts have pre-registered bias APs)
                        sgn = -1.0 if g > 0 else 1.0
                        nc.scalar.activation(
                            r_t[:], x_t[:].bitcast(f32), AF.Abs, bias=abs(g), scale=sgn
                        )
                        nc.scalar.activation(r_t[:], r_t[:], AF.Relu, bias=1.0, scale=-1.0)
                        b_t = bp.tile([P, T], f32, tag=f"b{gi}")
                        nc.gpsimd.tensor_tensor(b_t[:], r_t[:], r_t[:], OP.mult)
                        b.append(b_t)
                    s_t = bp.tile([P, T], f32, tag="s")
                    nc.vector.tensor_tensor(s_t[:], b[0][:], b[1][:], OP.add)
                    nc.vector.scalar_tensor_tensor(
                        s_t[:], b[2][:], EPS_BASIS, s_t[:], OP.add, OP.add
                    )
                    inv_t = bp.tile([P, T], f32, tag="inv")
                    nc.vector.reciprocal_approx_fast(out=inv_t[:], in_=s_t[:])
                    bnm_t = bnp.tile([P, T], f32r, tag="bnm")
                    nc.vector.tensor_tensor(bnm_t[:], b[0][:], inv_t[:], OP.mult)
                    bnp_t = bnp.tile([P, T], f32r, tag="bnp")
                    nc.vector.tensor_tensor(bnp_t[:], b[2][:], inv_t[:], OP.mult)
                    sg_t = bnp.tile([P, T], f32r, tag="sgm")
                    nc.vector.tensor_scalar(
                        sg_t[:], inv_t[:], -EPS_BASIS, 1.0, OP.mult, OP.add
                    )
                    rhs_list = [x_t, bnm_t, bnp_t, sg_t]

                    w_t = wp.tile([P, NCH, D], f32r, tag="w")
                    w_src = w_d.ap()[c].rearrange("p (ch d) -> p ch d", ch=NCH)
                    for ch in range(NCH):
                        nc.sync.dma_start(w_t[:, ch], w_src[:, ch])
                    for ch in range(NCH):
                        rhs = rhs_list[ch][:]
                        for o in range(NO):
                            nc.tensor.matmul(
                                ps[o][:],
                                lhsT=w_t[:, ch, ts(o, P)],
                                rhs=rhs,
                                start=(c == 0 and ch == 0),
                                stop=(c == NC_I - 1 and ch == NCH - 1),
                            )

                # ---- copy out, squares, LN stats via ones-matmuls ----
                outs = []
                psA = pp.tile([1, T], f32, tag="ps", name=f"psA_{m}")
                psB = pp.tile([1, T], f32, tag="ps", name=f"psB_{m}")
                for o in range(NO):
                    o_t = op_pool.tile([P, T], f32r, tag=f"out{o}")
                    nc.vector.tensor_copy(out=o_t[:], in_=ps[o][:])
                    outs.append(o_t)
                    sq_t = sqp.tile([P, T], f32r, tag="sq")
                    nc.scalar.activation(sq_t[:], ps[o][:], AF.Square)
                    nc.tensor.matmul(
                        psA[:],
                        lhsT=ones_t[:],
                        rhs=o_t[:],
                        start=(o == 0),
                        stop=(o == NO - 1),
                    )
                    nc.tensor.matmul(
                        psB[:],
                        lhsT=ones_t[:],
                        rhs=sq_t[:],
                        start=(o == 0),
                        stop=(o == NO - 1),
                    )
                # free the stats PSUM bank immediately so the next tile's main
                # accumulation can take all 8 banks while the stats chain runs
                sA_t = stp.tile([1, T], f32, tag="sA")
                nc.vector.tensor_copy(out=sA_t[:], in_=psA[:])
                sB_t = stp.tile([1, T], f32, tag="sB")
                nc.vector.tensor_copy(out=sB_t[:], in_=psB[:])
                outs_all.append(outs)
                sA_all.append(sA_t)
                sB_all.append(sB_t)

            for m in range(NT):
                outs = outs_all[m]
                # ---- per-token stats: mu, var, rsqrt (bit-hack + 2x Newton) ----
                mu_t = stp.tile([1, T], f32, tag="mu")
                nc.vector.tensor_scalar(mu_t[:], sA_all[m][:], 1.0 / D, 0.0, OP.mult)
                e2_t = stp.tile([1, T], f32, tag="e2")
                nc.vector.tensor_scalar(e2_t[:], sB_all[m][:], 1.0 / D, LN_EPS, OP.mult, OP.add)
                var_t = stp.tile([1, T], f32, tag="var")
                # var+eps = e2 - mu*mu
                nc.vector.scalar_tensor_tensor(
                    var_t[:], mu_t[:], 0.0, mu_t[:], OP.bypass, OP.mult
                )
                nc.vector.scalar_tensor_tensor(
                    var_t[:], var_t[:], -1.0, e2_t[:], OP.mult, OP.add
                )
                zw_t = stp.tile([1, T], f32, tag="zw")
                nc.vector.tensor_scalar(
                    zw_t[:].bitcast(i32), var_t[:].bitcast(i32), 1, 0, OP.arith_shift_right
                )
                nc.vector.tensor_scalar(
                    zw_t[:].bitcast(i32), zw_t[:].bitcast(i32), -1, RSQRT_MAGIC,
                    OP.mult, OP.add,
                )
                t1_t = stp.tile([1, T], f32, tag="t1")
                z_t = stp.tile([1, T], f32r, tag="z")
                for it in range(2):
                    nc.vector.tensor_tensor(t1_t[:], zw_t[:], zw_t[:], OP.mult)
                    nc.vector.tensor_tensor(t1_t[:], t1_t[:], var_t[:], OP.mult)
                    nc.vector.tensor_scalar(t1_t[:], t1_t[:], -0.5, 1.5, OP.mult, OP.add)
                    dst = z_t if it == 1 else zw_t
                    nc.vector.tensor_tensor(dst[:], zw_t[:], t1_t[:], OP.mult)
                mr_t = stp.tile([1, T], f32r, tag="mr")
                nc.vector.scalar_tensor_tensor(
                    mr_t[:], mu_t[:], -1.0, z_t[:], OP.mult, OP.mult
                )

                # ---- LN apply + SE hidden accumulation ----
                psH = pp.tile([32, T], f32, tag="ps", name=f"psH_{m}")
                for o in range(NO):
                    rl = pp.tile([P, T], f32, tag="ps", name=f"rl_{m}_{o}")
                    nc.tensor.matmul(
                        rl[:],
                        lhsT=lnw1p_t[:, o, :],
                        rhs=z_t[:],
                        start=True,
                        stop=True,
                    )
                    bc = pp.tile([P, T], f32, tag="ps", name=f"bc_{m}_{o}")
                    nc.tensor.matmul(
                        bc[:],
                        lhsT=lnw1p_t[:, o, :],
                        rhs=mr_t[:],
                        start=True,
                        stop=True,
                    )
                    y_t = outs[o]
                    nc.vector.tensor_tensor(y_t[:], y_t[:], rl[:], OP.mult)
                    nc.vector.scalar_tensor_tensor(
                        y_t[:], y_t[:], lnb_t[:, o:o + 1], bc[:], OP.add, OP.add
                    )
                    nc.tensor.matmul(
                        psH[:],
                        lhsT=w1t_t[:, o, :],
                        rhs=y_t[:],
                        start=(o == 0),
                        stop=(o == NO - 1),
                    )

                hr_t = sep.tile([32, T], f32r, tag="hr")
                nc.scalar.activation(hr_t[:], psH[:], AF.Relu, bias=b1_t[:], scale=1.0)

                # ---- SE gate + final multiply + store ----
                for o in range(NO):
                    psS = pp.tile([P, T], f32, tag="ps", name=f"psS_{m}_{o}")
                    nc.tensor.matmul(
                        psS[:],
                        lhsT=w2t_t[:, ts(o, P)],
                        rhs=hr_t[:],
                        start=True,
                        stop=True,
                    )
                    se_t = sep.tile([P, T], f32, tag="se")
                    nc.scalar.activation(
                        se_t[:], psS[:], AF.Sigmoid, bias=b2_t[:, o:o + 1], scale=1.0
                    )
                    y_t = outs[o]
                    fin_t = sep.tile([P, T], f32, tag="fin")
                    nc.vector.tensor_tensor(fin_t[:], y_t[:].bitcast(f32), se_t[:], OP.mult)
                    nc.sync.dma_start(out_d.ap()[o, :, ts(m, T)], fin_t[:])

        for _rep in range(reps):
            emit_body()

    nc.compile()
    return nc


def _get_nc():
    if "nc" not in _cache:
        _cache["nc"] = _build_nc()
    return _cache["nc"]


def _prep_host(inputs):
    f = np.float32
    x = np.asarray(inputs["x"], f)
    base_weight = np.asarray(inputs["base_weight"], f)
    spline_weight = np.asarray(inputs["spline_weight"], f)
    ln_w = np.asarray(inputs["ln_w"], f)
    ln_b = np.asarray(inputs["ln_b"], f)
    se_w1 = np.asarray(inputs["se_w1"], f)
    se_b1 = np.asarray(inputs["se_b1"], f)
    se_w2 = np.asarray(inputs["se_w2"], f)
    se_b2 = np.asarray(inputs["se_b2"], f)

    xt_all = x.reshape(N_CORES, NTOK, D).transpose(0, 2, 1)  # [core, D, ntok]

    w_all = np.empty((NC_I, P, NCH, D), f)
    w_all[:, :, 0, :] = base_weight.T.reshape(NC_I, P, D)
    wsT = spline_weight.transpose(1, 2, 0)  # [i, g, o]
    # sigma-trick: bn_0 = sigma - bn_-1 - bn_+1, so
    # sum_g bn_g Ws_g = bn_-1 (W_-1 - W_0) + bn_+1 (W_+1 - W_0) + sigma W_0
    w_all[:, :, 1, :] = (wsT[:, 0, :] - wsT[:, 1, :]).reshape(NC_I, P, D)
    w_all[:, :, 2, :] = (wsT[:, 2, :] - wsT[:, 1, :]).reshape(NC_I, P, D)
    w_all[:, :, 3, :] = wsT[:, 1, :].reshape(NC_I, P, D)
    w_all = np.ascontiguousarray(w_all.reshape(NC_I, P, NCH * D))

    shared = {
        "w": w_all,
        "w1t": np.ascontiguousarray(se_w1.T.reshape(NO, P, 32)),
        "w2t": np.ascontiguousarray(se_w2.T),
        "lnw1p": np.ascontiguousarray(ln_w.reshape(1, NO * P)),
        "lnb": np.ascontiguousarray(ln_b.reshape(NO, P).T),
        "ones": np.ones((P, 1), f),
        "b1": np.ascontiguousarray(se_b1.reshape(32, 1)),
        "b2": np.ascontiguousarray(se_b2.reshape(NO, P).T),
    }
    in_maps = []
    for k in range(N_CORES):
        m = dict(shared)
        m["xt"] = np.ascontiguousarray(
            xt_all[k].reshape(NC_I, P, NTOK)
        )
        in_maps.append(m)
    return in_maps


def kernel(**inputs) -> np.ndarray:
    from concourse.bass_utils import run_bass_kernel_spmd

    nc = _get_nc()
    in_maps = _prep_host(inputs)
    trace = bool(int(os.environ.get("KERNEL_TRACE", "0")))
    res = run_bass_kernel_spmd(
        nc, in_maps, core_ids=list(range(N_CORES)), trace=trace
    )
    _cache["last_result"] = res
    outs = []
    for k in range(N_CORES):
        outT = res.results[k]["outT"]          # [NO, P, NTOK]
        outs.append(outT.reshape(D, NTOK).T)   # [ntok, o]
    out = np.concatenate(outs, axis=0).reshape(8, 1024, 1024)
    return np.ascontiguousarray(out.astype(np.float32))



# revision 35
# speedup vs baseline: 1.1636x; 1.1636x over previous
"""Trainium2 Bass kernel for nn_EnhancedDRKANTreeNet (KAN layer + LayerNorm + SE gate).

Data-parallel over 8192 tokens across 8 NeuronCores (1024 tokens/core), all
compute feature-major: tiles are [feature_partition, token].

  out^T[o, n] = sum_i x^T[i, n]*Wb[o, i] + sum_{i,g} bn_g[i, n]*Ws[o, i, g]

All matmul operands are bf16 (1 cycle/row on the PE, same as fp32r, but half
the HBM traffic and 2x DVE throughput on elementwise tiles); PSUM accumulates
fp32. The sigma trick folds the bn_0 spline channel into [x, bn_-1, bn_+1,
sigma] with host-combined weights. Weights are loaded into SBUF ONCE and
reused by every token tile.

Token tiles are [512, 384, 128]: the small final tile shrinks the serial
LN/SE tail. Main accumulation is split into two o-groups of 4 PSUM banks so
the LayerNorm/SE auxiliary matmuls (stats, broadcast outer products, SE) of
tile t can run in dedicated PSUM banks concurrently with tile t+1's main
accumulation; the aux work is emitted interleaved into tile t+1's chunk loop
for engine-stream overlap.

LayerNorm: per-token mean/rsqrt(var) stats via ones-matmuls (mean folded into
the ones scale), an int16 bit-hack rsqrt seed + one bf16 Newton step, then
y = out*(ones (x) z) + (ones (x) -mu*z), with ln_w/ln_b applied per-feature by
a 4x-rate tensor_scalar op (ln_w is also folded into the SE W1 on the host,
ln_b into the SE b1). SE: h = relu(W1'.y + b1'), se = sigmoid(W2.h + b2),
final = (ln_w*y + ln_b) * se, stored as bf16.
"""

import os
from contextlib import ExitStack

import numpy as np

P = 128
D = 1024
NC_I = 8           # contraction chunks of 128 over D_IN
NCH = 4            # rhs channels per i-chunk: x, bn[-1], bn[+1], sigma
NO = 8             # output-feature chunks of 128
NTOK = 1024        # tokens per core
N_CORES = 8
TILES = [(0, 512), (512, 384), (896, 128)]   # (tok0, T) per token tile
OGROUPS = [(0, 6), (6, 2)]                   # o-block groups (start, count)
GRID = [-1.0, 0.0, 1.0]
EPS_BASIS = 1e-6
LN_EPS = 1e-5
RSQRT_MAGIC32 = 0x5F3759DF   # fp32 rsqrt bit-hack seed

_cache = {}


def _build_nc():
    import concourse.bass as bass
    import concourse.mybir as mybir
    import concourse.tile as tile
    from concourse import bacc

    f32 = mybir.dt.float32
    bf16 = mybir.dt.bfloat16
    i32 = mybir.dt.int32
    AF = mybir.ActivationFunctionType
    OP = mybir.AluOpType
    ts = bass.ts

    nc = bacc.Bacc(
        "TRN2",
        target_bir_lowering=False,
        debug=False,
        enable_asserts=False,
        num_devices=N_CORES,
    )

    xt_d = nc.dram_tensor("xt", [NC_I, P, NTOK], bf16, kind="ExternalInput")
    w_d = nc.dram_tensor("w", [NC_I, P, NCH * D], bf16, kind="ExternalInput")
    w1t_d = nc.dram_tensor("w1t", [NO, P, 32], bf16, kind="ExternalInput")
    w2t_d = nc.dram_tensor("w2t", [32, D], bf16, kind="ExternalInput")
    lnw_d = nc.dram_tensor("lnw", [P, NO], f32, kind="ExternalInput")
    lnb_d = nc.dram_tensor("lnb", [P, NO], f32, kind="ExternalInput")
    onesc_d = nc.dram_tensor("onesc", [P, 1], bf16, kind="ExternalInput")  # 1/D
    onesp_d = nc.dram_tensor("onesp", [1, P], bf16, kind="ExternalInput")  # 1.0
    b1_d = nc.dram_tensor("b1", [32, 1], f32, kind="ExternalInput")
    b2_d = nc.dram_tensor("b2", [P, NO], f32, kind="ExternalInput")
    out_d = nc.dram_tensor("outT", [NO, P, NTOK], bf16, kind="ExternalOutput")

    with tile.TileContext(nc) as tc, ExitStack() as ctx:
        cp = ctx.enter_context(tc.tile_pool(name="cp", bufs=1))
        wp = ctx.enter_context(tc.tile_pool(name="wp", bufs=1))
        xp = ctx.enter_context(tc.tile_pool(name="xp", bufs=1))
        bp = ctx.enter_context(tc.tile_pool(name="bp", bufs=2))
        op_pool = ctx.enter_context(tc.tile_pool(name="op", bufs=2))
        sqp = ctx.enter_context(tc.tile_pool(name="sqp", bufs=2))
        stp = ctx.enter_context(tc.tile_pool(name="stp", bufs=2))
        yp = ctx.enter_context(tc.tile_pool(name="yp", bufs=1))
        sep = ctx.enter_context(tc.tile_pool(name="sep", bufs=2))
        finp = ctx.enter_context(tc.tile_pool(name="finp", bufs=1))
        # PSUM: 6 (main) + 1 (stats/broadcast, shared tag) + 1 (SE) = 8 banks
        mps = ctx.enter_context(tc.tile_pool(name="mps", bufs=6, space="PSUM"))
        auxps = ctx.enter_context(tc.tile_pool(name="auxps", bufs=1, space="PSUM"))
        seps = ctx.enter_context(tc.tile_pool(name="seps", bufs=1, space="PSUM"))

        # warm the sigmoid_and_others ACT table at t=0 (overlaps initial DMA)
        warm_t = cp.tile([P, 1], f32, tag="warm")
        nc.scalar.activation(warm_t[:], nc.const_aps.tensor(1.0, (P, 1)), AF.Relu)

        C = {}  # constant tiles, DMA'd after the first chunk's x/w (startup)

        def emit_consts():
            C["w1t"] = cp.tile([P, NO, 32], bf16, tag="w1t", name="w1t_t")
            nc.gpsimd.dma_start(C["w1t"][:], w1t_d.ap().rearrange("c p j -> p c j"))
            C["w2t"] = cp.tile([32, D], bf16, tag="w2t", name="w2t_t")
            nc.gpsimd.dma_start(C["w2t"][:], w2t_d.ap())
            C["lnw"] = cp.tile([P, NO], f32, tag="lnw", name="lnw_t")
            nc.gpsimd.dma_start(C["lnw"][:], lnw_d.ap())
            C["lnb"] = cp.tile([P, NO], f32, tag="lnb", name="lnb_t")
            nc.gpsimd.dma_start(C["lnb"][:], lnb_d.ap())
            C["onesc"] = cp.tile([P, 1], bf16, tag="onesc", name="onesc_t")
            nc.gpsimd.dma_start(C["onesc"][:], onesc_d.ap())
            C["onesp"] = cp.tile([1, P], bf16, tag="onesp", name="onesp_t")
            nc.gpsimd.dma_start(C["onesp"][:], onesp_d.ap())
            C["b1"] = cp.tile([32, 1], f32, tag="b1", name="b1_t")
            nc.gpsimd.dma_start(C["b1"][:], b1_d.ap())
            C["b2"] = cp.tile([P, NO], f32, tag="b2", name="b2_t")
            nc.gpsimd.dma_start(C["b2"][:], b2_d.ap())

        w_tiles = [None] * NC_I

        def emit_w_dma(c, chans):
            # one tile per channel so a matmul only waits on its own DMA
            w_src = w_d.ap()[c].rearrange("p (ch d) -> p ch d", ch=NCH)
            if w_tiles[c] is None:
                w_tiles[c] = [None] * NCH
            for ch in chans:
                w_t = wp.tile([P, D], bf16, tag=f"w{c}c{ch}")
                nc.sync.dma_start(w_t[:], w_src[:, ch])
                w_tiles[c][ch] = w_t

        def emit_basis(ti, c, T, tok0):
            """x DMA + normalized-basis channels for chunk c of tile ti."""
            x_t = xp.tile([P, T], bf16, tag=f"x{c}")
            nc.sync.dma_start(x_t[:], xt_d.ap()[c, :, tok0:tok0 + T])
            b = []
            for gi, g in enumerate(GRID):
                r_t = bp.tile([P, T], bf16, tag=f"r{gi}")
                sgn = -1.0 if g > 0 else 1.0
                nc.scalar.activation(r_t[:], x_t[:], AF.Abs, bias=abs(g), scale=sgn)
                nc.scalar.activation(r_t[:], r_t[:], AF.Relu, bias=1.0, scale=-1.0)
                b_t = bp.tile([P, T], bf16, tag=f"b{gi}")
                # squares: grid 0/+1 on gpsimd, grid -1 on DVE (engine balance)
                if gi == 0:
                    nc.vector.tensor_tensor(b_t[:], r_t[:], r_t[:], OP.mult)
                else:
                    nc.gpsimd.tensor_tensor(b_t[:], r_t[:], r_t[:], OP.mult)
                b.append(b_t)
            t_t = bp.tile([P, T], bf16, tag="tsum")
            nc.vector.tensor_tensor(t_t[:], b[0][:], b[1][:], OP.add)
            s_t = bp.tile([P, T], f32, tag="ssum")
            nc.vector.scalar_tensor_tensor(
                s_t[:], b[2][:], EPS_BASIS, t_t[:], OP.add, OP.add
            )
            inv_t = bp.tile([P, T], f32, tag="inv")
            nc.vector.reciprocal_approx_fast(out=inv_t[:], in_=s_t[:])
            inv16_t = bp.tile([P, T], bf16, tag="inv16")
            nc.vector.tensor_copy(out=inv16_t[:], in_=inv_t[:])
            bnm_t = xp.tile([P, T], bf16, tag=f"bnm{c}")
            nc.vector.tensor_tensor(bnm_t[:], b[0][:], inv16_t[:], OP.mult)
            bnp_t = xp.tile([P, T], bf16, tag=f"bnp{c}")
            nc.vector.tensor_tensor(bnp_t[:], b[2][:], inv16_t[:], OP.mult)
            sg_t = xp.tile([P, T], bf16, tag=f"sg{c}")
            nc.vector.tensor_scalar(
                sg_t[:], inv16_t[:], -EPS_BASIS, 1.0, OP.mult, OP.add
            )
            return [x_t, bnm_t, bnp_t, sg_t]

        def emit_group_mains(ps, rhs_by_chunk, c, o0, no, last, block_post,
                             chans=None):
            """16 matmuls for chunk c of one o-group; on the last chunk the
            emission is o-outer so per-block copies/stats can chase it."""
            rhs_list = rhs_by_chunk[c]
            w_t = w_tiles[c]
            if not last:
                for ch in chans if chans is not None else range(NCH):
                    rhs = rhs_list[ch][:]
                    for oi in range(no):
                        o = o0 + oi
                        nc.tensor.matmul(
                            ps[oi][:],
                            lhsT=w_t[ch][:, ts(o, P)],
                            rhs=rhs,
                            start=(c == 0 and ch == 0),
                            stop=False,
                        )
            else:
                chl = list(chans) if chans is not None else list(range(NCH))
                for oi in range(no):
                    o = o0 + oi
                    for ch in chl:
                        nc.tensor.matmul(
                            ps[oi][:],
                            lhsT=w_t[ch][:, ts(o, P)],
                            rhs=rhs_list[ch][:],
                            start=False,
                            stop=(ch == chl[-1]),
                        )
                    block_post(ps[oi], o)

        def emit_tile(ti, tok0, T, pending_aux, rhs_by_chunk, basis_next):
            """Emit one token tile's mains+stats; interleave prev tile's aux
            into group 0 and the NEXT tile's basis into group 1."""
            outs = [None] * NO
            sq = [None] * NO
            st = {}

            def get_psAB():
                # allocated lazily so the "aux" tag rotation matches runtime
                # order (after the previous tile's zb/mrb/psS allocations)
                if "psAB" not in st:
                    st["psAB"] = auxps.tile([33, T], f32, tag="aux",
                                            name=f"psAB_{ti}")
                return st["psAB"]

            def block_post(ps_ap, o):
                psAB = get_psAB()
                psA = psAB[0:1, :]
                psB = psAB[32:33, :]
                o_t = op_pool.tile([P, T], bf16, tag=f"out{o}", name=f"o_{ti}_{o}")
                nc.vector.tensor_copy(out=o_t[:], in_=ps_ap[:])
                outs[o] = o_t
                sq_t = sqp.tile([P, T], bf16, tag="sq")
                nc.scalar.activation(sq_t[:], ps_ap[:], AF.Square)
                sq[o] = sq_t
                nc.tensor.matmul(
                    psA, lhsT=C["onesc"][:], rhs=o_t[:],
                    start=(o == 0), stop=(o == NO - 1),
                )
                nc.tensor.matmul(
                    psB, lhsT=C["onesc"][:], rhs=sq_t[:],
                    start=(o == 0), stop=(o == NO - 1),
                )

            next_rhs = [None] * NC_I
            for gi, (o0, no) in enumerate(OGROUPS):
                ps = [
                    mps.tile([P, T], f32, tag="mps", name=f"ps_{ti}_{o0 + i}")
                    for i in range(no)
                ]
                if ti == 0 and gi == 0:
                    # startup: the x channel of every chunk only needs x + its
                    # weights, so run all 8 of those first while the basis
                    # pipeline fills; the bn channels follow in a second pass
                    for c in range(NC_I):
                        rhs_by_chunk[c] = emit_basis(ti, c, T, tok0)
                        emit_w_dma(c, [0])
                        emit_group_mains(ps, rhs_by_chunk, c, o0, no,
                                         last=False, block_post=block_post,
                                         chans=[0])
                        if c == 0:
                            emit_consts()
                    for c in range(NC_I):
                        emit_w_dma(c, [1, 2, 3])
                        emit_group_mains(ps, rhs_by_chunk, c, o0, no,
                                         last=(c == NC_I - 1),
                                         block_post=block_post, chans=[1, 2, 3])
                    continue
                for c in range(NC_I):
                    emit_group_mains(
                        ps, rhs_by_chunk, c, o0, no,
                        last=(c == NC_I - 1), block_post=block_post,
                    )
                    # interleave the previous tile's aux across both groups
                    if pending_aux is not None:
                        stage = pending_aux.get((gi, c))
                        if stage is not None:
                            stage()
                    # compute the NEXT tile's basis during group 1 (ACT/Pool/
                    # DVE are otherwise idle here)
                    if gi == 1 and basis_next is not None:
                        ntok0, nT = basis_next
                        next_rhs[c] = emit_basis(ti + 1, c, nT, ntok0)

            return {"T": T, "tok0": tok0, "ti": ti, "psAB": st["psAB"],
                    "outs": outs, "sq": sq, "next_rhs": next_rhs}

        def make_aux(tile_st):
            """Aux stage emitters for a completed tile: stats chain -> LN -> SE."""
            T = tile_st["T"]
            ti = tile_st["ti"]
            tok0 = tile_st["tok0"]
            outs = tile_st["outs"]
            psAB = tile_st["psAB"]
            ctx_st = {}

            def chain():
                # negmu/e2 rows from psum, fp32 bit-hack rsqrt + 1 Newton step
                negmu = stp.tile([1, T], f32, tag="negmu")
                nc.vector.tensor_scalar(
                    negmu[:], psAB[0:1, :], -1.0, 0.0, OP.mult, OP.add
                )
                e2 = stp.tile([1, T], f32, tag="e2")
                nc.vector.tensor_scalar(
                    e2[:], psAB[32:33, :], 1.0, LN_EPS, OP.mult, OP.add
                )
                mu2 = stp.tile([1, T], f32, tag="mu2")
                nc.vector.tensor_tensor(mu2[:], negmu[:], negmu[:], OP.mult)
                var = stp.tile([1, T], f32, tag="var")
                nc.vector.tensor_tensor(var[:], e2[:], mu2[:], OP.subtract)
                zw = stp.tile([1, T], f32, tag="zw")
                nc.vector.tensor_scalar(
                    zw[:].bitcast(i32), var[:].bitcast(i32), 1, 0,
                    OP.arith_shift_right,
                )
                nc.vector.tensor_scalar(
                    zw[:].bitcast(i32), zw[:].bitcast(i32), -1, RSQRT_MAGIC32,
                    OP.mult, OP.add,
                )
                t1 = stp.tile([1, T], f32, tag="t1")
                nc.vector.tensor_tensor(t1[:], zw[:], zw[:], OP.mult)
                nc.vector.tensor_tensor(t1[:], t1[:], var[:], OP.mult)
                nc.vector.tensor_scalar(t1[:], t1[:], -0.5, 1.5, OP.mult, OP.add)
                z16 = stp.tile([1, T], bf16, tag="z16")
                nc.vector.tensor_tensor(z16[:], zw[:], t1[:], OP.mult)
                mr16 = stp.tile([1, T], bf16, tag="mr16")
                nc.vector.tensor_tensor(mr16[:], negmu[:], z16[:], OP.mult)
                # broadcast rows across partitions via K=1 outer products
                zbp = auxps.tile([P, T], f32, tag="aux", name=f"zb_{ti}")
                nc.tensor.matmul(zbp[:], lhsT=C["onesp"][:], rhs=z16[:],
                                 start=True, stop=True)
                zb16 = stp.tile([P, T], bf16, tag="zb16")
                nc.scalar.activation(zb16[:], zbp[:], AF.Copy)
                mrp = auxps.tile([P, T], f32, tag="aux", name=f"mrb_{ti}")
                nc.tensor.matmul(mrp[:], lhsT=C["onesp"][:], rhs=mr16[:],
                                 start=True, stop=True)
                mrb16 = stp.tile([P, T], bf16, tag="mrb16")
                nc.scalar.activation(mrb16[:], mrp[:], AF.Copy)
                ctx_st["zb16"] = zb16
                ctx_st["mrb16"] = mrb16

            def ln():
                zb16, mrb16 = ctx_st["zb16"], ctx_st["mrb16"]
                psH = seps.tile([32, T], f32, tag="sps", name=f"psH_{ti}")
                yhat = []
                for o in range(NO):
                    q_t = sqp.tile([P, T], bf16, tag="q")
                    nc.vector.tensor_tensor(q_t[:], outs[o][:], zb16[:], OP.mult)
                    yh_t = yp.tile([P, T], bf16, tag=f"yh{o}")
                    nc.vector.tensor_tensor(yh_t[:], q_t[:], mrb16[:], OP.add)
                    yhat.append(yh_t)
                    nc.tensor.matmul(
                        psH[:], lhsT=C["w1t"][:, o, :], rhs=yh_t[:],
                        start=(o == 0), stop=(o == NO - 1),
                    )
                hr = sep.tile([32, T], bf16, tag="hr")
                nc.scalar.activation(hr[:], psH[:], AF.Relu, bias=C["b1"][:], scale=1.0)
                ctx_st["yhat"] = yhat
                ctx_st["hr"] = hr

            def se_blocks(olist):
                yhat, hr = ctx_st["yhat"], ctx_st["hr"]
                if "fin" not in ctx_st:
                    # one wide tile so all 8 out blocks leave in a single DMA
                    # (the descriptor engine processes DMAs serially)
                    ctx_st["fin"] = finp.tile([P, NO, T], bf16, tag="fin",
                                             name=f"fin_{ti}")
                fin_t = ctx_st["fin"]
                for o in olist:
                    # alternate psS between the two non-main PSUM banks so the
                    # W2 matmul of block o+1 overlaps the sigmoid of block o
                    pool, tg = (seps, "sps") if o % 2 else (auxps, "aux")
                    psS = pool.tile([P, T], f32, tag=tg, name=f"psS_{ti}_{o}")
                    nc.tensor.matmul(
                        psS[:], lhsT=C["w2t"][:, ts(o, P)], rhs=hr[:],
                        start=True, stop=True,
                    )
                    se_t = sep.tile([P, T], bf16, tag="se")
                    nc.scalar.activation(
                        se_t[:], psS[:], AF.Sigmoid, bias=C["b2"][:, o:o + 1], scale=1.0
                    )
                    yf_t = sep.tile([P, T], bf16, tag="yf")
                    nc.vector.tensor_scalar(
                        yf_t[:], yhat[o][:], C["lnw"][:, o:o + 1], C["lnb"][:, o:o + 1],
                        OP.mult, OP.add,
                    )
                    nc.vector.tensor_tensor(fin_t[:, o], yf_t[:], se_t[:], OP.mult)
                if olist[-1] == NO - 1:
                    nc.sync.dma_start(
                        out_d.ap()[:, :, tok0:tok0 + T].rearrange("o p t -> p o t"),
                        fin_t[:],
                    )

            return {"chain": chain, "ln": ln,
                    "se_a": lambda: se_blocks(range(0, 4)),
                    "se_b": lambda: se_blocks(range(4, NO))}

        AUX_SLOTS = {"chain": (0, 0), "ln": (0, 2), "se_a": (0, 4), "se_b": (1, 1)}

        pending = None
        rhs_cur = [None] * NC_I
        for ti, (tok0, T) in enumerate(TILES):
            basis_next = TILES[ti + 1] if ti + 1 < len(TILES) else None
            aux_by_slot = (
                {slot: pending[name] for name, slot in AUX_SLOTS.items()}
                if pending else None
            )
            tile_st = emit_tile(ti, tok0, T, aux_by_slot, rhs_cur, basis_next)
            rhs_cur = tile_st["next_rhs"]
            pending = make_aux(tile_st)
        # final tile's aux runs at the end (smallest tile -> short tail)
        pending["chain"]()
        pending["ln"]()
        pending["se_a"]()
        pending["se_b"]()

    nc.compile()
    return nc


def _get_nc():
    if "nc" not in _cache:
        _cache["nc"] = _build_nc()
    return _cache["nc"]


def _prep_host(inputs):
    import ml_dtypes

    f = np.float32
    bf = ml_dtypes.bfloat16
    x = np.asarray(inputs["x"], f)
    base_weight = np.asarray(inputs["base_weight"], f)
    spline_weight = np.asarray(inputs["spline_weight"], f)
    ln_w = np.asarray(inputs["ln_w"], f)
    ln_b = np.asarray(inputs["ln_b"], f)
    se_w1 = np.asarray(inputs["se_w1"], f)
    se_b1 = np.asarray(inputs["se_b1"], f)
    se_w2 = np.asarray(inputs["se_w2"], f)
    se_b2 = np.asarray(inputs["se_b2"], f)

    xt_all = x.reshape(N_CORES, NTOK, D).transpose(0, 2, 1)  # [core, D, ntok]

    w_all = np.empty((NC_I, P, NCH, D), f)
    w_all[:, :, 0, :] = base_weight.T.reshape(NC_I, P, D)
    wsT = spline_weight.transpose(1, 2, 0)  # [i, g, o]
    # sigma trick: bn_0 = sigma - bn_-1 - bn_+1
    w_all[:, :, 1, :] = (wsT[:, 0, :] - wsT[:, 1, :]).reshape(NC_I, P, D)
    w_all[:, :, 2, :] = (wsT[:, 2, :] - wsT[:, 1, :]).reshape(NC_I, P, D)
    w_all[:, :, 3, :] = wsT[:, 1, :].reshape(NC_I, P, D)
    w_all = np.ascontiguousarray(w_all.reshape(NC_I, P, NCH * D)).astype(bf)

    w1p = (se_w1 * ln_w[None, :]).astype(f)          # ln_w folded into W1
    b1p = (se_b1 + se_w1 @ ln_b).astype(f)           # ln_b folded into b1

    shared = {
        "w": w_all,
        "w1t": np.ascontiguousarray(w1p.T.reshape(NO, P, 32)).astype(bf),
        "w2t": np.ascontiguousarray(se_w2.T).astype(bf),
        "lnw": np.ascontiguousarray(ln_w.reshape(NO, P).T),
        "lnb": np.ascontiguousarray(ln_b.reshape(NO, P).T),
        "onesc": np.full((P, 1), 1.0 / D, bf),
        "onesp": np.ones((1, P), bf),
        "b1": np.ascontiguousarray(b1p.reshape(32, 1)),
        "b2": np.ascontiguousarray(se_b2.reshape(NO, P).T),
    }
    in_maps = []
    for k in range(N_CORES):
        m = dict(shared)
        m["xt"] = np.ascontiguousarray(
            xt_all[k].reshape(NC_I, P, NTOK)
        ).astype(bf)
        in_maps.append(m)
    return in_maps


def kernel(**inputs) -> np.ndarray:
    from concourse.bass_utils import run_bass_kernel_spmd

    nc = _get_nc()
    in_maps = _prep_host(inputs)
    trace = bool(int(os.environ.get("KERNEL_TRACE", "0")))
    res = run_bass_kernel_spmd(
        nc, in_maps, core_ids=list(range(N_CORES)), trace=trace
    )
    _cache["last_result"] = res
    outs = []
    for k in range(N_CORES):
        outT = np.asarray(res.results[k]["outT"], dtype=np.float32)  # [NO, P, NTOK]
        outs.append(outT.reshape(D, NTOK).T)                          # [ntok, o]
    out = np.concatenate(outs, axis=0).reshape(8, 1024, 1024)
    return np.ascontiguousarray(out.astype(np.float32))


# revision 54
# speedup vs baseline: 1.2533x; 1.0771x over previous
"""Trainium2 Bass kernel for nn_EnhancedDRKANTreeNet (KAN layer + LayerNorm + SE gate).

Data-parallel over 8192 tokens across 8 NeuronCores (1024 tokens/core), all
compute feature-major: tiles are [feature_partition, token].

  out^T[o, n] = sum_i x^T[i, n]*Wb[o, i] + sum_{i,g} bn_g[i, n]*Ws[o, i, g]

All matmul operands are bf16 (1 cycle/row on the PE, same as fp32r, but half
the HBM traffic and 2x DVE throughput on elementwise tiles); PSUM accumulates
fp32. The sigma trick folds the bn_0 spline channel into [x, bn_-1, bn_+1,
sigma] with host-combined weights. Weights are loaded into SBUF ONCE and
reused by every token tile.

Token tiles are [512, 384, 128]: the small final tile shrinks the serial
LN/SE tail. Main accumulation is split into two o-groups of 4 PSUM banks so
the LayerNorm/SE auxiliary matmuls (stats, broadcast outer products, SE) of
tile t can run in dedicated PSUM banks concurrently with tile t+1's main
accumulation; the aux work is emitted interleaved into tile t+1's chunk loop
for engine-stream overlap.

LayerNorm: per-token mean/rsqrt(var) stats via ones-matmuls (mean folded into
the ones scale), an int16 bit-hack rsqrt seed + one bf16 Newton step, then
y = out*(ones (x) z) + (ones (x) -mu*z), with ln_w/ln_b applied per-feature by
a 4x-rate tensor_scalar op (ln_w is also folded into the SE W1 on the host,
ln_b into the SE b1). SE: h = relu(W1'.y + b1'), se = sigmoid(W2.h + b2),
final = (ln_w*y + ln_b) * se, stored as bf16.
"""

import os
from contextlib import ExitStack

import numpy as np

P = 128
D = 1024
NC_I = 8           # contraction chunks of 128 over D_IN
NCH = 4            # rhs channels per i-chunk: x, bn[-1], bn[+1], sigma
NPAIR = 4          # chunk pairs for fp8 DoubleRow spline matmuls
NSCH = 6           # spline DR channels: (bnm, bnp, sg) x (w_hi, w_lo)
NO = 8             # output-feature chunks of 128
NTOK = 1024        # tokens per core
N_CORES = 8
TILES = [(0, 512), (512, 384), (896, 128)]   # (tok0, T) per token tile
OGROUPS = [(0, 6), (6, 2)]                   # o-block groups (start, count)
GRID = [-1.0, 0.0, 1.0]
EPS_BASIS = 1e-6
KSLOPE = 4400.0    # step-channel sigmoid slope (matches the eps=1e-6
                   # rational transition width sqrt(eps)~1e-3 at |x|=2)
LN_EPS = 1e-5
RSQRT_MAGIC32 = 0x5F3759DF   # fp32 rsqrt bit-hack seed

_cache = {}


def _build_nc():
    import concourse.bass as bass
    import concourse.mybir as mybir
    import concourse.tile as tile
    from concourse import bacc

    f32 = mybir.dt.float32
    bf16 = mybir.dt.bfloat16
    f8e4 = mybir.dt.float8e4
    PM = mybir.MatmulPerfMode
    i32 = mybir.dt.int32
    AF = mybir.ActivationFunctionType
    OP = mybir.AluOpType
    ts = bass.ts

    nc = bacc.Bacc(
        "TRN2",
        target_bir_lowering=False,
        debug=False,
        enable_asserts=False,
        num_devices=N_CORES,
    )

    xt_d = nc.dram_tensor("xt", [NC_I, P, NTOK], bf16, kind="ExternalInput")
    xt32_d = nc.dram_tensor("xt32", [NC_I, P, NTOK], f32, kind="ExternalInput")
    c0_d = nc.dram_tensor("c0", [P, NO], f32, kind="ExternalInput")
    w_d = nc.dram_tensor("w", [NC_I, P, D], bf16, kind="ExternalInput")
    w8_d = nc.dram_tensor("w8", [NPAIR, P, NSCH * 2 * D], f8e4,
                          kind="ExternalInput")
    w1t_d = nc.dram_tensor("w1t", [NO, P, 32], bf16, kind="ExternalInput")
    w2t_d = nc.dram_tensor("w2t", [32, D], bf16, kind="ExternalInput")
    lnw_d = nc.dram_tensor("lnw", [P, NO], f32, kind="ExternalInput")
    lnb_d = nc.dram_tensor("lnb", [P, NO], f32, kind="ExternalInput")
    onesc_d = nc.dram_tensor("onesc", [P, 1], bf16, kind="ExternalInput")  # 1/D
    onesp_d = nc.dram_tensor("onesp", [1, P], bf16, kind="ExternalInput")  # 1.0
    b1_d = nc.dram_tensor("b1", [32, 1], f32, kind="ExternalInput")
    b2_d = nc.dram_tensor("b2", [P, NO], f32, kind="ExternalInput")
    out_d = nc.dram_tensor("outT", [NO, P, NTOK], bf16, kind="ExternalOutput")

    with tile.TileContext(nc) as tc, ExitStack() as ctx:
        cp = ctx.enter_context(tc.tile_pool(name="cp", bufs=1))
        wp = ctx.enter_context(tc.tile_pool(name="wp", bufs=1))
        xp = ctx.enter_context(tc.tile_pool(name="xp", bufs=1))
        bn8p = ctx.enter_context(tc.tile_pool(name="bn8p", bufs=1))
        bp = ctx.enter_context(tc.tile_pool(name="bp", bufs=2))
        op_pool = ctx.enter_context(tc.tile_pool(name="op", bufs=2))
        sqp = ctx.enter_context(tc.tile_pool(name="sqp", bufs=2))
        x32p = ctx.enter_context(tc.tile_pool(name="x32p", bufs=2))
        stp = ctx.enter_context(tc.tile_pool(name="stp", bufs=2))
        yp = ctx.enter_context(tc.tile_pool(name="yp", bufs=1))
        sep = ctx.enter_context(tc.tile_pool(name="sep", bufs=2))
        finp = ctx.enter_context(tc.tile_pool(name="finp", bufs=1))
        # PSUM: 6 (main) + 1 (stats/broadcast, shared tag) + 1 (SE) = 8 banks
        mps = ctx.enter_context(tc.tile_pool(name="mps", bufs=6, space="PSUM"))
        auxps = ctx.enter_context(tc.tile_pool(name="auxps", bufs=1, space="PSUM"))
        seps = ctx.enter_context(tc.tile_pool(name="seps", bufs=1, space="PSUM"))

        # warm the sigmoid_and_others ACT table at t=0 (overlaps initial DMA)
        warm_t = cp.tile([P, 1], f32, tag="warm")
        nc.scalar.activation(warm_t[:], nc.const_aps.tensor(1.0, (P, 1)), AF.Relu)

        bk_t = cp.tile([P, 1], f32, tag="bk")
        nc.gpsimd.memset(bk_t[:], -2.0 * KSLOPE)
        half_t = cp.tile([P, 512], bf16, tag="half")
        nc.gpsimd.memset(half_t[:], 0.5)

        C = {}  # constant tiles, DMA'd after the first chunk's x/w (startup)

        def emit_consts():
            C["w1t"] = cp.tile([P, NO, 32], bf16, tag="w1t", name="w1t_t")
            nc.gpsimd.dma_start(C["w1t"][:], w1t_d.ap().rearrange("c p j -> p c j"))
            C["w2t"] = cp.tile([32, D], bf16, tag="w2t", name="w2t_t")
            nc.gpsimd.dma_start(C["w2t"][:], w2t_d.ap())
            C["lnw"] = cp.tile([P, NO], f32, tag="lnw", name="lnw_t")
            nc.gpsimd.dma_start(C["lnw"][:], lnw_d.ap())
            C["lnb"] = cp.tile([P, NO], f32, tag="lnb", name="lnb_t")
            nc.gpsimd.dma_start(C["lnb"][:], lnb_d.ap())
            C["onesc"] = cp.tile([P, 1], bf16, tag="onesc", name="onesc_t")
            nc.gpsimd.dma_start(C["onesc"][:], onesc_d.ap())
            C["onesp"] = cp.tile([1, P], bf16, tag="onesp", name="onesp_t")
            nc.gpsimd.dma_start(C["onesp"][:], onesp_d.ap())
            C["b1"] = cp.tile([32, 1], f32, tag="b1", name="b1_t")
            nc.gpsimd.dma_start(C["b1"][:], b1_d.ap())
            C["b2"] = cp.tile([P, NO], f32, tag="b2", name="b2_t")
            nc.gpsimd.dma_start(C["b2"][:], b2_d.ap())
            C["c0"] = cp.tile([P, NO], f32, tag="c0", name="c0_t")
            nc.gpsimd.dma_start(C["c0"][:], c0_d.ap())

        w_tiles = [None] * NC_I
        w8_tiles = [None] * NPAIR

        def emit_wb_dma(c):
            w_t = wp.tile([P, D], bf16, tag=f"w{c}")
            nc.sync.dma_start(w_t[:], w_d.ap()[c])
            w_tiles[c] = w_t

        def emit_w8_dma(pair):
            w8_src = w8_d.ap()[pair].rearrange(
                "p (c8 j d) -> p c8 j d", c8=NSCH, j=2
            )
            tiles = []
            for c8 in range(NSCH):
                w8_t = wp.tile([P, 2, D], f8e4, tag=f"w8p{pair}c{c8}")
                nc.sync.dma_start(w8_t[:], w8_src[:, c8])
                tiles.append(w8_t)
            w8_tiles[pair] = tiles

        def emit_basis(ti, c, T, tok0, bn8_by_pair):
            """Basis channels for chunk c via the h/step reformulation.

            bn_p = h(clamp(x,0,1)) - step(x>2), bn_m = h(clamp(-x,0,1)) -
            step(x<-2), sigma = 1 - step+ - step-, with h(c) = c^2/(c^2 +
            (1-c)^2) = 0.5 + u/(2u^2+0.5), u = c-0.5. The 0.5 offsets and the
            W0 channel fold into a per-feature constant (c0) applied at the
            PSUM descale. v = u/(2u^2+0.5) and the steps are written as fp8
            into slot c%2 of the chunk-pair tiles for DoubleRow matmuls. The
            steps use the fp32 copy of x (the bf16 ulp at |x|=2 is 16x wider
            than the eps-rational transition band being approximated).
            """
            pair, j = c // 2, c % 2
            x_t = xp.tile([P, T], bf16, tag=f"x{c}")
            nc.sync.dma_start(x_t[:], xt_d.ap()[c, :, tok0:tok0 + T])
            x32_t = x32p.tile([P, T], f32, tag="x32")
            nc.sync.dma_start(x32_t[:], xt32_d.ap()[c, :, tok0:tok0 + T])
            if j == 0:
                bn8_by_pair[pair] = [
                    bn8p.tile([P, 2, T], f8e4, tag=f"bn8{k}{pair}",
                              name=f"bn8{k}{pair}_{ti}")
                    for k in ("vd", "vc", "km", "kp")
                ]
            vd8, vc8, km8, kp8 = bn8_by_pair[pair]
            uc_t = bp.tile([P, T], bf16, tag="uc")
            nc.vector.tensor_scalar(uc_t[:], x_t[:], 1.0, 0.0, OP.min, OP.max)
            nc.vector.tensor_scalar(uc_t[:], uc_t[:], -0.5, None, OP.add)
            ud_t = bp.tile([P, T], bf16, tag="ud")
            nc.vector.tensor_scalar(ud_t[:], x_t[:], 0.0, -1.0, OP.min, OP.mult)
            nc.vector.tensor_scalar(ud_t[:], ud_t[:], 1.0, -0.5, OP.min, OP.add)
            u2c = bp.tile([P, T], bf16, tag="u2c")
            nc.scalar.activation(u2c[:], uc_t[:], AF.Square, scale=2.0 ** 0.5)
            u2d = bp.tile([P, T], bf16, tag="u2d")
            nc.scalar.activation(u2d[:], ud_t[:], AF.Square, scale=2.0 ** 0.5)
            shc = bp.tile([P, T], f32, tag="shc")
            nc.gpsimd.tensor_tensor(shc[:], u2c[:], half_t[:, :T], OP.add)
            shd = bp.tile([P, T], f32, tag="shd")
            nc.gpsimd.tensor_tensor(shd[:], u2d[:], half_t[:, :T], OP.add)
            invc = bp.tile([P, T], f32, tag="invc")
            nc.vector.reciprocal_approx_fast(out=invc[:], in_=shc[:])
            invd = bp.tile([P, T], f32, tag="invd")
            nc.vector.reciprocal_approx_fast(out=invd[:], in_=shd[:])
            nc.gpsimd.tensor_tensor(vc8[:, j], uc_t[:], invc[:], OP.mult)
            nc.gpsimd.tensor_tensor(vd8[:, j], ud_t[:], invd[:], OP.mult)
            nc.scalar.activation(kp8[:, j], x32_t[:], AF.Sigmoid, bias=bk_t[:],
                                 scale=KSLOPE)
            nc.scalar.activation(km8[:, j], x32_t[:], AF.Sigmoid, bias=bk_t[:],
                                 scale=-KSLOPE)
            return x_t

        def emit_base_mains(ps, x_t, c, o0, no):
            """bf16 base-channel matmuls (x * 512Wb) for one chunk."""
            for oi in range(no):
                o = o0 + oi
                nc.tensor.matmul(
                    ps[oi][:],
                    lhsT=w_tiles[c][:, ts(o, P)],
                    rhs=x_t[:],
                    start=(c == 0),
                    stop=False,
                )

        def emit_spline_mains(ps, bn8, pair, o0, no, last, block_post, psl=None):
            """fp8 DoubleRow spline matmuls for one chunk pair (6 channels:
            3 bn tensors x hi/lo weights). On the final pair the emission is
            o-outer with per-block copies/stats chasing the stop."""
            w8 = w8_tiles[pair]
            # channels: vd*(Wm'-hi/lo), vc*(Wp'-hi/lo), km*(-Wm), kp*(-Wp)
            rhs = [bn8[0], bn8[0], bn8[1], bn8[1], bn8[2], bn8[3]]
            if not last:
                for c8 in range(NSCH):
                    for oi in range(no):
                        o = o0 + oi
                        nc.tensor.matmul(
                            ps[oi][:],
                            lhsT=w8[c8][:, :, ts(o, P)],
                            rhs=rhs[c8][:],
                            start=False,
                            stop=False,
                            perf_mode=PM.DoubleRow,
                        )
            else:
                for oi in range(no):
                    o = o0 + oi
                    for c8 in range(NSCH):
                        nc.tensor.matmul(
                            ps[oi][:],
                            lhsT=w8[c8][:, :, ts(o, P)],
                            rhs=rhs[c8][:],
                            start=False,
                            stop=(c8 == NSCH - 1),
                            perf_mode=PM.DoubleRow,
                        )
                    block_post(ps[oi], o)

        def emit_tile(ti, tok0, T, pending_aux, rhs_by_chunk, bn8_by_pair,
                      basis_next):
            """Emit one token tile's mains+stats; interleave prev tile's aux
            into group 0 and the NEXT tile's basis into group 1."""
            outs = [None] * NO
            sq = [None] * NO
            st = {}

            def get_psAB():
                # allocated lazily so the "aux" tag rotation matches runtime
                # order (after the previous tile's zb/mrb/psS allocations)
                if "psAB" not in st:
                    st["psAB"] = auxps.tile([33, T], f32, tag="aux",
                                            name=f"psAB_{ti}")
                return st["psAB"]

            def block_post(ps_ap, o):
                psAB = get_psAB()
                psA = psAB[0:1, :]
                psB = psAB[32:33, :]
                o_t = op_pool.tile([P, T], bf16, tag=f"out{o}", name=f"o_{ti}_{o}")
                nc.vector.tensor_scalar(o_t[:], ps_ap[:], 1.0 / 512.0,
                                        C["c0"][:, o:o + 1], OP.mult, OP.add)
                outs[o] = o_t
                sq_t = sqp.tile([P, T], bf16, tag="sq")
                nc.scalar.activation(sq_t[:], ps_ap[:], AF.Square,
                                     bias=C["c0"][:, o:o + 1],
                                     scale=1.0 / 512.0)
                sq[o] = sq_t
                nc.tensor.matmul(
                    psA, lhsT=C["onesc"][:], rhs=o_t[:],
                    start=(o == 0), stop=(o == NO - 1),
                )
                nc.tensor.matmul(
                    psB, lhsT=C["onesc"][:], rhs=sq_t[:],
                    start=(o == 0), stop=(o == NO - 1),
                )

            next_rhs = [None] * NC_I
            next_bn8 = [None] * NPAIR
            for gi, (o0, no) in enumerate(OGROUPS):
                ps = [
                    mps.tile([P, T], f32, tag="mps", name=f"ps_{ti}_{o0 + i}")
                    for i in range(no)
                ]
                if ti == 0 and gi == 0:
                    # startup: the base channel of every chunk only needs x +
                    # its weights, so run all 8 of those first while the basis
                    # pipeline fills; the spline pairs follow in a second pass
                    for c in range(NC_I):
                        rhs_by_chunk[c] = emit_basis(ti, c, T, tok0,
                                                     bn8_by_pair)
                        emit_wb_dma(c)
                        emit_base_mains(ps, rhs_by_chunk[c], c, o0, no)
                        if c == 0:
                            emit_consts()
                    for pair in range(NPAIR):
                        emit_w8_dma(pair)
                        emit_spline_mains(ps, bn8_by_pair[pair], pair, o0, no,
                                          last=(pair == NPAIR - 1),
                                          block_post=block_post)
                    continue
                for c in range(NC_I):
                    emit_base_mains(ps, rhs_by_chunk[c], c, o0, no)
                    if c % 2 == 1:
                        emit_spline_mains(ps, bn8_by_pair[c // 2], c // 2,
                                          o0, no, last=(c == NC_I - 1),
                                          block_post=block_post)
                    # interleave the previous tile's aux across both groups
                    if pending_aux is not None:
                        stage = pending_aux.get((gi, c))
                        if stage is not None:
                            stage()
                    # compute the NEXT tile's basis during group 1 (ACT/Pool/
                    # DVE are otherwise idle here); shifted one chunk so the
                    # bufs=1 bn8 pair tiles are already past their last reader
                    if gi == 1 and basis_next is not None and c >= 1:
                        ntok0, nT = basis_next
                        next_rhs[c - 1] = emit_basis(ti + 1, c - 1, nT, ntok0,
                                                     next_bn8)
                        if c == NC_I - 1:
                            next_rhs[c] = emit_basis(ti + 1, c, nT, ntok0,
                                                     next_bn8)

            return {"T": T, "tok0": tok0, "ti": ti, "psAB": st["psAB"],
                    "outs": outs, "sq": sq, "next_rhs": next_rhs,
                    "next_bn8": next_bn8}

        def make_aux(tile_st):
            """Aux stage emitters for a completed tile: stats chain -> LN -> SE."""
            T = tile_st["T"]
            ti = tile_st["ti"]
            tok0 = tile_st["tok0"]
            outs = tile_st["outs"]
            psAB = tile_st["psAB"]
            ctx_st = {}

            def chain():
                # negmu/e2 rows from psum, fp32 bit-hack rsqrt + 1 Newton step
                negmu = stp.tile([1, T], f32, tag="negmu")
                nc.vector.tensor_scalar(
                    negmu[:], psAB[0:1, :], -1.0, 0.0, OP.mult, OP.add
                )
                e2 = stp.tile([1, T], f32, tag="e2")
                nc.vector.tensor_scalar(
                    e2[:], psAB[32:33, :], 1.0, LN_EPS, OP.mult, OP.add
                )
                mu2 = stp.tile([1, T], f32, tag="mu2")
                nc.vector.tensor_tensor(mu2[:], negmu[:], negmu[:], OP.mult)
                var = stp.tile([1, T], f32, tag="var")
                nc.vector.tensor_tensor(var[:], e2[:], mu2[:], OP.subtract)
                zw = stp.tile([1, T], f32, tag="zw")
                nc.vector.tensor_scalar(
                    zw[:].bitcast(i32), var[:].bitcast(i32), 1, 0,
                    OP.arith_shift_right,
                )
                nc.vector.tensor_scalar(
                    zw[:].bitcast(i32), zw[:].bitcast(i32), -1, RSQRT_MAGIC32,
                    OP.mult, OP.add,
                )
                t1 = stp.tile([1, T], f32, tag="t1")
                nc.vector.tensor_tensor(t1[:], zw[:], zw[:], OP.mult)
                nc.vector.tensor_tensor(t1[:], t1[:], var[:], OP.mult)
                nc.vector.tensor_scalar(t1[:], t1[:], -0.5, 1.5, OP.mult, OP.add)
                z16 = stp.tile([1, T], bf16, tag="z16")
                nc.vector.tensor_tensor(z16[:], zw[:], t1[:], OP.mult)
                mr16 = stp.tile([1, T], bf16, tag="mr16")
                nc.vector.tensor_tensor(mr16[:], negmu[:], z16[:], OP.mult)
                # broadcast rows across partitions via K=1 outer products
                zbp = auxps.tile([P, T], f32, tag="aux", name=f"zb_{ti}")
                nc.tensor.matmul(zbp[:], lhsT=C["onesp"][:], rhs=z16[:],
                                 start=True, stop=True)
                zb16 = stp.tile([P, T], bf16, tag="zb16")
                nc.scalar.activation(zb16[:], zbp[:], AF.Copy)
                mrp = auxps.tile([P, T], f32, tag="aux", name=f"mrb_{ti}")
                nc.tensor.matmul(mrp[:], lhsT=C["onesp"][:], rhs=mr16[:],
                                 start=True, stop=True)
                mrb16 = stp.tile([P, T], bf16, tag="mrb16")
                nc.scalar.activation(mrb16[:], mrp[:], AF.Copy)
                ctx_st["zb16"] = zb16
                ctx_st["mrb16"] = mrb16

            def ln():
                zb16, mrb16 = ctx_st["zb16"], ctx_st["mrb16"]
                psH = seps.tile([32, T], f32, tag="sps", name=f"psH_{ti}")
                yhat = []
                for o in range(NO):
                    q_t = sqp.tile([P, T], bf16, tag="q")
                    nc.vector.tensor_tensor(q_t[:], outs[o][:], zb16[:], OP.mult)
                    yh_t = yp.tile([P, T], bf16, tag=f"yh{o}")
                    nc.vector.tensor_tensor(yh_t[:], q_t[:], mrb16[:], OP.add)
                    yhat.append(yh_t)
                    nc.tensor.matmul(
                        psH[:], lhsT=C["w1t"][:, o, :], rhs=yh_t[:],
                        start=(o == 0), stop=(o == NO - 1),
                    )
                hr = sep.tile([32, T], bf16, tag="hr")
                nc.scalar.activation(hr[:], psH[:], AF.Relu, bias=C["b1"][:], scale=1.0)
                ctx_st["yhat"] = yhat
                ctx_st["hr"] = hr

            def se_blocks(olist):
                yhat, hr = ctx_st["yhat"], ctx_st["hr"]
                if "fin" not in ctx_st:
                    # one wide tile so all 8 out blocks leave in a single DMA
                    # (the descriptor engine processes DMAs serially)
                    ctx_st["fin"] = finp.tile([P, NO, T], bf16, tag="fin",
                                             name=f"fin_{ti}")
                fin_t = ctx_st["fin"]
                for o in olist:
                    # alternate psS between the two non-main PSUM banks so the
                    # W2 matmul of block o+1 overlaps the sigmoid of block o
                    pool, tg = (seps, "sps") if o % 2 else (auxps, "aux")
                    psS = pool.tile([P, T], f32, tag=tg, name=f"psS_{ti}_{o}")
                    nc.tensor.matmul(
                        psS[:], lhsT=C["w2t"][:, ts(o, P)], rhs=hr[:],
                        start=True, stop=True,
                    )
                    se_t = sep.tile([P, T], bf16, tag="se")
                    nc.scalar.activation(
                        se_t[:], psS[:], AF.Sigmoid, bias=C["b2"][:, o:o + 1], scale=1.0
                    )
                    yf_t = sep.tile([P, T], bf16, tag="yf")
                    nc.vector.tensor_scalar(
                        yf_t[:], yhat[o][:], C["lnw"][:, o:o + 1], C["lnb"][:, o:o + 1],
                        OP.mult, OP.add,
                    )
                    nc.vector.tensor_tensor(fin_t[:, o], yf_t[:], se_t[:], OP.mult)
                if olist[-1] == NO - 1:
                    nc.sync.dma_start(
                        out_d.ap()[:, :, tok0:tok0 + T].rearrange("o p t -> p o t"),
                        fin_t[:],
                    )

            return {"chain": chain, "ln": ln,
                    "se_a": lambda: se_blocks(range(0, 4)),
                    "se_b": lambda: se_blocks(range(4, NO))}

        AUX_SLOTS = {"chain": (0, 0), "ln": (0, 2), "se_a": (0, 4), "se_b": (1, 1)}

        pending = None
        rhs_cur = [None] * NC_I
        bn8_cur = [None] * NPAIR
        for ti, (tok0, T) in enumerate(TILES):
            basis_next = TILES[ti + 1] if ti + 1 < len(TILES) else None
            aux_by_slot = (
                {slot: pending[name] for name, slot in AUX_SLOTS.items()}
                if pending else None
            )
            tile_st = emit_tile(ti, tok0, T, aux_by_slot, rhs_cur, bn8_cur,
                                basis_next)
            rhs_cur = tile_st["next_rhs"]
            bn8_cur = tile_st["next_bn8"]
            pending = make_aux(tile_st)
        # final tile's aux runs at the end (smallest tile -> short tail)
        pending["chain"]()
        pending["ln"]()
        pending["se_a"]()
        pending["se_b"]()

    nc.compile()
    return nc


def _get_nc():
    if "nc" not in _cache:
        _cache["nc"] = _build_nc()
    return _cache["nc"]


def _prep_host(inputs):
    import ml_dtypes

    f = np.float32
    bf = ml_dtypes.bfloat16
    x = np.asarray(inputs["x"], f)
    base_weight = np.asarray(inputs["base_weight"], f)
    spline_weight = np.asarray(inputs["spline_weight"], f)
    ln_w = np.asarray(inputs["ln_w"], f)
    ln_b = np.asarray(inputs["ln_b"], f)
    se_w1 = np.asarray(inputs["se_w1"], f)
    se_b1 = np.asarray(inputs["se_b1"], f)
    se_w2 = np.asarray(inputs["se_w2"], f)
    se_b2 = np.asarray(inputs["se_b2"], f)

    f8 = ml_dtypes.float8_e4m3
    xt_all = x.reshape(N_CORES, NTOK, D).transpose(0, 2, 1)  # [core, D, ntok]

    # base channel: 512*Wb in bf16 (2^9 scale is exact); the 512 factor
    # matches the fp8 spline product scale so both share one PSUM bank
    w_base = (512.0 * base_weight.T).reshape(NC_I, P, D).astype(bf)

    wsT = spline_weight.transpose(1, 2, 0)  # [i, g, o]
    wm = wsT[:, 0, :].astype(f)
    w0 = wsT[:, 1, :].astype(f)
    wp = wsT[:, 2, :].astype(f)
    # h/step channels (x512): vd*(Wm-W0) [hi+lo], vc*(Wp-W0) [hi+lo],
    # km*(-Wm), kp*(-Wp); the 0.5 offsets of h and the W0 channel fold into
    # the per-feature constant c0 applied at the PSUM descale
    wmp = (wm - w0) * 512.0
    wpp = (wp - w0) * 512.0
    wmp_hi = wmp.astype(f8)
    wpp_hi = wpp.astype(f8)
    chans = [
        wmp_hi,
        (wmp - wmp_hi.astype(f)).astype(f8),
        wpp_hi,
        (wpp - wpp_hi.astype(f)).astype(f8),
        (-512.0 * wm).astype(f8),
        (-512.0 * wp).astype(f8),
    ]
    w8_all = np.empty((NPAIR, P, NSCH, 2, D), f8)
    for c8, wsrc in enumerate(chans):
        w8_all[:, :, c8, :, :] = wsrc.reshape(NPAIR, 2, P, D).transpose(0, 2, 1, 3)
    w8_all = np.ascontiguousarray(w8_all.reshape(NPAIR, P, NSCH * 2 * D))
    c0 = (w0.sum(axis=0) + 0.5 * (wm - w0).sum(axis=0)
          + 0.5 * (wp - w0).sum(axis=0)).astype(f)              # [o]

    w1p = (se_w1 * ln_w[None, :]).astype(f)          # ln_w folded into W1
    b1p = (se_b1 + se_w1 @ ln_b).astype(f)           # ln_b folded into b1

    shared = {
        "w": w_base,
        "w8": w8_all,
        "c0": np.ascontiguousarray(c0.reshape(NO, P).T),
        "w1t": np.ascontiguousarray(w1p.T.reshape(NO, P, 32)).astype(bf),
        "w2t": np.ascontiguousarray(se_w2.T).astype(bf),
        "lnw": np.ascontiguousarray(ln_w.reshape(NO, P).T),
        "lnb": np.ascontiguousarray(ln_b.reshape(NO, P).T),
        "onesc": np.full((P, 1), 1.0 / D, bf),
        "onesp": np.ones((1, P), bf),
        "b1": np.ascontiguousarray(b1p.reshape(32, 1)),
        "b2": np.ascontiguousarray(se_b2.reshape(NO, P).T),
    }
    in_maps = []
    for k in range(N_CORES):
        m = dict(shared)
        xk = np.ascontiguousarray(xt_all[k].reshape(NC_I, P, NTOK))
        m["xt"] = xk.astype(bf)
        m["xt32"] = xk
        in_maps.append(m)
    return in_maps


def kernel(**inputs) -> np.ndarray:
    from concourse.bass_utils import run_bass_kernel_spmd

    nc = _get_nc()
    in_maps = _prep_host(inputs)
    trace = bool(int(os.environ.get("KERNEL_TRACE", "0")))
    res = run_bass_kernel_spmd(
        nc, in_maps, core_ids=list(range(N_CORES)), trace=trace
    )
    _cache["last_result"] = res
    outs = []
    for k in range(N_CORES):
        outT = np.asarray(res.results[k]["outT"], dtype=np.float32)  # [NO, P, NTOK]
        outs.append(outT.reshape(D, NTOK).T)                          # [ntok, o]
    out = np.concatenate(outs, axis=0).reshape(8, 1024, 1024)
    return np.ascontiguousarray(out.astype(np.float32))


# revision 72
# speedup vs baseline: 1.4018x; 1.1185x over previous
"""Trainium2 Bass kernel for nn_EnhancedDRKANTreeNet (KAN layer + LayerNorm + SE gate).

Data-parallel over 8192 tokens across 8 NeuronCores (1024 tokens/core), all
compute feature-major: tiles are [feature_partition, token].

  out^T[o, n] = sum_i x^T[i, n]*Wb[o, i] + sum_{i,g} bn_g[i, n]*Ws[o, i, g]

All matmul operands are bf16 (1 cycle/row on the PE, same as fp32r, but half
the HBM traffic and 2x DVE throughput on elementwise tiles); PSUM accumulates
fp32. The sigma trick folds the bn_0 spline channel into [x, bn_-1, bn_+1,
sigma] with host-combined weights. Weights are loaded into SBUF ONCE and
reused by every token tile.

Token tiles are [512, 384, 128]: the small final tile shrinks the serial
LN/SE tail. Main accumulation is split into two o-groups of 4 PSUM banks so
the LayerNorm/SE auxiliary matmuls (stats, broadcast outer products, SE) of
tile t can run in dedicated PSUM banks concurrently with tile t+1's main
accumulation; the aux work is emitted interleaved into tile t+1's chunk loop
for engine-stream overlap.

LayerNorm: per-token mean/rsqrt(var) stats via ones-matmuls (mean folded into
the ones scale), an int16 bit-hack rsqrt seed + one bf16 Newton step, then
y = out*(ones (x) z) + (ones (x) -mu*z), with ln_w/ln_b applied per-feature by
a 4x-rate tensor_scalar op (ln_w is also folded into the SE W1 on the host,
ln_b into the SE b1). SE: h = relu(W1'.y + b1'), se = sigmoid(W2.h + b2),
final = (ln_w*y + ln_b) * se, stored as bf16.
"""

import os
from contextlib import ExitStack

import numpy as np

P = 128
D = 1024
NC_I = 8           # contraction chunks of 128 over D_IN
NCH = 4            # rhs channels per i-chunk: x, bn[-1], bn[+1], sigma
NPAIR = 4          # chunk pairs for fp8 DoubleRow spline matmuls
NSCH = 6           # spline DR channels: (bnm, bnp, sg) x (w_hi, w_lo)
NO = 8             # output-feature chunks of 128
NTOK = 1024        # tokens per core
N_CORES = 8
TILES = [(0, 512), (512, 384), (896, 128)]   # (tok0, T) per token tile
OGROUPS = [(0, 6), (6, 2)]                   # o-block groups (start, count)
GRID = [-1.0, 0.0, 1.0]
EPS_BASIS = 1e-6
KSLOPE = 4400.0    # step-channel sigmoid slope (matches the eps=1e-6
                   # rational transition width sqrt(eps)~1e-3 at |x|=2)
LN_EPS = 1e-5
RSQRT_MAGIC32 = 0x5F3759DF   # fp32 rsqrt bit-hack seed

_cache = {}


def _build_nc():
    import concourse.bass as bass
    import concourse.mybir as mybir
    import concourse.tile as tile
    from concourse import bacc

    f32 = mybir.dt.float32
    bf16 = mybir.dt.bfloat16
    f8e4 = mybir.dt.float8e4
    PM = mybir.MatmulPerfMode
    i32 = mybir.dt.int32
    AF = mybir.ActivationFunctionType
    OP = mybir.AluOpType
    ts = bass.ts

    nc = bacc.Bacc(
        "TRN2",
        target_bir_lowering=False,
        debug=False,
        enable_asserts=False,
        num_devices=N_CORES,
    )

    xt_d = nc.dram_tensor("xt", [NC_I, P, NTOK], bf16, kind="ExternalInput")
    xt32_d = nc.dram_tensor("xt32", [NC_I, P, NTOK], f32, kind="ExternalInput")
    c0_d = nc.dram_tensor("c0", [P, NO], f32, kind="ExternalInput")
    w_d = nc.dram_tensor("w", [NC_I, P, D], bf16, kind="ExternalInput")
    w8_d = nc.dram_tensor("w8", [NPAIR, P, NSCH * 2 * D], f8e4,
                          kind="ExternalInput")
    w1t_d = nc.dram_tensor("w1t", [NO, P, 32], bf16, kind="ExternalInput")
    w2t_d = nc.dram_tensor("w2t", [32, D], bf16, kind="ExternalInput")
    lnw_d = nc.dram_tensor("lnw", [P, NO], f32, kind="ExternalInput")
    lnb_d = nc.dram_tensor("lnb", [P, NO], f32, kind="ExternalInput")
    onesc_d = nc.dram_tensor("onesc", [P, 1], bf16, kind="ExternalInput")  # 1/D
    onesp_d = nc.dram_tensor("onesp", [1, P], bf16, kind="ExternalInput")  # 1.0
    b1_d = nc.dram_tensor("b1", [32, 1], f32, kind="ExternalInput")
    b2_d = nc.dram_tensor("b2", [P, NO], f32, kind="ExternalInput")
    out_d = nc.dram_tensor("outT", [NO, P, NTOK], bf16, kind="ExternalOutput")

    with tile.TileContext(nc) as tc, ExitStack() as ctx:
        cp = ctx.enter_context(tc.tile_pool(name="cp", bufs=1))
        wp = ctx.enter_context(tc.tile_pool(name="wp", bufs=1))
        xp = ctx.enter_context(tc.tile_pool(name="xp", bufs=1))
        bn8p = ctx.enter_context(tc.tile_pool(name="bn8p", bufs=1))
        bp = ctx.enter_context(tc.tile_pool(name="bp", bufs=2))
        op_pool = ctx.enter_context(tc.tile_pool(name="op", bufs=2))
        sqp = ctx.enter_context(tc.tile_pool(name="sqp", bufs=2))
        x32p = ctx.enter_context(tc.tile_pool(name="x32p", bufs=2))
        stp = ctx.enter_context(tc.tile_pool(name="stp", bufs=2))
        yp = ctx.enter_context(tc.tile_pool(name="yp", bufs=1))
        sep = ctx.enter_context(tc.tile_pool(name="sep", bufs=2))
        finp = ctx.enter_context(tc.tile_pool(name="finp", bufs=1))
        # PSUM: 6 (main) + 1 (stats/broadcast, shared tag) + 1 (SE) = 8 banks
        mps = ctx.enter_context(tc.tile_pool(name="mps", bufs=6, space="PSUM"))
        auxps = ctx.enter_context(tc.tile_pool(name="auxps", bufs=1, space="PSUM"))
        seps = ctx.enter_context(tc.tile_pool(name="seps", bufs=1, space="PSUM"))

        # warm the sigmoid_and_others ACT table at t=0 (overlaps initial DMA)
        warm_t = cp.tile([P, 1], f32, tag="warm")
        nc.scalar.activation(warm_t[:], nc.const_aps.tensor(1.0, (P, 1)), AF.Relu)

        bk_t = cp.tile([P, 1], f32, tag="bk")
        nc.gpsimd.memset(bk_t[:], -2.0 * KSLOPE)
        half_t = cp.tile([P, 512], bf16, tag="half")
        nc.gpsimd.memset(half_t[:], 0.5)

        C = {}  # constant tiles, DMA'd after the first chunk's x/w (startup)

        def emit_consts():
            C["w1t"] = cp.tile([P, NO, 32], bf16, tag="w1t", name="w1t_t")
            nc.gpsimd.dma_start(C["w1t"][:], w1t_d.ap().rearrange("c p j -> p c j"))
            C["w2t"] = cp.tile([32, D], bf16, tag="w2t", name="w2t_t")
            nc.gpsimd.dma_start(C["w2t"][:], w2t_d.ap())
            C["lnw"] = cp.tile([P, NO], f32, tag="lnw", name="lnw_t")
            nc.gpsimd.dma_start(C["lnw"][:], lnw_d.ap())
            C["lnb"] = cp.tile([P, NO], f32, tag="lnb", name="lnb_t")
            nc.gpsimd.dma_start(C["lnb"][:], lnb_d.ap())
            C["onesc"] = cp.tile([P, 1], bf16, tag="onesc", name="onesc_t")
            nc.gpsimd.dma_start(C["onesc"][:], onesc_d.ap())
            C["onesp"] = cp.tile([1, P], bf16, tag="onesp", name="onesp_t")
            nc.gpsimd.dma_start(C["onesp"][:], onesp_d.ap())
            C["b1"] = cp.tile([32, 1], f32, tag="b1", name="b1_t")
            nc.gpsimd.dma_start(C["b1"][:], b1_d.ap())
            C["b2"] = cp.tile([P, NO], f32, tag="b2", name="b2_t")
            nc.gpsimd.dma_start(C["b2"][:], b2_d.ap())
            C["c0"] = cp.tile([P, NO], f32, tag="c0", name="c0_t")
            nc.gpsimd.dma_start(C["c0"][:], c0_d.ap())

        w_tiles = [None] * NC_I
        w8_tiles = [None] * NPAIR

        def emit_wb_dma(c):
            w_t = wp.tile([P, D], bf16, tag=f"w{c}")
            nc.sync.dma_start(w_t[:], w_d.ap()[c])
            w_tiles[c] = w_t

        def emit_w8_dma(pair):
            w8_src = w8_d.ap()[pair].rearrange(
                "p (c8 j d) -> p c8 j d", c8=NSCH, j=2
            )
            tiles = []
            for c8 in range(NSCH):
                w8_t = wp.tile([P, 2, D], f8e4, tag=f"w8p{pair}c{c8}")
                nc.sync.dma_start(w8_t[:], w8_src[:, c8])
                tiles.append(w8_t)
            w8_tiles[pair] = tiles

        def emit_basis(ti, c, T, tok0, bn8_by_pair):
            """Basis channels for chunk c via the h/step reformulation.

            bn_p = h(clamp(x,0,1)) - step(x>2), bn_m = h(clamp(-x,0,1)) -
            step(x<-2), sigma = 1 - step+ - step-, with h(c) = c^2/(c^2 +
            (1-c)^2) = 0.5 + u/(2u^2+0.5), u = c-0.5. The 0.5 offsets and the
            W0 channel fold into a per-feature constant (c0) applied at the
            PSUM descale. v = u/(2u^2+0.5) and the steps are written as fp8
            into slot c%2 of the chunk-pair tiles for DoubleRow matmuls. The
            steps use the fp32 copy of x (the bf16 ulp at |x|=2 is 16x wider
            than the eps-rational transition band being approximated).
            """
            pair, j = c // 2, c % 2
            x_t = xp.tile([P, T], bf16, tag=f"x{c}")
            nc.sync.dma_start(x_t[:], xt_d.ap()[c, :, tok0:tok0 + T])
            x32_t = x32p.tile([P, T], f32, tag="x32")
            nc.sync.dma_start(x32_t[:], xt32_d.ap()[c, :, tok0:tok0 + T])
            if j == 0:
                bn8_by_pair[pair] = [
                    bn8p.tile([P, 2, T], f8e4, tag=f"bn8{k}{pair}",
                              name=f"bn8{k}{pair}_{ti}")
                    for k in ("vd", "vc", "km", "kp")
                ]
            vd8, vc8, km8, kp8 = bn8_by_pair[pair]
            uc_t = bp.tile([P, T], bf16, tag="uc")
            nc.vector.tensor_scalar(uc_t[:], x_t[:], 1.0, 0.0, OP.min, OP.max)
            nc.vector.tensor_scalar(uc_t[:], uc_t[:], -0.5, None, OP.add)
            ud_t = bp.tile([P, T], bf16, tag="ud")
            nc.vector.tensor_scalar(ud_t[:], x_t[:], 0.0, -1.0, OP.min, OP.mult)
            nc.vector.tensor_scalar(ud_t[:], ud_t[:], 1.0, -0.5, OP.min, OP.add)
            u2c = bp.tile([P, T], bf16, tag="u2c")
            nc.scalar.activation(u2c[:], uc_t[:], AF.Square, scale=2.0 ** 0.5)
            u2d = bp.tile([P, T], bf16, tag="u2d")
            nc.scalar.activation(u2d[:], ud_t[:], AF.Square, scale=2.0 ** 0.5)
            shc = bp.tile([P, T], f32, tag="shc")
            nc.scalar.activation(shc[:], u2c[:], AF.Copy, bias=0.5, scale=1.0)
            shd = bp.tile([P, T], f32, tag="shd")
            nc.scalar.activation(shd[:], u2d[:], AF.Copy, bias=0.5, scale=1.0)
            invc = bp.tile([P, T], f32, tag="invc")
            nc.vector.reciprocal_approx_fast(out=invc[:], in_=shc[:])
            invd = bp.tile([P, T], f32, tag="invd")
            nc.vector.reciprocal_approx_fast(out=invd[:], in_=shd[:])
            nc.gpsimd.tensor_tensor(vc8[:, j], uc_t[:], invc[:], OP.mult)
            nc.gpsimd.tensor_tensor(vd8[:, j], ud_t[:], invd[:], OP.mult)
            nc.scalar.activation(kp8[:, j], x32_t[:], AF.Sigmoid, bias=bk_t[:],
                                 scale=KSLOPE)
            nc.scalar.activation(km8[:, j], x32_t[:], AF.Sigmoid, bias=bk_t[:],
                                 scale=-KSLOPE)
            return x_t

        def emit_base_mains(ps, x_t, c, o0, no):
            """bf16 base-channel matmuls (x * 512Wb) for one chunk."""
            for oi in range(no):
                o = o0 + oi
                nc.tensor.matmul(
                    ps[oi][:],
                    lhsT=w_tiles[c][:, ts(o, P)],
                    rhs=x_t[:],
                    start=(c == 0),
                    stop=False,
                )

        def emit_spline_mains(ps, bn8, pair, o0, no, last, block_post, psl=None):
            """fp8 DoubleRow spline matmuls for one chunk pair (6 channels:
            3 bn tensors x hi/lo weights). On the final pair the emission is
            o-outer with per-block copies/stats chasing the stop."""
            w8 = w8_tiles[pair]
            # channels: vd*(Wm'-hi/lo), vc*(Wp'-hi/lo), km*(-Wm), kp*(-Wp)
            rhs = [bn8[0], bn8[0], bn8[1], bn8[1], bn8[2], bn8[3]]
            if not last:
                for c8 in range(NSCH):
                    for oi in range(no):
                        o = o0 + oi
                        nc.tensor.matmul(
                            ps[oi][:],
                            lhsT=w8[c8][:, :, ts(o, P)],
                            rhs=rhs[c8][:],
                            start=False,
                            stop=False,
                            perf_mode=PM.DoubleRow,
                        )
            else:
                for oi in range(no):
                    o = o0 + oi
                    for c8 in range(NSCH):
                        nc.tensor.matmul(
                            ps[oi][:],
                            lhsT=w8[c8][:, :, ts(o, P)],
                            rhs=rhs[c8][:],
                            start=False,
                            stop=(c8 == NSCH - 1),
                            perf_mode=PM.DoubleRow,
                        )
                    block_post(ps[oi], o)

        def emit_tile(ti, tok0, T, pending_aux, rhs_by_chunk, bn8_by_pair,
                      basis_next):
            """Emit one token tile's mains+stats; interleave prev tile's aux
            into group 0 and the NEXT tile's basis into group 1."""
            outs = [None] * NO
            sq = [None] * NO
            st = {}

            def get_psAB():
                # allocated lazily so the "aux" tag rotation matches runtime
                # order (after the previous tile's zb/mrb/psS allocations)
                if "psAB" not in st:
                    st["psAB"] = auxps.tile([33, T], f32, tag="aux",
                                            name=f"psAB_{ti}")
                return st["psAB"]

            def block_post(ps_ap, o):
                psAB = get_psAB()
                psA = psAB[0:1, :]
                psB = psAB[32:33, :]
                o_t = op_pool.tile([P, T], bf16, tag=f"out{o}", name=f"o_{ti}_{o}")
                nc.vector.tensor_scalar(o_t[:], ps_ap[:], 1.0 / 512.0,
                                        C["c0"][:, o:o + 1], OP.mult, OP.add)
                outs[o] = o_t
                sq_t = sqp.tile([P, T], bf16, tag="sq")
                nc.gpsimd.tensor_tensor(sq_t[:], o_t[:], o_t[:], OP.mult)
                sq[o] = sq_t
                nc.tensor.matmul(
                    psA, lhsT=C["onesc"][:], rhs=o_t[:],
                    start=(o == 0), stop=(o == NO - 1),
                )
                nc.tensor.matmul(
                    psB, lhsT=C["onesc"][:], rhs=sq_t[:],
                    start=(o == 0), stop=(o == NO - 1),
                )

            next_rhs = [None] * NC_I
            next_bn8 = [None] * NPAIR
            for gi, (o0, no) in enumerate(OGROUPS):
                ps = [
                    mps.tile([P, T], f32, tag="mps", name=f"ps_{ti}_{o0 + i}")
                    for i in range(no)
                ]
                if ti == 0 and gi == 0:
                    # startup: the base channel of every chunk only needs x +
                    # its weights, so run all 8 of those first while the basis
                    # pipeline fills; the spline pairs follow in a second pass
                    for c in range(NC_I):
                        rhs_by_chunk[c] = emit_basis(ti, c, T, tok0,
                                                     bn8_by_pair)
                        emit_wb_dma(c)
                        emit_base_mains(ps, rhs_by_chunk[c], c, o0, no)
                        if c == 0:
                            emit_consts()
                    for pair in range(NPAIR):
                        emit_w8_dma(pair)
                        emit_spline_mains(ps, bn8_by_pair[pair], pair, o0, no,
                                          last=(pair == NPAIR - 1),
                                          block_post=block_post)
                    continue
                for c in range(NC_I):
                    emit_base_mains(ps, rhs_by_chunk[c], c, o0, no)
                    if c % 2 == 1:
                        emit_spline_mains(ps, bn8_by_pair[c // 2], c // 2,
                                          o0, no, last=(c == NC_I - 1),
                                          block_post=block_post)
                    # interleave the previous tile's aux across both groups
                    if pending_aux is not None:
                        stage = pending_aux.get((gi, c))
                        if stage is not None:
                            stage()
                    # compute the NEXT tile's basis during group 1 (ACT/Pool/
                    # DVE are otherwise idle here); shifted one chunk so the
                    # bufs=1 bn8 pair tiles are already past their last reader
                    if gi == 1 and basis_next is not None and c >= 1:
                        ntok0, nT = basis_next
                        next_rhs[c - 1] = emit_basis(ti + 1, c - 1, nT, ntok0,
                                                     next_bn8)
                        if c == NC_I - 1:
                            next_rhs[c] = emit_basis(ti + 1, c, nT, ntok0,
                                                     next_bn8)

            return {"T": T, "tok0": tok0, "ti": ti, "psAB": st["psAB"],
                    "outs": outs, "sq": sq, "next_rhs": next_rhs,
                    "next_bn8": next_bn8}

        def make_aux(tile_st):
            """Aux stage emitters for a completed tile: stats chain -> LN -> SE."""
            T = tile_st["T"]
            ti = tile_st["ti"]
            tok0 = tile_st["tok0"]
            outs = tile_st["outs"]
            psAB = tile_st["psAB"]
            ctx_st = {}

            def chain():
                # negmu/e2 rows from psum, fp32 bit-hack rsqrt + 1 Newton step
                negmu = stp.tile([1, T], f32, tag="negmu")
                nc.vector.tensor_scalar(
                    negmu[:], psAB[0:1, :], -1.0, 0.0, OP.mult, OP.add
                )
                e2 = stp.tile([1, T], f32, tag="e2")
                nc.vector.tensor_scalar(
                    e2[:], psAB[32:33, :], 1.0, LN_EPS, OP.mult, OP.add
                )
                mu2 = stp.tile([1, T], f32, tag="mu2")
                nc.vector.tensor_tensor(mu2[:], negmu[:], negmu[:], OP.mult)
                var = stp.tile([1, T], f32, tag="var")
                nc.vector.tensor_tensor(var[:], e2[:], mu2[:], OP.subtract)
                zw = stp.tile([1, T], f32, tag="zw")
                nc.vector.tensor_scalar(
                    zw[:].bitcast(i32), var[:].bitcast(i32), 1, 0,
                    OP.arith_shift_right,
                )
                nc.vector.tensor_scalar(
                    zw[:].bitcast(i32), zw[:].bitcast(i32), -1, RSQRT_MAGIC32,
                    OP.mult, OP.add,
                )
                t1 = stp.tile([1, T], f32, tag="t1")
                nc.vector.tensor_tensor(t1[:], zw[:], zw[:], OP.mult)
                nc.vector.tensor_tensor(t1[:], t1[:], var[:], OP.mult)
                nc.vector.tensor_scalar(t1[:], t1[:], -0.5, 1.5, OP.mult, OP.add)
                z16 = stp.tile([1, T], bf16, tag="z16")
                nc.vector.tensor_tensor(z16[:], zw[:], t1[:], OP.mult)
                mr16 = stp.tile([1, T], bf16, tag="mr16")
                nc.vector.tensor_tensor(mr16[:], negmu[:], z16[:], OP.mult)
                # broadcast rows across partitions via K=1 outer products
                zbp = auxps.tile([P, T], f32, tag="aux", name=f"zb_{ti}")
                nc.tensor.matmul(zbp[:], lhsT=C["onesp"][:], rhs=z16[:],
                                 start=True, stop=True)
                zb16 = stp.tile([P, T], bf16, tag="zb16")
                nc.vector.tensor_copy(out=zb16[:], in_=zbp[:])
                mrp = auxps.tile([P, T], f32, tag="aux", name=f"mrb_{ti}")
                nc.tensor.matmul(mrp[:], lhsT=C["onesp"][:], rhs=mr16[:],
                                 start=True, stop=True)
                mrb16 = stp.tile([P, T], bf16, tag="mrb16")
                nc.scalar.activation(mrb16[:], mrp[:], AF.Copy)
                ctx_st["zb16"] = zb16
                ctx_st["mrb16"] = mrb16

            def ln():
                zb16, mrb16 = ctx_st["zb16"], ctx_st["mrb16"]
                psH = seps.tile([32, T], f32, tag="sps", name=f"psH_{ti}")
                yhat = []
                for o in range(NO):
                    q_t = sqp.tile([P, T], bf16, tag="q")
                    nc.vector.tensor_tensor(q_t[:], outs[o][:], zb16[:], OP.mult)
                    yh_t = yp.tile([P, T], bf16, tag=f"yh{o}")
                    nc.vector.tensor_tensor(yh_t[:], q_t[:], mrb16[:], OP.add)
                    yhat.append(yh_t)
                    nc.tensor.matmul(
                        psH[:], lhsT=C["w1t"][:, o, :], rhs=yh_t[:],
                        start=(o == 0), stop=(o == NO - 1),
                    )
                hr = sep.tile([32, T], bf16, tag="hr")
                nc.scalar.activation(hr[:], psH[:], AF.Relu, bias=C["b1"][:], scale=1.0)
                ctx_st["yhat"] = yhat
                ctx_st["hr"] = hr

            def se_blocks(olist):
                yhat, hr = ctx_st["yhat"], ctx_st["hr"]
                if "fin" not in ctx_st:
                    # one wide tile so all 8 out blocks leave in a single DMA
                    # (the descriptor engine processes DMAs serially)
                    ctx_st["fin"] = finp.tile([P, NO, T], bf16, tag="fin",
                                             name=f"fin_{ti}")
                fin_t = ctx_st["fin"]
                for o in olist:
                    # alternate psS between the two non-main PSUM banks so the
                    # W2 matmul of block o+1 overlaps the sigmoid of block o
                    pool, tg = (seps, "sps") if o % 2 else (auxps, "aux")
                    psS = pool.tile([P, T], f32, tag=tg, name=f"psS_{ti}_{o}")
                    nc.tensor.matmul(
                        psS[:], lhsT=C["w2t"][:, ts(o, P)], rhs=hr[:],
                        start=True, stop=True,
                    )
                    se_t = sep.tile([P, T], bf16, tag="se")
                    nc.scalar.activation(
                        se_t[:], psS[:], AF.Sigmoid, bias=C["b2"][:, o:o + 1], scale=1.0
                    )
                    yf_t = sep.tile([P, T], bf16, tag="yf")
                    nc.vector.tensor_scalar(
                        yf_t[:], yhat[o][:], C["lnw"][:, o:o + 1], C["lnb"][:, o:o + 1],
                        OP.mult, OP.add,
                    )
                    nc.vector.tensor_tensor(fin_t[:, o], yf_t[:], se_t[:], OP.mult)
                if olist[-1] == NO - 1:
                    nc.sync.dma_start(
                        out_d.ap()[:, :, tok0:tok0 + T].rearrange("o p t -> p o t"),
                        fin_t[:],
                    )

            return {"chain": chain, "ln": ln,
                    "se_a": lambda: se_blocks(range(0, 4)),
                    "se_b": lambda: se_blocks(range(4, NO))}

        AUX_SLOTS = {"chain": (0, 0), "ln": (0, 2), "se_a": (0, 5), "se_b": (1, 2)}

        pending = None
        rhs_cur = [None] * NC_I
        bn8_cur = [None] * NPAIR
        for ti, (tok0, T) in enumerate(TILES):
            basis_next = TILES[ti + 1] if ti + 1 < len(TILES) else None
            aux_by_slot = (
                {slot: pending[name] for name, slot in AUX_SLOTS.items()}
                if pending else None
            )
            tile_st = emit_tile(ti, tok0, T, aux_by_slot, rhs_cur, bn8_cur,
                                basis_next)
            rhs_cur = tile_st["next_rhs"]
            bn8_cur = tile_st["next_bn8"]
            pending = make_aux(tile_st)
        # final tile's aux runs at the end (smallest tile -> short tail)
        pending["chain"]()
        pending["ln"]()
        pending["se_a"]()
        pending["se_b"]()

    nc.compile()
    return nc


def _get_nc():
    if "nc" not in _cache:
        _cache["nc"] = _build_nc()
    return _cache["nc"]


def _prep_host(inputs):
    import ml_dtypes

    f = np.float32
    bf = ml_dtypes.bfloat16
    x = np.asarray(inputs["x"], f)
    base_weight = np.asarray(inputs["base_weight"], f)
    spline_weight = np.asarray(inputs["spline_weight"], f)
    ln_w = np.asarray(inputs["ln_w"], f)
    ln_b = np.asarray(inputs["ln_b"], f)
    se_w1 = np.asarray(inputs["se_w1"], f)
    se_b1 = np.asarray(inputs["se_b1"], f)
    se_w2 = np.asarray(inputs["se_w2"], f)
    se_b2 = np.asarray(inputs["se_b2"], f)

    f8 = ml_dtypes.float8_e4m3
    xt_all = x.reshape(N_CORES, NTOK, D).transpose(0, 2, 1)  # [core, D, ntok]

    # base channel: 512*Wb in bf16 (2^9 scale is exact); the 512 factor
    # matches the fp8 spline product scale so both share one PSUM bank
    w_base = (512.0 * base_weight.T).reshape(NC_I, P, D).astype(bf)

    wsT = spline_weight.transpose(1, 2, 0)  # [i, g, o]
    wm = wsT[:, 0, :].astype(f)
    w0 = wsT[:, 1, :].astype(f)
    wp = wsT[:, 2, :].astype(f)
    # h/step channels (x512): vd*(Wm-W0) [hi+lo], vc*(Wp-W0) [hi+lo],
    # km*(-Wm), kp*(-Wp); the 0.5 offsets of h and the W0 channel fold into
    # the per-feature constant c0 applied at the PSUM descale
    wmp = (wm - w0) * 512.0
    wpp = (wp - w0) * 512.0
    wmp_hi = wmp.astype(f8)
    wpp_hi = wpp.astype(f8)
    chans = [
        wmp_hi,
        (wmp - wmp_hi.astype(f)).astype(f8),
        wpp_hi,
        (wpp - wpp_hi.astype(f)).astype(f8),
        (-512.0 * wm).astype(f8),
        (-512.0 * wp).astype(f8),
    ]
    w8_all = np.empty((NPAIR, P, NSCH, 2, D), f8)
    for c8, wsrc in enumerate(chans):
        w8_all[:, :, c8, :, :] = wsrc.reshape(NPAIR, 2, P, D).transpose(0, 2, 1, 3)
    w8_all = np.ascontiguousarray(w8_all.reshape(NPAIR, P, NSCH * 2 * D))
    c0 = (w0.sum(axis=0) + 0.5 * (wm - w0).sum(axis=0)
          + 0.5 * (wp - w0).sum(axis=0)).astype(f)              # [o]

    w1p = (se_w1 * ln_w[None, :]).astype(f)          # ln_w folded into W1
    b1p = (se_b1 + se_w1 @ ln_b).astype(f)           # ln_b folded into b1

    shared = {
        "w": w_base,
        "w8": w8_all,
        "c0": np.ascontiguousarray(c0.reshape(NO, P).T),
        "w1t": np.ascontiguousarray(w1p.T.reshape(NO, P, 32)).astype(bf),
        "w2t": np.ascontiguousarray(se_w2.T).astype(bf),
        "lnw": np.ascontiguousarray(ln_w.reshape(NO, P).T),
        "lnb": np.ascontiguousarray(ln_b.reshape(NO, P).T),
        "onesc": np.full((P, 1), 1.0 / D, bf),
        "onesp": np.ones((1, P), bf),
        "b1": np.ascontiguousarray(b1p.reshape(32, 1)),
        "b2": np.ascontiguousarray(se_b2.reshape(NO, P).T),
    }
    in_maps = []
    for k in range(N_CORES):
        m = dict(shared)
        xk = np.ascontiguousarray(xt_all[k].reshape(NC_I, P, NTOK))
        m["xt"] = xk.astype(bf)
        m["xt32"] = xk
        in_maps.append(m)
    return in_maps


def kernel(**inputs) -> np.ndarray:
    from concourse.bass_utils import run_bass_kernel_spmd

    nc = _get_nc()
    in_maps = _prep_host(inputs)
    trace = bool(int(os.environ.get("KERNEL_TRACE", "0")))
    res = run_bass_kernel_spmd(
        nc, in_maps, core_ids=list(range(N_CORES)), trace=trace
    )
    _cache["last_result"] = res
    outs = []
    for k in range(N_CORES):
        outT = np.asarray(res.results[k]["outT"], dtype=np.float32)  # [NO, P, NTOK]
        outs.append(outT.reshape(D, NTOK).T)                          # [ntok, o]
    out = np.concatenate(outs, axis=0).reshape(8, 1024, 1024)
    return np.ascontiguousarray(out.astype(np.float32))


# revision 91
# speedup vs baseline: 1.4514x; 1.0354x over previous
"""Trainium2 Bass kernel for nn_EnhancedDRKANTreeNet (KAN layer + LayerNorm + SE gate).

Data-parallel over 8192 tokens across 8 NeuronCores (1024 tokens/core), all
compute feature-major: tiles are [feature_partition, token]. Token tiles are
[512, 384, 128] (the small final tile shrinks the serial LN/SE tail).

Main contraction, per output block of 128 features:
 - base channel x*(512*Wb) in bf16 (1 cycle/row on the PE);
 - spline channels in fp8e4 with DoubleRow perf mode (2 contraction rows per
   partition, 0.5 cycles/row). The grid-3 quadratic-spline basis is
   reformulated as bn_p = h(clamp(x,0,1)) - step(x>2), bn_m = h(clamp(-x,0,1))
   - step(x<-2), sigma = 1 - step+ - step-, with h(c) = 0.5 + u/(2u^2+0.5),
   u = c - 0.5 (exactly the eps-regularized normalized basis away from a
   ~1e-3-wide band at |x|=2). Channels: v_d*(Wm-W0)[fp8 hi+lo], v_c*(Wp-W0)
   [fp8 hi+lo], step-*(-Wm), step+*(-Wp); the 0.5 offsets of h and the W0
   channel fold into a per-feature constant c0 added at the PSUM descale.
   Steps are ACT sigmoids with slope 4400 evaluated on an fp32 copy of x
   (the bf16 ulp at |x|=2 is wider than the band being approximated).
   Weights are loaded into SBUF once; PSUM accumulates everything at 512x.

Elementwise engine placement (balanced ~80us each): ACT does the two
h-squares, the 2u^2+0.5 Copy-biases and the step sigmoids; DVE does clamps,
reciprocals (18-bit approx), PSUM descales, LN apply and stats chain; the
gpsimd/Pool engine does the v=u*inv fp8 writes and out^2 squares.

PSUM (8 banks): main accumulation is split into o-groups of 6+2 banks plus
one stats/broadcast bank (shared-tag rotation: psAB -> zb -> mrb -> psS-even)
and one SE bank, so tile t's LayerNorm/SE aux runs concurrently with tile
t+1's accumulation; aux stages are emitted interleaved into tile t+1's chunk
loop, and tile t+1's basis is produced during tile t's mains.

LayerNorm: per-token stats via ones-matmuls (mean folded into the ones
scale), fp32 int32-bit-hack rsqrt seed + one Newton step, then
y = out*(ones (x) z) + (ones (x) -mu*z); ln_w/ln_b are applied per-feature by
a 4x-rate tensor_scalar op (ln_w is also folded into the SE W1 on the host,
ln_b into the SE b1). SE: h = relu(W1'.y + b1'), se = sigmoid(W2.h + b2),
final = (ln_w*y + ln_b)*se, stored bf16 and shipped per-tile in one DMA.
"""

import os
from contextlib import ExitStack

import numpy as np

P = 128
D = 1024
NC_I = 8           # contraction chunks of 128 over D_IN
NPAIR = 4          # chunk pairs for fp8 DoubleRow spline matmuls
NSCH = 6           # spline DR channels: (bnm, bnp, sg) x (w_hi, w_lo)
NO = 8             # output-feature chunks of 128
NTOK = 1024        # tokens per core
N_CORES = 8
TILES = [(0, 512), (512, 384), (896, 128)]   # (tok0, T) per token tile
OGROUPS = [(0, 6), (6, 2)]                   # o-block groups (start, count)
KSLOPE = 4400.0    # step-channel sigmoid slope (matches the eps=1e-6
                   # rational transition width sqrt(eps)~1e-3 at |x|=2)
LN_EPS = 1e-5
RSQRT_MAGIC32 = 0x5F3759DF   # fp32 rsqrt bit-hack seed

_cache = {}


def _build_nc():
    import concourse.bass as bass
    import concourse.mybir as mybir
    import concourse.tile as tile
    from concourse import bacc

    f32 = mybir.dt.float32
    bf16 = mybir.dt.bfloat16
    f8e4 = mybir.dt.float8e4
    PM = mybir.MatmulPerfMode
    i32 = mybir.dt.int32
    AF = mybir.ActivationFunctionType
    OP = mybir.AluOpType
    ts = bass.ts

    nc = bacc.Bacc(
        "TRN2",
        target_bir_lowering=False,
        debug=False,
        enable_asserts=False,
        num_devices=N_CORES,
    )

    xt_d = nc.dram_tensor("xt", [NC_I, P, NTOK], bf16, kind="ExternalInput")
    xt32_d = nc.dram_tensor("xt32", [NC_I, P, NTOK], f32, kind="ExternalInput")
    c0_d = nc.dram_tensor("c0", [P, NO], f32, kind="ExternalInput")
    w_d = nc.dram_tensor("w", [NC_I, P, D], bf16, kind="ExternalInput")
    w8_d = nc.dram_tensor("w8", [NPAIR, P, NSCH * 2 * D], f8e4,
                          kind="ExternalInput")
    w1t_d = nc.dram_tensor("w1t", [NO, P, 32], bf16, kind="ExternalInput")
    w2t_d = nc.dram_tensor("w2t", [32, D], bf16, kind="ExternalInput")
    lnw_d = nc.dram_tensor("lnw", [P, NO], f32, kind="ExternalInput")
    lnb_d = nc.dram_tensor("lnb", [P, NO], f32, kind="ExternalInput")
    onesc_d = nc.dram_tensor("onesc", [P, 1], bf16, kind="ExternalInput")  # 1/D
    onesp_d = nc.dram_tensor("onesp", [1, P], bf16, kind="ExternalInput")  # 1.0
    b1_d = nc.dram_tensor("b1", [32, 1], f32, kind="ExternalInput")
    b2_d = nc.dram_tensor("b2", [P, NO], f32, kind="ExternalInput")
    out_d = nc.dram_tensor("outT", [NO, P, NTOK], bf16, kind="ExternalOutput")

    with tile.TileContext(nc) as tc, ExitStack() as ctx:
        cp = ctx.enter_context(tc.tile_pool(name="cp", bufs=1))
        wp = ctx.enter_context(tc.tile_pool(name="wp", bufs=1))
        xp = ctx.enter_context(tc.tile_pool(name="xp", bufs=2))
        bn8p = ctx.enter_context(tc.tile_pool(name="bn8p", bufs=2))
        bp = ctx.enter_context(tc.tile_pool(name="bp", bufs=2))
        op_pool = ctx.enter_context(tc.tile_pool(name="op", bufs=1))
        sqp = ctx.enter_context(tc.tile_pool(name="sqp", bufs=2))
        x32p = ctx.enter_context(tc.tile_pool(name="x32p", bufs=2))
        stp = ctx.enter_context(tc.tile_pool(name="stp", bufs=1))
        yp = ctx.enter_context(tc.tile_pool(name="yp", bufs=1))
        sep = ctx.enter_context(tc.tile_pool(name="sep", bufs=2))
        finp = ctx.enter_context(tc.tile_pool(name="finp", bufs=1))
        # PSUM: 6 (main) + 1 (stats/broadcast, shared tag) + 1 (SE) = 8 banks
        mps = ctx.enter_context(tc.tile_pool(name="mps", bufs=6, space="PSUM"))
        auxps = ctx.enter_context(tc.tile_pool(name="auxps", bufs=1, space="PSUM"))
        seps = ctx.enter_context(tc.tile_pool(name="seps", bufs=1, space="PSUM"))

        # warm the sigmoid_and_others ACT table at t=0 (overlaps initial DMA)
        warm_t = cp.tile([P, 1], f32, tag="warm")
        nc.scalar.activation(warm_t[:], nc.const_aps.tensor(1.0, (P, 1)), AF.Relu)

        bk_t = cp.tile([P, 1], f32, tag="bk")
        nc.gpsimd.memset(bk_t[:], -2.0 * KSLOPE)
        half_t = cp.tile([P, 512], bf16, tag="half")
        nc.gpsimd.memset(half_t[:], 0.5)

        C = {}  # constant tiles, DMA'd after the first chunk's x/w (startup)

        def emit_consts():
            C["w1t"] = cp.tile([P, NO, 32], bf16, tag="w1t", name="w1t_t")
            nc.gpsimd.dma_start(C["w1t"][:], w1t_d.ap().rearrange("c p j -> p c j"))
            C["w2t"] = cp.tile([32, D], bf16, tag="w2t", name="w2t_t")
            nc.gpsimd.dma_start(C["w2t"][:], w2t_d.ap())
            C["lnw"] = cp.tile([P, NO], f32, tag="lnw", name="lnw_t")
            nc.gpsimd.dma_start(C["lnw"][:], lnw_d.ap())
            C["lnb"] = cp.tile([P, NO], f32, tag="lnb", name="lnb_t")
            nc.gpsimd.dma_start(C["lnb"][:], lnb_d.ap())
            C["onesc"] = cp.tile([P, 1], bf16, tag="onesc", name="onesc_t")
            nc.gpsimd.dma_start(C["onesc"][:], onesc_d.ap())
            C["onesp"] = cp.tile([1, P], bf16, tag="onesp", name="onesp_t")
            nc.gpsimd.dma_start(C["onesp"][:], onesp_d.ap())
            C["b1"] = cp.tile([32, 1], f32, tag="b1", name="b1_t")
            nc.gpsimd.dma_start(C["b1"][:], b1_d.ap())
            C["b2"] = cp.tile([P, NO], f32, tag="b2", name="b2_t")
            nc.gpsimd.dma_start(C["b2"][:], b2_d.ap())
            C["c0"] = cp.tile([P, NO], f32, tag="c0", name="c0_t")
            nc.gpsimd.dma_start(C["c0"][:], c0_d.ap())

        w_tiles = [None] * NC_I
        w8_tiles = [None] * NPAIR

        def emit_wb_dma(c):
            w_t = wp.tile([P, D], bf16, tag=f"w{c}")
            nc.sync.dma_start(w_t[:], w_d.ap()[c])
            w_tiles[c] = w_t

        def emit_w8_dma(pair):
            w8_src = w8_d.ap()[pair].rearrange(
                "p (c8 j d) -> p c8 j d", c8=NSCH, j=2
            )
            tiles = []
            for c8 in range(NSCH):
                w8_t = wp.tile([P, 2, D], f8e4, tag=f"w8p{pair}c{c8}")
                nc.sync.dma_start(w8_t[:], w8_src[:, c8])
                tiles.append(w8_t)
            w8_tiles[pair] = tiles

        def emit_basis(ti, c, T, tok0, bn8_by_pair):
            """Basis channels for chunk c via the h/step reformulation.

            bn_p = h(clamp(x,0,1)) - step(x>2), bn_m = h(clamp(-x,0,1)) -
            step(x<-2), sigma = 1 - step+ - step-, with h(c) = c^2/(c^2 +
            (1-c)^2) = 0.5 + u/(2u^2+0.5), u = c-0.5. The 0.5 offsets and the
            W0 channel fold into a per-feature constant (c0) applied at the
            PSUM descale. v = u/(2u^2+0.5) and the steps are written as fp8
            into slot c%2 of the chunk-pair tiles for DoubleRow matmuls. The
            steps use the fp32 copy of x (the bf16 ulp at |x|=2 is 16x wider
            than the eps-rational transition band being approximated).
            """
            pair, j = c // 2, c % 2
            x_t = xp.tile([P, T], bf16, tag=f"x{c}")
            nc.sync.dma_start(x_t[:], xt_d.ap()[c, :, tok0:tok0 + T])
            x32_t = x32p.tile([P, T], f32, tag="x32")
            nc.sync.dma_start(x32_t[:], xt32_d.ap()[c, :, tok0:tok0 + T])
            if j == 0:
                bn8_by_pair[pair] = [
                    bn8p.tile([P, 2, T], f8e4, tag=f"bn8{k}{pair}",
                              name=f"bn8{k}{pair}_{ti}")
                    for k in ("vd", "vc", "km", "kp")
                ]
            vd8, vc8, km8, kp8 = bn8_by_pair[pair]
            uc_t = bp.tile([P, T], bf16, tag="uc")
            nc.vector.tensor_scalar(uc_t[:], x_t[:], 1.0, 0.0, OP.min, OP.max)
            nc.vector.tensor_scalar(uc_t[:], uc_t[:], -0.5, None, OP.add)
            ud_t = bp.tile([P, T], bf16, tag="ud")
            nc.vector.tensor_scalar(ud_t[:], x_t[:], 0.0, -1.0, OP.min, OP.mult)
            nc.vector.tensor_scalar(ud_t[:], ud_t[:], 1.0, -0.5, OP.min, OP.add)
            u2c = bp.tile([P, T], bf16, tag="u2c")
            nc.scalar.activation(u2c[:], uc_t[:], AF.Square, scale=2.0 ** 0.5)
            u2d = bp.tile([P, T], bf16, tag="u2d")
            nc.scalar.activation(u2d[:], ud_t[:], AF.Square, scale=2.0 ** 0.5)
            shc = bp.tile([P, T], f32, tag="shc")
            nc.scalar.activation(shc[:], u2c[:], AF.Copy, bias=0.5, scale=1.0)
            shd = bp.tile([P, T], f32, tag="shd")
            nc.scalar.activation(shd[:], u2d[:], AF.Copy, bias=0.5, scale=1.0)
            invc = bp.tile([P, T], f32, tag="invc")
            nc.vector.reciprocal_approx_fast(out=invc[:], in_=shc[:])
            invd = bp.tile([P, T], f32, tag="invd")
            nc.vector.reciprocal_approx_fast(out=invd[:], in_=shd[:])
            nc.gpsimd.tensor_tensor(vc8[:, j], uc_t[:], invc[:], OP.mult)
            nc.gpsimd.tensor_tensor(vd8[:, j], ud_t[:], invd[:], OP.mult)
            nc.scalar.activation(kp8[:, j], x32_t[:], AF.Sigmoid, bias=bk_t[:],
                                 scale=KSLOPE)
            nc.scalar.activation(km8[:, j], x32_t[:], AF.Sigmoid, bias=bk_t[:],
                                 scale=-KSLOPE)
            return x_t

        def emit_base_mains(ps, x_t, c, o0, no):
            """bf16 base-channel matmuls (x * 512Wb) for one chunk."""
            for oi in range(no):
                o = o0 + oi
                nc.tensor.matmul(
                    ps[oi][:],
                    lhsT=w_tiles[c][:, ts(o, P)],
                    rhs=x_t[:],
                    start=(c == 0),
                    stop=False,
                )

        def emit_spline_mains(ps, bn8, pair, o0, no, last, block_post, psl=None):
            """fp8 DoubleRow spline matmuls for one chunk pair (6 channels:
            3 bn tensors x hi/lo weights). On the final pair the emission is
            o-outer with per-block copies/stats chasing the stop."""
            w8 = w8_tiles[pair]
            # channels: vd*(Wm'-hi/lo), vc*(Wp'-hi/lo), km*(-Wm), kp*(-Wp)
            rhs = [bn8[0], bn8[0], bn8[1], bn8[1], bn8[2], bn8[3]]
            if not last:
                for c8 in range(NSCH):
                    for oi in range(no):
                        o = o0 + oi
                        nc.tensor.matmul(
                            ps[oi][:],
                            lhsT=w8[c8][:, :, ts(o, P)],
                            rhs=rhs[c8][:],
                            start=False,
                            stop=False,
                            perf_mode=PM.DoubleRow,
                        )
            else:
                for oi in range(no):
                    o = o0 + oi
                    for c8 in range(NSCH):
                        nc.tensor.matmul(
                            ps[oi][:],
                            lhsT=w8[c8][:, :, ts(o, P)],
                            rhs=rhs[c8][:],
                            start=False,
                            stop=(c8 == NSCH - 1),
                            perf_mode=PM.DoubleRow,
                        )
                    block_post(ps[oi], o)

        def emit_tile(ti, tok0, T, pending_aux, rhs_by_chunk, bn8_by_pair,
                      basis_next):
            """Emit one token tile's mains+stats; interleave prev tile's aux
            into group 0 and the NEXT tile's basis into group 1."""
            outs = [None] * NO
            sq = [None] * NO
            st = {}

            def get_psAB():
                # allocated lazily so the "aux" tag rotation matches runtime
                # order (after the previous tile's zb/mrb/psS allocations)
                if "psAB" not in st:
                    st["psAB"] = auxps.tile([33, T], f32, tag="aux",
                                            name=f"psAB_{ti}")
                return st["psAB"]

            def block_post(ps_ap, o):
                psAB = get_psAB()
                psA = psAB[0:1, :]
                psB = psAB[32:33, :]
                o_t = op_pool.tile([P, T], bf16, tag=f"out{o}", name=f"o_{ti}_{o}")
                nc.vector.tensor_scalar(o_t[:], ps_ap[:], 1.0 / 512.0,
                                        C["c0"][:, o:o + 1], OP.mult, OP.add)
                outs[o] = o_t
                sq_t = sqp.tile([P, T], bf16, tag="sq")
                nc.gpsimd.tensor_tensor(sq_t[:], o_t[:], o_t[:], OP.mult)
                sq[o] = sq_t
                nc.tensor.matmul(
                    psA, lhsT=C["onesc"][:], rhs=o_t[:],
                    start=(o == 0), stop=(o == NO - 1),
                )
                nc.tensor.matmul(
                    psB, lhsT=C["onesc"][:], rhs=sq_t[:],
                    start=(o == 0), stop=(o == NO - 1),
                )

            next_rhs = [None] * NC_I
            next_bn8 = [None] * NPAIR
            for gi, (o0, no) in enumerate(OGROUPS):
                ps = [
                    mps.tile([P, T], f32, tag="mps", name=f"ps_{ti}_{o0 + i}")
                    for i in range(no)
                ]
                if ti == 0 and gi == 0:
                    # startup: the base channel of every chunk only needs x +
                    # its weights, so run all 8 of those first while the basis
                    # pipeline fills; the spline pairs follow in a second pass
                    for c in range(NC_I):
                        rhs_by_chunk[c] = emit_basis(ti, c, T, tok0,
                                                     bn8_by_pair)
                        emit_wb_dma(c)
                        emit_base_mains(ps, rhs_by_chunk[c], c, o0, no)
                        if c == 0:
                            emit_consts()
                    for pair in range(NPAIR):
                        emit_w8_dma(pair)
                        emit_spline_mains(ps, bn8_by_pair[pair], pair, o0, no,
                                          last=(pair == NPAIR - 1),
                                          block_post=block_post)
                    continue
                for c in range(NC_I):
                    emit_base_mains(ps, rhs_by_chunk[c], c, o0, no)
                    if c % 2 == 1:
                        emit_spline_mains(ps, bn8_by_pair[c // 2], c // 2,
                                          o0, no, last=(c == NC_I - 1),
                                          block_post=block_post)
                    # interleave the previous tile's aux across both groups
                    if pending_aux is not None:
                        stage = pending_aux.get((gi, c))
                        if stage is not None:
                            stage()
                    # compute the NEXT tile's basis while this tile's mains
                    # run (this tile's basis was precomputed, so the basis
                    # engines are otherwise idle): chunks 0-3 late in group 0,
                    # chunks 4-7 early in group 1
                    if basis_next is not None and ti > 0:
                        ntok0, nT = basis_next
                        nc_ = None
                        if gi == 0 and c >= 3 and c < 7:
                            nc_ = c - 3
                        elif gi == 1 and c < 4:
                            nc_ = c + 4
                        elif gi == 0 and c == 7:
                            nc_ = None
                        if nc_ is not None:
                            next_rhs[nc_] = emit_basis(ti + 1, nc_, nT, ntok0,
                                                       next_bn8)
                    elif gi == 1 and basis_next is not None:
                        ntok0, nT = basis_next
                        next_rhs[c] = emit_basis(ti + 1, c, nT, ntok0,
                                                 next_bn8)

            return {"T": T, "tok0": tok0, "ti": ti, "psAB": st["psAB"],
                    "outs": outs, "sq": sq, "next_rhs": next_rhs,
                    "next_bn8": next_bn8}

        def make_aux(tile_st):
            """Aux stage emitters for a completed tile: stats chain -> LN -> SE."""
            T = tile_st["T"]
            ti = tile_st["ti"]
            tok0 = tile_st["tok0"]
            outs = tile_st["outs"]
            psAB = tile_st["psAB"]
            ctx_st = {}

            def chain():
                # negmu/e2 rows from psum, fp32 bit-hack rsqrt + 1 Newton step
                negmu = stp.tile([1, T], f32, tag="negmu")
                nc.vector.tensor_scalar(
                    negmu[:], psAB[0:1, :], -1.0, 0.0, OP.mult, OP.add
                )
                e2 = stp.tile([1, T], f32, tag="e2")
                nc.vector.tensor_scalar(
                    e2[:], psAB[32:33, :], 1.0, LN_EPS, OP.mult, OP.add
                )
                mu2 = stp.tile([1, T], f32, tag="mu2")
                nc.vector.tensor_tensor(mu2[:], negmu[:], negmu[:], OP.mult)
                var = stp.tile([1, T], f32, tag="var")
                nc.vector.tensor_tensor(var[:], e2[:], mu2[:], OP.subtract)
                zw = stp.tile([1, T], f32, tag="zw")
                nc.vector.tensor_scalar(
                    zw[:].bitcast(i32), var[:].bitcast(i32), 1, 0,
                    OP.arith_shift_right,
                )
                nc.vector.tensor_scalar(
                    zw[:].bitcast(i32), zw[:].bitcast(i32), -1, RSQRT_MAGIC32,
                    OP.mult, OP.add,
                )
                t1 = stp.tile([1, T], f32, tag="t1")
                nc.vector.tensor_tensor(t1[:], zw[:], zw[:], OP.mult)
                nc.vector.tensor_tensor(t1[:], t1[:], var[:], OP.mult)
                nc.vector.tensor_scalar(t1[:], t1[:], -0.5, 1.5, OP.mult, OP.add)
                z16 = stp.tile([1, T], bf16, tag="z16")
                nc.vector.tensor_tensor(z16[:], zw[:], t1[:], OP.mult)
                mr16 = stp.tile([1, T], bf16, tag="mr16")
                nc.vector.tensor_tensor(mr16[:], negmu[:], z16[:], OP.mult)
                # broadcast rows across partitions via K=1 outer products
                zbp = auxps.tile([P, T], f32, tag="aux", name=f"zb_{ti}")
                nc.tensor.matmul(zbp[:], lhsT=C["onesp"][:], rhs=z16[:],
                                 start=True, stop=True)
                zb16 = stp.tile([P, T], bf16, tag="zb16")
                nc.vector.tensor_copy(out=zb16[:], in_=zbp[:])
                mrp = auxps.tile([P, T], f32, tag="aux", name=f"mrb_{ti}")
                nc.tensor.matmul(mrp[:], lhsT=C["onesp"][:], rhs=mr16[:],
                                 start=True, stop=True)
                mrb16 = stp.tile([P, T], bf16, tag="mrb16")
                nc.scalar.activation(mrb16[:], mrp[:], AF.Copy)
                ctx_st["zb16"] = zb16
                ctx_st["mrb16"] = mrb16

            def ln():
                zb16, mrb16 = ctx_st["zb16"], ctx_st["mrb16"]
                psH = seps.tile([32, T], f32, tag="sps", name=f"psH_{ti}")
                yhat = []
                for o in range(NO):
                    q_t = sqp.tile([P, T], bf16, tag="q")
                    nc.vector.tensor_tensor(q_t[:], outs[o][:], zb16[:], OP.mult)
                    yh_t = yp.tile([P, T], bf16, tag=f"yh{o}")
                    nc.vector.tensor_tensor(yh_t[:], q_t[:], mrb16[:], OP.add)
                    yhat.append(yh_t)
                    nc.tensor.matmul(
                        psH[:], lhsT=C["w1t"][:, o, :], rhs=yh_t[:],
                        start=(o == 0), stop=(o == NO - 1),
                    )
                hr = sep.tile([32, T], bf16, tag="hr")
                nc.scalar.activation(hr[:], psH[:], AF.Relu, bias=C["b1"][:], scale=1.0)
                ctx_st["yhat"] = yhat
                ctx_st["hr"] = hr

            def se_blocks(olist):
                yhat, hr = ctx_st["yhat"], ctx_st["hr"]
                if "fin" not in ctx_st:
                    # one wide tile so all 8 out blocks leave in a single DMA
                    # (the descriptor engine processes DMAs serially)
                    ctx_st["fin"] = finp.tile([P, NO, T], bf16, tag="fin",
                                             name=f"fin_{ti}")
                fin_t = ctx_st["fin"]
                for o in olist:
                    # alternate psS between the two non-main PSUM banks so the
                    # W2 matmul of block o+1 overlaps the sigmoid of block o
                    pool, tg = (seps, "sps") if o % 2 else (auxps, "aux")
                    psS = pool.tile([P, T], f32, tag=tg, name=f"psS_{ti}_{o}")
                    nc.tensor.matmul(
                        psS[:], lhsT=C["w2t"][:, ts(o, P)], rhs=hr[:],
                        start=True, stop=True,
                    )
                    se_t = sep.tile([P, T], bf16, tag="se")
                    nc.scalar.activation(
                        se_t[:], psS[:], AF.Sigmoid, bias=C["b2"][:, o:o + 1], scale=1.0
                    )
                    yf_t = sep.tile([P, T], bf16, tag="yf")
                    nc.vector.tensor_scalar(
                        yf_t[:], yhat[o][:], C["lnw"][:, o:o + 1], C["lnb"][:, o:o + 1],
                        OP.mult, OP.add,
                    )
                    nc.vector.tensor_tensor(fin_t[:, o], yf_t[:], se_t[:], OP.mult)
                if olist[-1] == NO - 1:
                    nc.sync.dma_start(
                        out_d.ap()[:, :, tok0:tok0 + T].rearrange("o p t -> p o t"),
                        fin_t[:],
                    )

            return {"chain": chain, "ln": ln,
                    "se_a": lambda: se_blocks(range(0, 4)),
                    "se_b": lambda: se_blocks(range(4, NO))}

        AUX_SLOTS = {"chain": (0, 0), "ln": (0, 2), "se_a": (0, 5), "se_b": (1, 2)}

        pending = None
        rhs_cur = [None] * NC_I
        bn8_cur = [None] * NPAIR
        for ti, (tok0, T) in enumerate(TILES):
            basis_next = TILES[ti + 1] if ti + 1 < len(TILES) else None
            aux_by_slot = (
                {slot: pending[name] for name, slot in AUX_SLOTS.items()}
                if pending else None
            )
            tile_st = emit_tile(ti, tok0, T, aux_by_slot, rhs_cur, bn8_cur,
                                basis_next)
            rhs_cur = tile_st["next_rhs"]
            bn8_cur = tile_st["next_bn8"]
            pending = make_aux(tile_st)
        # final tile's aux runs at the end (smallest tile -> short tail)
        pending["chain"]()
        pending["ln"]()
        pending["se_a"]()
        pending["se_b"]()

    nc.compile()
    return nc


def _get_nc():
    if "nc" not in _cache:
        _cache["nc"] = _build_nc()
    return _cache["nc"]


def _prep_host(inputs):
    import ml_dtypes

    f = np.float32
    bf = ml_dtypes.bfloat16
    x = np.asarray(inputs["x"], f)
    base_weight = np.asarray(inputs["base_weight"], f)
    spline_weight = np.asarray(inputs["spline_weight"], f)
    ln_w = np.asarray(inputs["ln_w"], f)
    ln_b = np.asarray(inputs["ln_b"], f)
    se_w1 = np.asarray(inputs["se_w1"], f)
    se_b1 = np.asarray(inputs["se_b1"], f)
    se_w2 = np.asarray(inputs["se_w2"], f)
    se_b2 = np.asarray(inputs["se_b2"], f)

    f8 = ml_dtypes.float8_e4m3
    xt_all = x.reshape(N_CORES, NTOK, D).transpose(0, 2, 1)  # [core, D, ntok]

    # base channel: 512*Wb in bf16 (2^9 scale is exact); the 512 factor
    # matches the fp8 spline product scale so both share one PSUM bank
    w_base = (512.0 * base_weight.T).reshape(NC_I, P, D).astype(bf)

    wsT = spline_weight.transpose(1, 2, 0)  # [i, g, o]
    wm = wsT[:, 0, :].astype(f)
    w0 = wsT[:, 1, :].astype(f)
    wp = wsT[:, 2, :].astype(f)
    # h/step channels (x512): vd*(Wm-W0) [hi+lo], vc*(Wp-W0) [hi+lo],
    # km*(-Wm), kp*(-Wp); the 0.5 offsets of h and the W0 channel fold into
    # the per-feature constant c0 applied at the PSUM descale
    wmp = (wm - w0) * 512.0
    wpp = (wp - w0) * 512.0
    wmp_hi = wmp.astype(f8)
    wpp_hi = wpp.astype(f8)
    chans = [
        wmp_hi,
        (wmp - wmp_hi.astype(f)).astype(f8),
        wpp_hi,
        (wpp - wpp_hi.astype(f)).astype(f8),
        (-512.0 * wm).astype(f8),
        (-512.0 * wp).astype(f8),
    ]
    w8_all = np.empty((NPAIR, P, NSCH, 2, D), f8)
    for c8, wsrc in enumerate(chans):
        w8_all[:, :, c8, :, :] = wsrc.reshape(NPAIR, 2, P, D).transpose(0, 2, 1, 3)
    w8_all = np.ascontiguousarray(w8_all.reshape(NPAIR, P, NSCH * 2 * D))
    c0 = (w0.sum(axis=0) + 0.5 * (wm - w0).sum(axis=0)
          + 0.5 * (wp - w0).sum(axis=0)).astype(f)              # [o]

    w1p = (se_w1 * ln_w[None, :]).astype(f)          # ln_w folded into W1
    b1p = (se_b1 + se_w1 @ ln_b).astype(f)           # ln_b folded into b1

    shared = {
        "w": w_base,
        "w8": w8_all,
        "c0": np.ascontiguousarray(c0.reshape(NO, P).T),
        "w1t": np.ascontiguousarray(w1p.T.reshape(NO, P, 32)).astype(bf),
        "w2t": np.ascontiguousarray(se_w2.T).astype(bf),
        "lnw": np.ascontiguousarray(ln_w.reshape(NO, P).T),
        "lnb": np.ascontiguousarray(ln_b.reshape(NO, P).T),
        "onesc": np.full((P, 1), 1.0 / D, bf),
        "onesp": np.ones((1, P), bf),
        "b1": np.ascontiguousarray(b1p.reshape(32, 1)),
        "b2": np.ascontiguousarray(se_b2.reshape(NO, P).T),
    }
    in_maps = []
    for k in range(N_CORES):
        m = dict(shared)
        xk = np.ascontiguousarray(xt_all[k].reshape(NC_I, P, NTOK))
        m["xt"] = xk.astype(bf)
        m["xt32"] = xk
        in_maps.append(m)
    return in_maps


def kernel(**inputs) -> np.ndarray:
    from concourse.bass_utils import run_bass_kernel_spmd

    nc = _get_nc()
    in_maps = _prep_host(inputs)
    trace = bool(int(os.environ.get("KERNEL_TRACE", "0")))
    res = run_bass_kernel_spmd(
        nc, in_maps, core_ids=list(range(N_CORES)), trace=trace
    )
    _cache["last_result"] = res
    outs = []
    for k in range(N_CORES):
        outT = np.asarray(res.results[k]["outT"], dtype=np.float32)  # [NO, P, NTOK]
        outs.append(outT.reshape(D, NTOK).T)                          # [ntok, o]
    out = np.concatenate(outs, axis=0).reshape(8, 1024, 1024)
    return np.ascontiguousarray(out.astype(np.float32))


# revision 101
# speedup vs baseline: 1.4614x; 1.0069x over previous
"""Trainium2 Bass kernel for nn_EnhancedDRKANTreeNet (KAN layer + LayerNorm + SE gate).

Data-parallel over 8192 tokens across 8 NeuronCores (1024 tokens/core), all
compute feature-major: tiles are [feature_partition, token]. Token tiles are
[512, 384, 128] (the small final tile shrinks the serial LN/SE tail).

Main contraction, per output block of 128 features:
 - base channel x*(512*Wb) in bf16 (1 cycle/row on the PE);
 - spline channels in fp8e4 with DoubleRow perf mode (2 contraction rows per
   partition, 0.5 cycles/row). The grid-3 quadratic-spline basis is
   reformulated as bn_p = h(clamp(x,0,1)) - step(x>2), bn_m = h(clamp(-x,0,1))
   - step(x<-2), sigma = 1 - step+ - step-, with h(c) = 0.5 + u/(2u^2+0.5),
   u = c - 0.5 (exactly the eps-regularized normalized basis away from a
   ~1e-3-wide band at |x|=2). Channels: v_d*(Wm-W0)[fp8 hi+lo], v_c*(Wp-W0)
   [fp8 hi+lo], step-*(-Wm), step+*(-Wp); the 0.5 offsets of h and the W0
   channel fold into a per-feature constant c0 added at the PSUM descale.
   Steps are ACT sigmoids with slope 4400 evaluated on an fp32 copy of x
   (the bf16 ulp at |x|=2 is wider than the band being approximated).
   Weights are loaded into SBUF once; PSUM accumulates everything at 512x.

Elementwise engine placement (balanced ~80us each): ACT does the two
h-squares, the 2u^2+0.5 Copy-biases and the step sigmoids; DVE does clamps,
reciprocals (18-bit approx), PSUM descales, LN apply and stats chain; the
gpsimd/Pool engine does the v=u*inv fp8 writes and out^2 squares.

PSUM (8 banks): main accumulation is split into o-groups of 6+2 banks plus
one stats/broadcast bank (shared-tag rotation: psAB -> zb -> mrb -> psS-even)
and one SE bank, so tile t's LayerNorm/SE aux runs concurrently with tile
t+1's accumulation; aux stages are emitted interleaved into tile t+1's chunk
loop, and tile t+1's basis is produced during tile t's mains.

LayerNorm: per-token stats via ones-matmuls (mean folded into the ones
scale), fp32 int32-bit-hack rsqrt seed + one Newton step, then
y = out*(ones (x) z) + (ones (x) -mu*z); ln_w/ln_b are applied per-feature by
a 4x-rate tensor_scalar op (ln_w is also folded into the SE W1 on the host,
ln_b into the SE b1). SE: h = relu(W1'.y + b1'), se = sigmoid(W2.h + b2),
final = (ln_w*y + ln_b)*se, stored bf16 and shipped per-tile in one DMA.
"""

import os
from contextlib import ExitStack

import numpy as np

P = 128
D = 1024
NC_I = 8           # contraction chunks of 128 over D_IN
NPAIR = 4          # chunk pairs for fp8 DoubleRow spline matmuls
NSCH = 6           # spline DR channels: (bnm, bnp, sg) x (w_hi, w_lo)
NO = 8             # output-feature chunks of 128
NTOK = 1024        # tokens per core
N_CORES = 8
TILES = [(0, 512), (512, 384), (896, 128)]   # (tok0, T) per token tile
OGROUPS = [(0, 6), (6, 2)]                   # o-block groups (start, count)
KSLOPE = 4400.0    # step-channel sigmoid slope (matches the eps=1e-6
                   # rational transition width sqrt(eps)~1e-3 at |x|=2)
LN_EPS = 1e-5
RSQRT_MAGIC32 = 0x5F3759DF   # fp32 rsqrt bit-hack seed

_cache = {}


def _build_nc():
    import concourse.bass as bass
    import concourse.mybir as mybir
    import concourse.tile as tile
    from concourse import bacc

    f32 = mybir.dt.float32
    bf16 = mybir.dt.bfloat16
    f8e4 = mybir.dt.float8e4
    PM = mybir.MatmulPerfMode
    i32 = mybir.dt.int32
    AF = mybir.ActivationFunctionType
    OP = mybir.AluOpType
    ts = bass.ts

    nc = bacc.Bacc(
        "TRN2",
        target_bir_lowering=False,
        debug=False,
        enable_asserts=False,
        num_devices=N_CORES,
    )

    xt_d = nc.dram_tensor("xt", [NC_I, P, NTOK], bf16, kind="ExternalInput")
    xt32_d = nc.dram_tensor("xt32", [NC_I, P, NTOK], f32, kind="ExternalInput")
    c0_d = nc.dram_tensor("c0", [P, NO], f32, kind="ExternalInput")
    w_d = nc.dram_tensor("w", [NC_I, P, D], bf16, kind="ExternalInput")
    w8_d = nc.dram_tensor("w8", [NPAIR, P, NSCH * 2 * D], f8e4,
                          kind="ExternalInput")
    w1t_d = nc.dram_tensor("w1t", [NO, P, 32], bf16, kind="ExternalInput")
    w2t_d = nc.dram_tensor("w2t", [32, D], bf16, kind="ExternalInput")
    lnw_d = nc.dram_tensor("lnw", [P, NO], f32, kind="ExternalInput")
    lnb_d = nc.dram_tensor("lnb", [P, NO], f32, kind="ExternalInput")
    onesc_d = nc.dram_tensor("onesc", [P, 1], bf16, kind="ExternalInput")  # 1/D
    onesp_d = nc.dram_tensor("onesp", [1, P], bf16, kind="ExternalInput")  # 1.0
    b1_d = nc.dram_tensor("b1", [32, 1], f32, kind="ExternalInput")
    b2_d = nc.dram_tensor("b2", [P, NO], f32, kind="ExternalInput")
    out_d = nc.dram_tensor("outT", [NO, P, NTOK], bf16, kind="ExternalOutput")

    with tile.TileContext(nc) as tc, ExitStack() as ctx:
        cp = ctx.enter_context(tc.tile_pool(name="cp", bufs=1))
        wp = ctx.enter_context(tc.tile_pool(name="wp", bufs=1))
        xp = ctx.enter_context(tc.tile_pool(name="xp", bufs=2))
        bn8p = ctx.enter_context(tc.tile_pool(name="bn8p", bufs=2))
        bp = ctx.enter_context(tc.tile_pool(name="bp", bufs=2))
        op_pool = ctx.enter_context(tc.tile_pool(name="op", bufs=1))
        sqp = ctx.enter_context(tc.tile_pool(name="sqp", bufs=2))
        x32p = ctx.enter_context(tc.tile_pool(name="x32p", bufs=2))
        stp = ctx.enter_context(tc.tile_pool(name="stp", bufs=1))
        yp = ctx.enter_context(tc.tile_pool(name="yp", bufs=1))
        sep = ctx.enter_context(tc.tile_pool(name="sep", bufs=2))
        finp = ctx.enter_context(tc.tile_pool(name="finp", bufs=1))
        # PSUM: 6 (main) + 1 (stats/broadcast, shared tag) + 1 (SE) = 8 banks
        mps = ctx.enter_context(tc.tile_pool(name="mps", bufs=6, space="PSUM"))
        auxps = ctx.enter_context(tc.tile_pool(name="auxps", bufs=1, space="PSUM"))
        seps = ctx.enter_context(tc.tile_pool(name="seps", bufs=1, space="PSUM"))

        # warm the sigmoid_and_others ACT table at t=0 (overlaps initial DMA)
        warm_t = cp.tile([P, 1], f32, tag="warm")
        nc.scalar.activation(warm_t[:], nc.const_aps.tensor(1.0, (P, 1)), AF.Relu)

        bk_t = cp.tile([P, 1], f32, tag="bk")
        nc.gpsimd.memset(bk_t[:], -2.0 * KSLOPE)
        half_t = cp.tile([P, 512], bf16, tag="half")
        nc.gpsimd.memset(half_t[:], 0.5)

        C = {}  # constant tiles, DMA'd after the first chunk's x/w (startup)

        def emit_consts():
            C["w1t"] = cp.tile([P, NO, 32], bf16, tag="w1t", name="w1t_t")
            nc.gpsimd.dma_start(C["w1t"][:], w1t_d.ap().rearrange("c p j -> p c j"))
            C["w2t"] = cp.tile([32, D], bf16, tag="w2t", name="w2t_t")
            nc.gpsimd.dma_start(C["w2t"][:], w2t_d.ap())
            C["lnw"] = cp.tile([P, NO], f32, tag="lnw", name="lnw_t")
            nc.gpsimd.dma_start(C["lnw"][:], lnw_d.ap())
            C["lnb"] = cp.tile([P, NO], f32, tag="lnb", name="lnb_t")
            nc.gpsimd.dma_start(C["lnb"][:], lnb_d.ap())
            C["onesc"] = cp.tile([P, 1], bf16, tag="onesc", name="onesc_t")
            nc.gpsimd.dma_start(C["onesc"][:], onesc_d.ap())
            C["onesp"] = cp.tile([1, P], bf16, tag="onesp", name="onesp_t")
            nc.gpsimd.dma_start(C["onesp"][:], onesp_d.ap())
            C["b1"] = cp.tile([32, 1], f32, tag="b1", name="b1_t")
            nc.gpsimd.dma_start(C["b1"][:], b1_d.ap())
            C["b2"] = cp.tile([P, NO], f32, tag="b2", name="b2_t")
            nc.gpsimd.dma_start(C["b2"][:], b2_d.ap())
            C["c0"] = cp.tile([P, NO], f32, tag="c0", name="c0_t")
            nc.gpsimd.dma_start(C["c0"][:], c0_d.ap())

        w_tiles = [None] * NC_I
        w8_tiles = [None] * NPAIR

        def emit_wb_dma(c):
            w_t = wp.tile([P, D], bf16, tag=f"w{c}")
            nc.sync.dma_start(w_t[:], w_d.ap()[c])
            w_tiles[c] = w_t

        def emit_w8_dma(pair):
            w8_src = w8_d.ap()[pair].rearrange(
                "p (c8 j d) -> p c8 j d", c8=NSCH, j=2
            )
            tiles = []
            for c8 in range(NSCH):
                w8_t = wp.tile([P, 2, D], f8e4, tag=f"w8p{pair}c{c8}")
                nc.sync.dma_start(w8_t[:], w8_src[:, c8])
                tiles.append(w8_t)
            w8_tiles[pair] = tiles

        def emit_basis(ti, c, T, tok0, bn8_by_pair):
            """Basis channels for chunk c via the h/step reformulation.

            bn_p = h(clamp(x,0,1)) - step(x>2), bn_m = h(clamp(-x,0,1)) -
            step(x<-2), sigma = 1 - step+ - step-, with h(c) = c^2/(c^2 +
            (1-c)^2) = 0.5 + u/(2u^2+0.5), u = c-0.5. The 0.5 offsets and the
            W0 channel fold into a per-feature constant (c0) applied at the
            PSUM descale. v = u/(2u^2+0.5) and the steps are written as fp8
            into slot c%2 of the chunk-pair tiles for DoubleRow matmuls. The
            steps use the fp32 copy of x (the bf16 ulp at |x|=2 is 16x wider
            than the eps-rational transition band being approximated).
            """
            pair, j = c // 2, c % 2
            x_t = xp.tile([P, T], bf16, tag=f"x{c}")
            nc.sync.dma_start(x_t[:], xt_d.ap()[c, :, tok0:tok0 + T])
            x32_t = x32p.tile([P, T], f32, tag="x32")
            nc.sync.dma_start(x32_t[:], xt32_d.ap()[c, :, tok0:tok0 + T])
            if j == 0:
                bn8_by_pair[pair] = [
                    bn8p.tile([P, 2, T], f8e4, tag=f"bn8{k}{pair}",
                              name=f"bn8{k}{pair}_{ti}")
                    for k in ("vd", "vc", "km", "kp")
                ]
            vd8, vc8, km8, kp8 = bn8_by_pair[pair]
            uc_t = bp.tile([P, T], bf16, tag="uc")
            nc.vector.tensor_scalar(uc_t[:], x_t[:], 1.0, 0.0, OP.min, OP.max)
            nc.vector.tensor_scalar(uc_t[:], uc_t[:], -0.5, None, OP.add)
            ud_t = bp.tile([P, T], bf16, tag="ud")
            nc.vector.tensor_scalar(ud_t[:], x_t[:], 0.0, -1.0, OP.min, OP.mult)
            nc.vector.tensor_scalar(ud_t[:], ud_t[:], 1.0, -0.5, OP.min, OP.add)
            u2c = bp.tile([P, T], bf16, tag="u2c")
            nc.scalar.activation(u2c[:], uc_t[:], AF.Square, scale=2.0 ** 0.5)
            u2d = bp.tile([P, T], bf16, tag="u2d")
            nc.scalar.activation(u2d[:], ud_t[:], AF.Square, scale=2.0 ** 0.5)
            shc = bp.tile([P, T], f32, tag="shc")
            nc.scalar.activation(shc[:], u2c[:], AF.Copy, bias=0.5, scale=1.0)
            shd = bp.tile([P, T], f32, tag="shd")
            nc.scalar.activation(shd[:], u2d[:], AF.Copy, bias=0.5, scale=1.0)
            invc = bp.tile([P, T], f32, tag="invc")
            nc.vector.reciprocal_approx_fast(out=invc[:], in_=shc[:])
            invd = bp.tile([P, T], f32, tag="invd")
            nc.vector.reciprocal_approx_fast(out=invd[:], in_=shd[:])
            nc.gpsimd.tensor_tensor(vc8[:, j], uc_t[:], invc[:], OP.mult)
            nc.gpsimd.tensor_tensor(vd8[:, j], ud_t[:], invd[:], OP.mult)
            nc.scalar.activation(kp8[:, j], x32_t[:], AF.Sigmoid, bias=bk_t[:],
                                 scale=KSLOPE)
            nc.scalar.activation(km8[:, j], x32_t[:], AF.Sigmoid, bias=bk_t[:],
                                 scale=-KSLOPE)
            return x_t

        def emit_base_mains(ps, x_t, c, o0, no):
            """bf16 base-channel matmuls (x * 512Wb) for one chunk."""
            for oi in range(no):
                o = o0 + oi
                nc.tensor.matmul(
                    ps[oi][:],
                    lhsT=w_tiles[c][:, ts(o, P)],
                    rhs=x_t[:],
                    start=(c == 0),
                    stop=False,
                )

        def emit_spline_mains(ps, bn8, pair, o0, no, last, block_post, psl=None):
            """fp8 DoubleRow spline matmuls for one chunk pair (6 channels:
            3 bn tensors x hi/lo weights). On the final pair the emission is
            o-outer with per-block copies/stats chasing the stop."""
            w8 = w8_tiles[pair]
            # channels: vd*(Wm'-hi/lo), vc*(Wp'-hi/lo), km*(-Wm), kp*(-Wp)
            rhs = [bn8[0], bn8[0], bn8[1], bn8[1], bn8[2], bn8[3]]
            if not last:
                for c8 in range(NSCH):
                    for oi in range(no):
                        o = o0 + oi
                        nc.tensor.matmul(
                            ps[oi][:],
                            lhsT=w8[c8][:, :, ts(o, P)],
                            rhs=rhs[c8][:],
                            start=False,
                            stop=False,
                            perf_mode=PM.DoubleRow,
                        )
            else:
                for oi in range(no):
                    o = o0 + oi
                    for c8 in range(NSCH):
                        nc.tensor.matmul(
                            ps[oi][:],
                            lhsT=w8[c8][:, :, ts(o, P)],
                            rhs=rhs[c8][:],
                            start=False,
                            stop=(c8 == NSCH - 1),
                            perf_mode=PM.DoubleRow,
                        )
                    block_post(ps[oi], o)

        def emit_tile(ti, tok0, T, pending_aux, rhs_by_chunk, bn8_by_pair,
                      basis_next):
            """Emit one token tile's mains+stats; interleave prev tile's aux
            into group 0 and the NEXT tile's basis into group 1."""
            outs = [None] * NO
            sq = [None] * NO
            st = {}

            def get_psAB():
                # allocated lazily so the "aux" tag rotation matches runtime
                # order (after the previous tile's zb/mrb/psS allocations)
                if "psAB" not in st:
                    st["psAB"] = auxps.tile([33, T], f32, tag="aux",
                                            name=f"psAB_{ti}")
                return st["psAB"]

            def block_post(ps_ap, o):
                psAB = get_psAB()
                psA = psAB[0:1, :]
                psB = psAB[32:33, :]
                o_t = op_pool.tile([P, T], bf16, tag=f"out{o}", name=f"o_{ti}_{o}")
                nc.vector.tensor_scalar(o_t[:], ps_ap[:], 1.0 / 512.0,
                                        C["c0"][:, o:o + 1], OP.mult, OP.add)
                outs[o] = o_t
                sq_t = sqp.tile([P, T], bf16, tag="sq")
                nc.gpsimd.tensor_tensor(sq_t[:], o_t[:], o_t[:], OP.mult)
                sq[o] = sq_t
                nc.tensor.matmul(
                    psA, lhsT=C["onesc"][:], rhs=o_t[:],
                    start=(o == 0), stop=(o == NO - 1),
                )
                nc.tensor.matmul(
                    psB, lhsT=C["onesc"][:], rhs=sq_t[:],
                    start=(o == 0), stop=(o == NO - 1),
                )

            next_rhs = [None] * NC_I
            next_bn8 = [None] * NPAIR
            for gi, (o0, no) in enumerate(OGROUPS):
                ps = [
                    mps.tile([P, T], f32, tag="mps", name=f"ps_{ti}_{o0 + i}")
                    for i in range(no)
                ]
                if ti == 0 and gi == 0:
                    # startup: the base channel of every chunk only needs x +
                    # its weights, so run all 8 of those first while the basis
                    # pipeline fills; the spline pairs follow in a second pass
                    for c in range(NC_I):
                        rhs_by_chunk[c] = emit_basis(ti, c, T, tok0,
                                                     bn8_by_pair)
                        emit_wb_dma(c)
                        emit_base_mains(ps, rhs_by_chunk[c], c, o0, no)
                        if c == 0:
                            emit_consts()
                    for pair in range(NPAIR):
                        emit_w8_dma(pair)
                        emit_spline_mains(ps, bn8_by_pair[pair], pair, o0, no,
                                          last=(pair == NPAIR - 1),
                                          block_post=block_post)
                    continue
                for c in range(NC_I):
                    emit_base_mains(ps, rhs_by_chunk[c], c, o0, no)
                    if c % 2 == 1:
                        emit_spline_mains(ps, bn8_by_pair[c // 2], c // 2,
                                          o0, no, last=(c == NC_I - 1),
                                          block_post=block_post)
                    # interleave the previous tile's aux across both groups
                    if pending_aux is not None:
                        stage = pending_aux.get((gi, c))
                        if stage is not None:
                            stage()
                    # compute the NEXT tile's basis while this tile's mains
                    # run (this tile's basis was precomputed, so the basis
                    # engines are otherwise idle): chunks 0-3 late in group 0,
                    # chunks 4-7 early in group 1
                    if basis_next is not None and ti > 0:
                        ntok0, nT = basis_next
                        nc_ = None
                        if gi == 0 and c >= 3 and c < 7:
                            nc_ = c - 3
                        elif gi == 1 and c < 4:
                            nc_ = c + 4
                        elif gi == 0 and c == 7:
                            nc_ = None
                        if nc_ is not None:
                            next_rhs[nc_] = emit_basis(ti + 1, nc_, nT, ntok0,
                                                       next_bn8)
                    elif gi == 1 and basis_next is not None:
                        ntok0, nT = basis_next
                        next_rhs[c] = emit_basis(ti + 1, c, nT, ntok0,
                                                 next_bn8)

            return {"T": T, "tok0": tok0, "ti": ti, "psAB": st["psAB"],
                    "outs": outs, "sq": sq, "next_rhs": next_rhs,
                    "next_bn8": next_bn8}

        def make_aux(tile_st):
            """Aux stage emitters for a completed tile: stats chain -> LN -> SE."""
            T = tile_st["T"]
            ti = tile_st["ti"]
            tok0 = tile_st["tok0"]
            outs = tile_st["outs"]
            psAB = tile_st["psAB"]
            ctx_st = {}

            def chain():
                # negmu/e2 rows from psum, fp32 bit-hack rsqrt + 1 Newton step
                negmu = stp.tile([1, T], f32, tag="negmu")
                nc.vector.tensor_scalar(
                    negmu[:], psAB[0:1, :], -1.0, 0.0, OP.mult, OP.add
                )
                e2 = stp.tile([1, T], f32, tag="e2")
                nc.vector.tensor_scalar(
                    e2[:], psAB[32:33, :], 1.0, LN_EPS, OP.mult, OP.add
                )
                mu2 = stp.tile([1, T], f32, tag="mu2")
                nc.vector.tensor_tensor(mu2[:], negmu[:], negmu[:], OP.mult)
                var = stp.tile([1, T], f32, tag="var")
                nc.vector.tensor_tensor(var[:], e2[:], mu2[:], OP.subtract)
                zw = stp.tile([1, T], f32, tag="zw")
                nc.vector.tensor_scalar(
                    zw[:].bitcast(i32), var[:].bitcast(i32), 1, 0,
                    OP.arith_shift_right,
                )
                nc.vector.tensor_scalar(
                    zw[:].bitcast(i32), zw[:].bitcast(i32), -1, RSQRT_MAGIC32,
                    OP.mult, OP.add,
                )
                t1 = stp.tile([1, T], f32, tag="t1")
                nc.vector.tensor_tensor(t1[:], zw[:], zw[:], OP.mult)
                nc.vector.tensor_tensor(t1[:], t1[:], var[:], OP.mult)
                nc.vector.tensor_scalar(t1[:], t1[:], -0.5, 1.5, OP.mult, OP.add)
                z16 = stp.tile([1, T], bf16, tag="z16")
                nc.vector.tensor_tensor(z16[:], zw[:], t1[:], OP.mult)
                mr16 = stp.tile([1, T], bf16, tag="mr16")
                nc.vector.tensor_tensor(mr16[:], negmu[:], z16[:], OP.mult)
                # broadcast rows across partitions via K=1 outer products
                zbp = auxps.tile([P, T], f32, tag="aux", name=f"zb_{ti}")
                nc.tensor.matmul(zbp[:], lhsT=C["onesp"][:], rhs=z16[:],
                                 start=True, stop=True)
                zb16 = stp.tile([P, T], bf16, tag="zb16")
                nc.vector.tensor_copy(out=zb16[:], in_=zbp[:])
                mrp = auxps.tile([P, T], f32, tag="aux", name=f"mrb_{ti}")
                nc.tensor.matmul(mrp[:], lhsT=C["onesp"][:], rhs=mr16[:],
                                 start=True, stop=True)
                mrb16 = stp.tile([P, T], bf16, tag="mrb16")
                nc.scalar.activation(mrb16[:], mrp[:], AF.Copy)
                ctx_st["zb16"] = zb16
                ctx_st["mrb16"] = mrb16

            def ln():
                zb16, mrb16 = ctx_st["zb16"], ctx_st["mrb16"]
                psH = seps.tile([32, T], f32, tag="sps", name=f"psH_{ti}")
                yhat = []
                for o in range(NO):
                    q_t = sqp.tile([P, T], bf16, tag="q")
                    nc.vector.tensor_tensor(q_t[:], outs[o][:], zb16[:], OP.mult)
                    yh_t = yp.tile([P, T], bf16, tag=f"yh{o}")
                    nc.vector.tensor_tensor(yh_t[:], q_t[:], mrb16[:], OP.add)
                    yhat.append(yh_t)
                    nc.tensor.matmul(
                        psH[:], lhsT=C["w1t"][:, o, :], rhs=yh_t[:],
                        start=(o == 0), stop=(o == NO - 1),
                    )
                hr = sep.tile([32, T], bf16, tag="hr")
                nc.scalar.activation(hr[:], psH[:], AF.Relu, bias=C["b1"][:], scale=1.0)
                ctx_st["yhat"] = yhat
                ctx_st["hr"] = hr

            def se_blocks(olist):
                yhat, hr = ctx_st["yhat"], ctx_st["hr"]
                if "fin" not in ctx_st:
                    # one wide tile so all 8 out blocks leave in a single DMA
                    # (the descriptor engine processes DMAs serially)
                    ctx_st["fin"] = finp.tile([P, NO, T], bf16, tag="fin",
                                             name=f"fin_{ti}")
                fin_t = ctx_st["fin"]
                for o in olist:
                    # alternate psS between the two non-main PSUM banks so the
                    # W2 matmul of block o+1 overlaps the sigmoid of block o
                    pool, tg = (seps, "sps") if o % 2 else (auxps, "aux")
                    psS = pool.tile([P, T], f32, tag=tg, name=f"psS_{ti}_{o}")
                    nc.tensor.matmul(
                        psS[:], lhsT=C["w2t"][:, ts(o, P)], rhs=hr[:],
                        start=True, stop=True,
                    )
                    yf_t = sep.tile([P, T], bf16, tag="yf")
                    nc.vector.tensor_scalar(
                        yf_t[:], yhat[o][:], C["lnw"][:, o:o + 1], C["lnb"][:, o:o + 1],
                        OP.mult, OP.add,
                    )
                    se_t = sep.tile([P, T], bf16, tag="se")
                    nc.scalar.activation(
                        se_t[:], psS[:], AF.Sigmoid, bias=C["b2"][:, o:o + 1], scale=1.0
                    )
                    nc.vector.tensor_tensor(fin_t[:, o], yf_t[:], se_t[:], OP.mult)
                h0, h1 = olist[0], olist[-1] + 1
                nc.sync.dma_start(
                    out_d.ap()[h0:h1, :, tok0:tok0 + T].rearrange("o p t -> p o t"),
                    fin_t[:, h0:h1],
                )

            return {"chain": chain, "ln": ln,
                    "se_a": lambda: se_blocks(range(0, 4)),
                    "se_b": lambda: se_blocks(range(4, NO))}

        AUX_SLOTS = {"chain": (0, 0), "ln": (0, 2), "se_a": (0, 5), "se_b": (1, 2)}

        pending = None
        rhs_cur = [None] * NC_I
        bn8_cur = [None] * NPAIR
        for ti, (tok0, T) in enumerate(TILES):
            basis_next = TILES[ti + 1] if ti + 1 < len(TILES) else None
            aux_by_slot = (
                {slot: pending[name] for name, slot in AUX_SLOTS.items()}
                if pending else None
            )
            tile_st = emit_tile(ti, tok0, T, aux_by_slot, rhs_cur, bn8_cur,
                                basis_next)
            rhs_cur = tile_st["next_rhs"]
            bn8_cur = tile_st["next_bn8"]
            pending = make_aux(tile_st)
        # final tile's aux runs at the end (smallest tile -> short tail)
        pending["chain"]()
        pending["ln"]()
        pending["se_a"]()
        pending["se_b"]()

    nc.compile()
    return nc


def _get_nc():
    if "nc" not in _cache:
        _cache["nc"] = _build_nc()
    return _cache["nc"]


def _prep_host(inputs):
    import ml_dtypes

    f = np.float32
    bf = ml_dtypes.bfloat16
    x = np.asarray(inputs["x"], f)
    base_weight = np.asarray(inputs["base_weight"], f)
    spline_weight = np.asarray(inputs["spline_weight"], f)
    ln_w = np.asarray(inputs["ln_w"], f)
    ln_b = np.asarray(inputs["ln_b"], f)
    se_w1 = np.asarray(inputs["se_w1"], f)
    se_b1 = np.asarray(inputs["se_b1"], f)
    se_w2 = np.asarray(inputs["se_w2"], f)
    se_b2 = np.asarray(inputs["se_b2"], f)

    f8 = ml_dtypes.float8_e4m3
    xt_all = x.reshape(N_CORES, NTOK, D).transpose(0, 2, 1)  # [core, D, ntok]

    # base channel: 512*Wb in bf16 (2^9 scale is exact); the 512 factor
    # matches the fp8 spline product scale so both share one PSUM bank
    w_base = (512.0 * base_weight.T).reshape(NC_I, P, D).astype(bf)

    wsT = spline_weight.transpose(1, 2, 0)  # [i, g, o]
    wm = wsT[:, 0, :].astype(f)
    w0 = wsT[:, 1, :].astype(f)
    wp = wsT[:, 2, :].astype(f)
    # h/step channels (x512): vd*(Wm-W0) [hi+lo], vc*(Wp-W0) [hi+lo],
    # km*(-Wm), kp*(-Wp); the 0.5 offsets of h and the W0 channel fold into
    # the per-feature constant c0 applied at the PSUM descale
    wmp = (wm - w0) * 512.0
    wpp = (wp - w0) * 512.0
    wmp_hi = wmp.astype(f8)
    wpp_hi = wpp.astype(f8)
    chans = [
        wmp_hi,
        (wmp - wmp_hi.astype(f)).astype(f8),
        wpp_hi,
        (wpp - wpp_hi.astype(f)).astype(f8),
        (-512.0 * wm).astype(f8),
        (-512.0 * wp).astype(f8),
    ]
    w8_all = np.empty((NPAIR, P, NSCH, 2, D), f8)
    for c8, wsrc in enumerate(chans):
        w8_all[:, :, c8, :, :] = wsrc.reshape(NPAIR, 2, P, D).transpose(0, 2, 1, 3)
    w8_all = np.ascontiguousarray(w8_all.reshape(NPAIR, P, NSCH * 2 * D))
    c0 = (w0.sum(axis=0) + 0.5 * (wm - w0).sum(axis=0)
          + 0.5 * (wp - w0).sum(axis=0)).astype(f)              # [o]

    w1p = (se_w1 * ln_w[None, :]).astype(f)          # ln_w folded into W1
    b1p = (se_b1 + se_w1 @ ln_b).astype(f)           # ln_b folded into b1

    shared = {
        "w": w_base,
        "w8": w8_all,
        "c0": np.ascontiguousarray(c0.reshape(NO, P).T),
        "w1t": np.ascontiguousarray(w1p.T.reshape(NO, P, 32)).astype(bf),
        "w2t": np.ascontiguousarray(se_w2.T).astype(bf),
        "lnw": np.ascontiguousarray(ln_w.reshape(NO, P).T),
        "lnb": np.ascontiguousarray(ln_b.reshape(NO, P).T),
        "onesc": np.full((P, 1), 1.0 / D, bf),
        "onesp": np.ones((1, P), bf),
        "b1": np.ascontiguousarray(b1p.reshape(32, 1)),
        "b2": np.ascontiguousarray(se_b2.reshape(NO, P).T),
    }
    in_maps = []
    for k in range(N_CORES):
        m = dict(shared)
        xk = np.ascontiguousarray(xt_all[k].reshape(NC_I, P, NTOK))
        m["xt"] = xk.astype(bf)
        m["xt32"] = xk
        in_maps.append(m)
    return in_maps


def kernel(**inputs) -> np.ndarray:
    from concourse.bass_utils import run_bass_kernel_spmd

    nc = _get_nc()
    in_maps = _prep_host(inputs)
    trace = bool(int(os.environ.get("KERNEL_TRACE", "0")))
    res = run_bass_kernel_spmd(
        nc, in_maps, core_ids=list(range(N_CORES)), trace=trace
    )
    _cache["last_result"] = res
    outs = []
    for k in range(N_CORES):
        outT = np.asarray(res.results[k]["outT"], dtype=np.float32)  # [NO, P, NTOK]
        outs.append(outT.reshape(D, NTOK).T)                          # [ntok, o]
    out = np.concatenate(outs, axis=0).reshape(8, 1024, 1024)
    return np.ascontiguousarray(out.astype(np.float32))


# revision 107
# speedup vs baseline: 1.4624x; 1.0007x over previous
"""Trainium2 Bass kernel for nn_EnhancedDRKANTreeNet (KAN layer + LayerNorm + SE gate).

Data-parallel over 8192 tokens across 8 NeuronCores (1024 tokens/core), all
compute feature-major: tiles are [feature_partition, token]. Token tiles are
[512, 384, 128] (the small final tile shrinks the serial LN/SE tail).

Main contraction, per output block of 128 features:
 - base channel x*(512*Wb) in bf16 (1 cycle/row on the PE);
 - spline channels in fp8e4 with DoubleRow perf mode (2 contraction rows per
   partition, 0.5 cycles/row). The grid-3 quadratic-spline basis is
   reformulated as bn_p = h(clamp(x,0,1)) - step(x>2), bn_m = h(clamp(-x,0,1))
   - step(x<-2), sigma = 1 - step+ - step-, with h(c) = 0.5 + u/(2u^2+0.5),
   u = c - 0.5 (exactly the eps-regularized normalized basis away from a
   ~1e-3-wide band at |x|=2). Channels: v_d*(Wm-W0)[fp8 hi+lo], v_c*(Wp-W0)
   [fp8 hi+lo], step-*(-Wm), step+*(-Wp); the 0.5 offsets of h and the W0
   channel fold into a per-feature constant c0 added at the PSUM descale.
   Steps are ACT sigmoids with slope 4400 evaluated on an fp32 copy of x
   (the bf16 ulp at |x|=2 is wider than the band being approximated).
   Weights are loaded into SBUF once; PSUM accumulates everything at 512x.

Elementwise engine placement (balanced ~80us each): ACT does the two
h-squares, the 2u^2+0.5 Copy-biases and the step sigmoids; DVE does clamps,
reciprocals (18-bit approx), PSUM descales, LN apply and stats chain; the
gpsimd/Pool engine does the v=u*inv fp8 writes and out^2 squares.

PSUM (8 banks): main accumulation is split into o-groups of 6+2 banks plus
one stats/broadcast bank (shared-tag rotation: psAB -> zb -> mrb -> psS-even)
and one SE bank, so tile t's LayerNorm/SE aux runs concurrently with tile
t+1's accumulation; aux stages are emitted interleaved into tile t+1's chunk
loop, and tile t+1's basis is produced during tile t's mains.

LayerNorm: per-token stats via ones-matmuls (mean folded into the ones
scale), fp32 int32-bit-hack rsqrt seed + one Newton step, then
y = out*(ones (x) z) + (ones (x) -mu*z); ln_w/ln_b are applied per-feature by
a 4x-rate tensor_scalar op (ln_w is also folded into the SE W1 on the host,
ln_b into the SE b1). SE: h = relu(W1'.y + b1'), se = sigmoid(W2.h + b2),
final = (ln_w*y + ln_b)*se, stored bf16 and shipped per-tile in one DMA.
"""

import os
from contextlib import ExitStack

import numpy as np

P = 128
D = 1024
NC_I = 8           # contraction chunks of 128 over D_IN
NPAIR = 4          # chunk pairs for fp8 DoubleRow spline matmuls
NSCH = 6           # spline DR channels: (bnm, bnp, sg) x (w_hi, w_lo)
NO = 8             # output-feature chunks of 128
NTOK = 1024        # tokens per core
N_CORES = 8
TILES = [(0, 512), (512, 384), (896, 128)]   # (tok0, T) per token tile
OGROUPS = [(0, 6), (6, 2)]                   # o-block groups (start, count)
KSLOPE = 4400.0    # step-channel sigmoid slope (matches the eps=1e-6
                   # rational transition width sqrt(eps)~1e-3 at |x|=2)
LN_EPS = 1e-5
RSQRT_MAGIC32 = 0x5F3759DF   # fp32 rsqrt bit-hack seed

_cache = {}


def _build_nc():
    import concourse.bass as bass
    import concourse.mybir as mybir
    import concourse.tile as tile
    from concourse import bacc

    f32 = mybir.dt.float32
    bf16 = mybir.dt.bfloat16
    f8e4 = mybir.dt.float8e4
    PM = mybir.MatmulPerfMode
    i32 = mybir.dt.int32
    AF = mybir.ActivationFunctionType
    OP = mybir.AluOpType
    ts = bass.ts

    nc = bacc.Bacc(
        "TRN2",
        target_bir_lowering=False,
        debug=False,
        enable_asserts=False,
        num_devices=N_CORES,
    )

    xt_d = nc.dram_tensor("xt", [NC_I, P, NTOK], bf16, kind="ExternalInput")
    xt32_d = nc.dram_tensor("xt32", [NC_I, P, NTOK], f32, kind="ExternalInput")
    c0_d = nc.dram_tensor("c0", [P, NO], f32, kind="ExternalInput")
    w_d = nc.dram_tensor("w", [NC_I, P, D], bf16, kind="ExternalInput")
    w8_d = nc.dram_tensor("w8", [NPAIR, P, NSCH * 2 * D], f8e4,
                          kind="ExternalInput")
    w1t_d = nc.dram_tensor("w1t", [NO, P, 32], bf16, kind="ExternalInput")
    w2t_d = nc.dram_tensor("w2t", [32, D], bf16, kind="ExternalInput")
    lnw_d = nc.dram_tensor("lnw", [P, NO], f32, kind="ExternalInput")
    lnb_d = nc.dram_tensor("lnb", [P, NO], f32, kind="ExternalInput")
    onesc_d = nc.dram_tensor("onesc", [P, 1], bf16, kind="ExternalInput")  # 1/D
    onesp_d = nc.dram_tensor("onesp", [1, P], bf16, kind="ExternalInput")  # 1.0
    b1_d = nc.dram_tensor("b1", [32, 1], f32, kind="ExternalInput")
    b2_d = nc.dram_tensor("b2", [P, NO], f32, kind="ExternalInput")
    out_d = nc.dram_tensor("outT", [NO, P, NTOK], bf16, kind="ExternalOutput")

    with tile.TileContext(nc) as tc, ExitStack() as ctx:
        cp = ctx.enter_context(tc.tile_pool(name="cp", bufs=1))
        wp = ctx.enter_context(tc.tile_pool(name="wp", bufs=1))
        xp = ctx.enter_context(tc.tile_pool(name="xp", bufs=2))
        bn8p = ctx.enter_context(tc.tile_pool(name="bn8p", bufs=2))
        bp = ctx.enter_context(tc.tile_pool(name="bp", bufs=2))
        op_pool = ctx.enter_context(tc.tile_pool(name="op", bufs=1))
        sqp = ctx.enter_context(tc.tile_pool(name="sqp", bufs=2))
        x32p = ctx.enter_context(tc.tile_pool(name="x32p", bufs=2))
        stp = ctx.enter_context(tc.tile_pool(name="stp", bufs=1))
        yp = ctx.enter_context(tc.tile_pool(name="yp", bufs=1))
        sep = ctx.enter_context(tc.tile_pool(name="sep", bufs=3))
        finp = ctx.enter_context(tc.tile_pool(name="finp", bufs=1))
        # PSUM: 6 (main) + 1 (stats/broadcast, shared tag) + 1 (SE) = 8 banks
        mps = ctx.enter_context(tc.tile_pool(name="mps", bufs=6, space="PSUM"))
        auxps = ctx.enter_context(tc.tile_pool(name="auxps", bufs=1, space="PSUM"))
        seps = ctx.enter_context(tc.tile_pool(name="seps", bufs=1, space="PSUM"))

        # warm the sigmoid_and_others ACT table at t=0 (overlaps initial DMA)
        warm_t = cp.tile([P, 1], f32, tag="warm")
        nc.scalar.activation(warm_t[:], nc.const_aps.tensor(1.0, (P, 1)), AF.Relu)

        bk_t = cp.tile([P, 1], f32, tag="bk")
        nc.gpsimd.memset(bk_t[:], -2.0 * KSLOPE)
        half_t = cp.tile([P, 512], bf16, tag="half")
        nc.gpsimd.memset(half_t[:], 0.5)

        C = {}  # constant tiles, DMA'd after the first chunk's x/w (startup)

        def emit_consts():
            C["w1t"] = cp.tile([P, NO, 32], bf16, tag="w1t", name="w1t_t")
            nc.gpsimd.dma_start(C["w1t"][:], w1t_d.ap().rearrange("c p j -> p c j"))
            C["w2t"] = cp.tile([32, D], bf16, tag="w2t", name="w2t_t")
            nc.gpsimd.dma_start(C["w2t"][:], w2t_d.ap())
            C["lnw"] = cp.tile([P, NO], f32, tag="lnw", name="lnw_t")
            nc.gpsimd.dma_start(C["lnw"][:], lnw_d.ap())
            C["lnb"] = cp.tile([P, NO], f32, tag="lnb", name="lnb_t")
            nc.gpsimd.dma_start(C["lnb"][:], lnb_d.ap())
            C["onesc"] = cp.tile([P, 1], bf16, tag="onesc", name="onesc_t")
            nc.gpsimd.dma_start(C["onesc"][:], onesc_d.ap())
            C["onesp"] = cp.tile([1, P], bf16, tag="onesp", name="onesp_t")
            nc.gpsimd.dma_start(C["onesp"][:], onesp_d.ap())
            C["b1"] = cp.tile([32, 1], f32, tag="b1", name="b1_t")
            nc.gpsimd.dma_start(C["b1"][:], b1_d.ap())
            C["b2"] = cp.tile([P, NO], f32, tag="b2", name="b2_t")
            nc.gpsimd.dma_start(C["b2"][:], b2_d.ap())
            C["c0"] = cp.tile([P, NO], f32, tag="c0", name="c0_t")
            nc.gpsimd.dma_start(C["c0"][:], c0_d.ap())

        w_tiles = [None] * NC_I
        w8_tiles = [None] * NPAIR

        def emit_wb_dma(c):
            w_t = wp.tile([P, D], bf16, tag=f"w{c}")
            nc.sync.dma_start(w_t[:], w_d.ap()[c])
            w_tiles[c] = w_t

        def emit_w8_dma(pair):
            w8_src = w8_d.ap()[pair].rearrange(
                "p (c8 j d) -> p c8 j d", c8=NSCH, j=2
            )
            tiles = []
            for c8 in range(NSCH):
                w8_t = wp.tile([P, 2, D], f8e4, tag=f"w8p{pair}c{c8}")
                nc.sync.dma_start(w8_t[:], w8_src[:, c8])
                tiles.append(w8_t)
            w8_tiles[pair] = tiles

        def emit_basis(ti, c, T, tok0, bn8_by_pair):
            """Basis channels for chunk c via the h/step reformulation.

            bn_p = h(clamp(x,0,1)) - step(x>2), bn_m = h(clamp(-x,0,1)) -
            step(x<-2), sigma = 1 - step+ - step-, with h(c) = c^2/(c^2 +
            (1-c)^2) = 0.5 + u/(2u^2+0.5), u = c-0.5. The 0.5 offsets and the
            W0 channel fold into a per-feature constant (c0) applied at the
            PSUM descale. v = u/(2u^2+0.5) and the steps are written as fp8
            into slot c%2 of the chunk-pair tiles for DoubleRow matmuls. The
            steps use the fp32 copy of x (the bf16 ulp at |x|=2 is 16x wider
            than the eps-rational transition band being approximated).
            """
            pair, j = c // 2, c % 2
            x_t = xp.tile([P, T], bf16, tag=f"x{c}")
            nc.sync.dma_start(x_t[:], xt_d.ap()[c, :, tok0:tok0 + T])
            x32_t = x32p.tile([P, T], f32, tag="x32")
            nc.sync.dma_start(x32_t[:], xt32_d.ap()[c, :, tok0:tok0 + T])
            if j == 0:
                bn8_by_pair[pair] = [
                    bn8p.tile([P, 2, T], f8e4, tag=f"bn8{k}{pair}",
                              name=f"bn8{k}{pair}_{ti}")
                    for k in ("vd", "vc", "km", "kp")
                ]
            vd8, vc8, km8, kp8 = bn8_by_pair[pair]
            uc_t = bp.tile([P, T], bf16, tag="uc")
            nc.vector.tensor_scalar(uc_t[:], x_t[:], 1.0, 0.0, OP.min, OP.max)
            nc.vector.tensor_scalar(uc_t[:], uc_t[:], -0.5, None, OP.add)
            ud_t = bp.tile([P, T], bf16, tag="ud")
            nc.vector.tensor_scalar(ud_t[:], x_t[:], 0.0, -1.0, OP.min, OP.mult)
            nc.vector.tensor_scalar(ud_t[:], ud_t[:], 1.0, -0.5, OP.min, OP.add)
            u2c = bp.tile([P, T], bf16, tag="u2c")
            nc.scalar.activation(u2c[:], uc_t[:], AF.Square, scale=2.0 ** 0.5)
            u2d = bp.tile([P, T], bf16, tag="u2d")
            nc.scalar.activation(u2d[:], ud_t[:], AF.Square, scale=2.0 ** 0.5)
            shc = bp.tile([P, T], f32, tag="shc")
            nc.scalar.activation(shc[:], u2c[:], AF.Copy, bias=0.5, scale=1.0)
            shd = bp.tile([P, T], f32, tag="shd")
            nc.scalar.activation(shd[:], u2d[:], AF.Copy, bias=0.5, scale=1.0)
            invc = bp.tile([P, T], f32, tag="invc")
            nc.vector.reciprocal_approx_fast(out=invc[:], in_=shc[:])
            invd = bp.tile([P, T], f32, tag="invd")
            nc.vector.reciprocal_approx_fast(out=invd[:], in_=shd[:])
            nc.gpsimd.tensor_tensor(vc8[:, j], uc_t[:], invc[:], OP.mult)
            nc.gpsimd.tensor_tensor(vd8[:, j], ud_t[:], invd[:], OP.mult)
            nc.scalar.activation(kp8[:, j], x32_t[:], AF.Sigmoid, bias=bk_t[:],
                                 scale=KSLOPE)
            nc.scalar.activation(km8[:, j], x32_t[:], AF.Sigmoid, bias=bk_t[:],
                                 scale=-KSLOPE)
            return x_t

        def emit_base_mains(ps, x_t, c, o0, no):
            """bf16 base-channel matmuls (x * 512Wb) for one chunk."""
            for oi in range(no):
                o = o0 + oi
                nc.tensor.matmul(
                    ps[oi][:],
                    lhsT=w_tiles[c][:, ts(o, P)],
                    rhs=x_t[:],
                    start=(c == 0),
                    stop=False,
                )

        def emit_spline_mains(ps, bn8, pair, o0, no, last, block_post, psl=None):
            """fp8 DoubleRow spline matmuls for one chunk pair (6 channels:
            3 bn tensors x hi/lo weights). On the final pair the emission is
            o-outer with per-block copies/stats chasing the stop."""
            w8 = w8_tiles[pair]
            # channels: vd*(Wm'-hi/lo), vc*(Wp'-hi/lo), km*(-Wm), kp*(-Wp)
            rhs = [bn8[0], bn8[0], bn8[1], bn8[1], bn8[2], bn8[3]]
            if not last:
                for c8 in range(NSCH):
                    for oi in range(no):
                        o = o0 + oi
                        nc.tensor.matmul(
                            ps[oi][:],
                            lhsT=w8[c8][:, :, ts(o, P)],
                            rhs=rhs[c8][:],
                            start=False,
                            stop=False,
                            perf_mode=PM.DoubleRow,
                        )
            else:
                for oi in range(no):
                    o = o0 + oi
                    for c8 in range(NSCH):
                        nc.tensor.matmul(
                            ps[oi][:],
                            lhsT=w8[c8][:, :, ts(o, P)],
                            rhs=rhs[c8][:],
                            start=False,
                            stop=(c8 == NSCH - 1),
                            perf_mode=PM.DoubleRow,
                        )
                    block_post(ps[oi], o)

        def emit_tile(ti, tok0, T, pending_aux, rhs_by_chunk, bn8_by_pair,
                      basis_next):
            """Emit one token tile's mains+stats; interleave prev tile's aux
            into group 0 and the NEXT tile's basis into group 1."""
            outs = [None] * NO
            sq = [None] * NO
            st = {}

            def get_psAB():
                # allocated lazily so the "aux" tag rotation matches runtime
                # order (after the previous tile's zb/mrb/psS allocations)
                if "psAB" not in st:
                    st["psAB"] = auxps.tile([33, T], f32, tag="aux",
                                            name=f"psAB_{ti}")
                return st["psAB"]

            def block_post(ps_ap, o):
                psAB = get_psAB()
                psA = psAB[0:1, :]
                psB = psAB[32:33, :]
                o_t = op_pool.tile([P, T], bf16, tag=f"out{o}", name=f"o_{ti}_{o}")
                nc.vector.tensor_scalar(o_t[:], ps_ap[:], 1.0 / 512.0,
                                        C["c0"][:, o:o + 1], OP.mult, OP.add)
                outs[o] = o_t
                sq_t = sqp.tile([P, T], bf16, tag="sq")
                nc.gpsimd.tensor_tensor(sq_t[:], o_t[:], o_t[:], OP.mult)
                sq[o] = sq_t
                nc.tensor.matmul(
                    psA, lhsT=C["onesc"][:], rhs=o_t[:],
                    start=(o == 0), stop=(o == NO - 1),
                )
                nc.tensor.matmul(
                    psB, lhsT=C["onesc"][:], rhs=sq_t[:],
                    start=(o == 0), stop=(o == NO - 1),
                )

            next_rhs = [None] * NC_I
            next_bn8 = [None] * NPAIR
            for gi, (o0, no) in enumerate(OGROUPS):
                ps = [
                    mps.tile([P, T], f32, tag="mps", name=f"ps_{ti}_{o0 + i}")
                    for i in range(no)
                ]
                if ti == 0 and gi == 0:
                    # startup: the base channel of every chunk only needs x +
                    # its weights, so run all 8 of those first while the basis
                    # pipeline fills; the spline pairs follow in a second pass
                    for c in range(NC_I):
                        rhs_by_chunk[c] = emit_basis(ti, c, T, tok0,
                                                     bn8_by_pair)
                        emit_wb_dma(c)
                        emit_base_mains(ps, rhs_by_chunk[c], c, o0, no)
                        if c == 0:
                            emit_consts()
                    for pair in range(NPAIR):
                        emit_w8_dma(pair)
                        emit_spline_mains(ps, bn8_by_pair[pair], pair, o0, no,
                                          last=(pair == NPAIR - 1),
                                          block_post=block_post)
                    continue
                for c in range(NC_I):
                    emit_base_mains(ps, rhs_by_chunk[c], c, o0, no)
                    if c % 2 == 1:
                        emit_spline_mains(ps, bn8_by_pair[c // 2], c // 2,
                                          o0, no, last=(c == NC_I - 1),
                                          block_post=block_post)
                    # interleave the previous tile's aux across both groups
                    if pending_aux is not None:
                        stage = pending_aux.get((gi, c))
                        if stage is not None:
                            stage()
                    # compute the NEXT tile's basis while this tile's mains
                    # run (this tile's basis was precomputed, so the basis
                    # engines are otherwise idle): chunks 0-3 late in group 0,
                    # chunks 4-7 early in group 1
                    if basis_next is not None and ti > 0:
                        ntok0, nT = basis_next
                        nc_ = None
                        if gi == 0 and c >= 3 and c < 7:
                            nc_ = c - 3
                        elif gi == 1 and c < 4:
                            nc_ = c + 4
                        elif gi == 0 and c == 7:
                            nc_ = None
                        if nc_ is not None:
                            next_rhs[nc_] = emit_basis(ti + 1, nc_, nT, ntok0,
                                                       next_bn8)
                    elif gi == 1 and basis_next is not None:
                        ntok0, nT = basis_next
                        next_rhs[c] = emit_basis(ti + 1, c, nT, ntok0,
                                                 next_bn8)

            return {"T": T, "tok0": tok0, "ti": ti, "psAB": st["psAB"],
                    "outs": outs, "sq": sq, "next_rhs": next_rhs,
                    "next_bn8": next_bn8}

        def make_aux(tile_st):
            """Aux stage emitters for a completed tile: stats chain -> LN -> SE."""
            T = tile_st["T"]
            ti = tile_st["ti"]
            tok0 = tile_st["tok0"]
            outs = tile_st["outs"]
            psAB = tile_st["psAB"]
            ctx_st = {}

            def chain():
                # negmu/e2 rows from psum, fp32 bit-hack rsqrt + 1 Newton step
                negmu = stp.tile([1, T], f32, tag="negmu")
                nc.vector.tensor_scalar(
                    negmu[:], psAB[0:1, :], -1.0, 0.0, OP.mult, OP.add
                )
                e2 = stp.tile([1, T], f32, tag="e2")
                nc.vector.tensor_scalar(
                    e2[:], psAB[32:33, :], 1.0, LN_EPS, OP.mult, OP.add
                )
                mu2 = stp.tile([1, T], f32, tag="mu2")
                nc.vector.tensor_tensor(mu2[:], negmu[:], negmu[:], OP.mult)
                var = stp.tile([1, T], f32, tag="var")
                nc.vector.tensor_tensor(var[:], e2[:], mu2[:], OP.subtract)
                zw = stp.tile([1, T], f32, tag="zw")
                nc.vector.tensor_scalar(
                    zw[:].bitcast(i32), var[:].bitcast(i32), 1, 0,
                    OP.arith_shift_right,
                )
                nc.vector.tensor_scalar(
                    zw[:].bitcast(i32), zw[:].bitcast(i32), -1, RSQRT_MAGIC32,
                    OP.mult, OP.add,
                )
                t1 = stp.tile([1, T], f32, tag="t1")
                nc.vector.tensor_tensor(t1[:], zw[:], zw[:], OP.mult)
                nc.vector.tensor_tensor(t1[:], t1[:], var[:], OP.mult)
                nc.vector.tensor_scalar(t1[:], t1[:], -0.5, 1.5, OP.mult, OP.add)
                z16 = stp.tile([1, T], bf16, tag="z16")
                nc.vector.tensor_tensor(z16[:], zw[:], t1[:], OP.mult)
                mr16 = stp.tile([1, T], bf16, tag="mr16")
                nc.vector.tensor_tensor(mr16[:], negmu[:], z16[:], OP.mult)
                # broadcast rows across partitions via K=1 outer products
                zbp = auxps.tile([P, T], f32, tag="aux", name=f"zb_{ti}")
                nc.tensor.matmul(zbp[:], lhsT=C["onesp"][:], rhs=z16[:],
                                 start=True, stop=True)
                zb16 = stp.tile([P, T], bf16, tag="zb16")
                nc.vector.tensor_copy(out=zb16[:], in_=zbp[:])
                mrp = auxps.tile([P, T], f32, tag="aux", name=f"mrb_{ti}")
                nc.tensor.matmul(mrp[:], lhsT=C["onesp"][:], rhs=mr16[:],
                                 start=True, stop=True)
                mrb16 = stp.tile([P, T], bf16, tag="mrb16")
                nc.scalar.activation(mrb16[:], mrp[:], AF.Copy)
                ctx_st["zb16"] = zb16
                ctx_st["mrb16"] = mrb16

            def ln():
                zb16, mrb16 = ctx_st["zb16"], ctx_st["mrb16"]
                psH = seps.tile([32, T], f32, tag="sps", name=f"psH_{ti}")
                yhat = []
                for o in range(NO):
                    q_t = sqp.tile([P, T], bf16, tag="q")
                    nc.vector.tensor_tensor(q_t[:], outs[o][:], zb16[:], OP.mult)
                    yh_t = yp.tile([P, T], bf16, tag=f"yh{o}")
                    nc.vector.tensor_tensor(yh_t[:], q_t[:], mrb16[:], OP.add)
                    yhat.append(yh_t)
                    nc.tensor.matmul(
                        psH[:], lhsT=C["w1t"][:, o, :], rhs=yh_t[:],
                        start=(o == 0), stop=(o == NO - 1),
                    )
                hr = sep.tile([32, T], bf16, tag="hr")
                nc.scalar.activation(hr[:], psH[:], AF.Relu, bias=C["b1"][:], scale=1.0)
                ctx_st["yhat"] = yhat
                ctx_st["hr"] = hr

            def se_blocks(olist):
                yhat, hr = ctx_st["yhat"], ctx_st["hr"]
                if "fin" not in ctx_st:
                    # one wide tile so all 8 out blocks leave in a single DMA
                    # (the descriptor engine processes DMAs serially)
                    ctx_st["fin"] = finp.tile([P, NO, T], bf16, tag="fin",
                                             name=f"fin_{ti}")
                fin_t = ctx_st["fin"]
                for o in olist:
                    # alternate psS between the two non-main PSUM banks so the
                    # W2 matmul of block o+1 overlaps the sigmoid of block o
                    pool, tg = (seps, "sps") if o % 2 else (auxps, "aux")
                    psS = pool.tile([P, T], f32, tag=tg, name=f"psS_{ti}_{o}")
                    nc.tensor.matmul(
                        psS[:], lhsT=C["w2t"][:, ts(o, P)], rhs=hr[:],
                        start=True, stop=True,
                    )
                    yf_t = sep.tile([P, T], bf16, tag="yf")
                    nc.vector.tensor_scalar(
                        yf_t[:], yhat[o][:], C["lnw"][:, o:o + 1], C["lnb"][:, o:o + 1],
                        OP.mult, OP.add,
                    )
                    se_t = sep.tile([P, T], bf16, tag="se")
                    nc.scalar.activation(
                        se_t[:], psS[:], AF.Sigmoid, bias=C["b2"][:, o:o + 1], scale=1.0
                    )
                    nc.vector.tensor_tensor(fin_t[:, o], yf_t[:], se_t[:], OP.mult)
                h0, h1 = olist[0], olist[-1] + 1
                nc.sync.dma_start(
                    out_d.ap()[h0:h1, :, tok0:tok0 + T].rearrange("o p t -> p o t"),
                    fin_t[:, h0:h1],
                )

            return {"chain": chain, "ln": ln,
                    "se_a": lambda: se_blocks(range(0, 4)),
                    "se_b": lambda: se_blocks(range(4, NO))}

        AUX_SLOTS = {"chain": (0, 0), "ln": (0, 2), "se_a": (0, 5), "se_b": (1, 2)}

        pending = None
        rhs_cur = [None] * NC_I
        bn8_cur = [None] * NPAIR
        for ti, (tok0, T) in enumerate(TILES):
            basis_next = TILES[ti + 1] if ti + 1 < len(TILES) else None
            aux_by_slot = (
                {slot: pending[name] for name, slot in AUX_SLOTS.items()}
                if pending else None
            )
            tile_st = emit_tile(ti, tok0, T, aux_by_slot, rhs_cur, bn8_cur,
                                basis_next)
            rhs_cur = tile_st["next_rhs"]
            bn8_cur = tile_st["next_bn8"]
            pending = make_aux(tile_st)
        # final tile's aux runs at the end (smallest tile -> short tail)
        pending["chain"]()
        pending["ln"]()
        pending["se_a"]()
        pending["se_b"]()

    nc.compile()
    return nc


def _get_nc():
    if "nc" not in _cache:
        _cache["nc"] = _build_nc()
    return _cache["nc"]


def _prep_host(inputs):
    import ml_dtypes

    f = np.float32
    bf = ml_dtypes.bfloat16
    x = np.asarray(inputs["x"], f)
    base_weight = np.asarray(inputs["base_weight"], f)
    spline_weight = np.asarray(inputs["spline_weight"], f)
    ln_w = np.asarray(inputs["ln_w"], f)
    ln_b = np.asarray(inputs["ln_b"], f)
    se_w1 = np.asarray(inputs["se_w1"], f)
    se_b1 = np.asarray(inputs["se_b1"], f)
    se_w2 = np.asarray(inputs["se_w2"], f)
    se_b2 = np.asarray(inputs["se_b2"], f)

    f8 = ml_dtypes.float8_e4m3
    xt_all = x.reshape(N_CORES, NTOK, D).transpose(0, 2, 1)  # [core, D, ntok]

    # base channel: 512*Wb in bf16 (2^9 scale is exact); the 512 factor
    # matches the fp8 spline product scale so both share one PSUM bank
    w_base = (512.0 * base_weight.T).reshape(NC_I, P, D).astype(bf)

    wsT = spline_weight.transpose(1, 2, 0)  # [i, g, o]
    wm = wsT[:, 0, :].astype(f)
    w0 = wsT[:, 1, :].astype(f)
    wp = wsT[:, 2, :].astype(f)
    # h/step channels (x512): vd*(Wm-W0) [hi+lo], vc*(Wp-W0) [hi+lo],
    # km*(-Wm), kp*(-Wp); the 0.5 offsets of h and the W0 channel fold into
    # the per-feature constant c0 applied at the PSUM descale
    wmp = (wm - w0) * 512.0
    wpp = (wp - w0) * 512.0
    wmp_hi = wmp.astype(f8)
    wpp_hi = wpp.astype(f8)
    chans = [
        wmp_hi,
        (wmp - wmp_hi.astype(f)).astype(f8),
        wpp_hi,
        (wpp - wpp_hi.astype(f)).astype(f8),
        (-512.0 * wm).astype(f8),
        (-512.0 * wp).astype(f8),
    ]
    w8_all = np.empty((NPAIR, P, NSCH, 2, D), f8)
    for c8, wsrc in enumerate(chans):
        w8_all[:, :, c8, :, :] = wsrc.reshape(NPAIR, 2, P, D).transpose(0, 2, 1, 3)
    w8_all = np.ascontiguousarray(w8_all.reshape(NPAIR, P, NSCH * 2 * D))
    c0 = (w0.sum(axis=0) + 0.5 * (wm - w0).sum(axis=0)
          + 0.5 * (wp - w0).sum(axis=0)).astype(f)              # [o]

    w1p = (se_w1 * ln_w[None, :]).astype(f)          # ln_w folded into W1
    b1p = (se_b1 + se_w1 @ ln_b).astype(f)           # ln_b folded into b1

    shared = {
        "w": w_base,
        "w8": w8_all,
        "c0": np.ascontiguousarray(c0.reshape(NO, P).T),
        "w1t": np.ascontiguousarray(w1p.T.reshape(NO, P, 32)).astype(bf),
        "w2t": np.ascontiguousarray(se_w2.T).astype(bf),
        "lnw": np.ascontiguousarray(ln_w.reshape(NO, P).T),
        "lnb": np.ascontiguousarray(ln_b.reshape(NO, P).T),
        "onesc": np.full((P, 1), 1.0 / D, bf),
        "onesp": np.ones((1, P), bf),
        "b1": np.ascontiguousarray(b1p.reshape(32, 1)),
        "b2": np.ascontiguousarray(se_b2.reshape(NO, P).T),
    }
    in_maps = []
    for k in range(N_CORES):
        m = dict(shared)
        xk = np.ascontiguousarray(xt_all[k].reshape(NC_I, P, NTOK))
        m["xt"] = xk.astype(bf)
        m["xt32"] = xk
        in_maps.append(m)
    return in_maps


def kernel(**inputs) -> np.ndarray:
    from concourse.bass_utils import run_bass_kernel_spmd

    nc = _get_nc()
    in_maps = _prep_host(inputs)
    trace = bool(int(os.environ.get("KERNEL_TRACE", "0")))
    res = run_bass_kernel_spmd(
        nc, in_maps, core_ids=list(range(N_CORES)), trace=trace
    )
    _cache["last_result"] = res
    outs = []
    for k in range(N_CORES):
        outT = np.asarray(res.results[k]["outT"], dtype=np.float32)  # [NO, P, NTOK]
        outs.append(outT.reshape(D, NTOK).T)                          # [ntok, o]
    out = np.concatenate(outs, axis=0).reshape(8, 1024, 1024)
    return np.ascontiguousarray(out.astype(np.float32))


# revision 108
# speedup vs baseline: 1.4831x; 1.0141x over previous
"""Trainium2 Bass kernel for nn_EnhancedDRKANTreeNet (KAN layer + LayerNorm + SE gate).

Data-parallel over 8192 tokens across 8 NeuronCores (1024 tokens/core), all
compute feature-major: tiles are [feature_partition, token]. Token tiles are
[512, 384, 128] (the small final tile shrinks the serial LN/SE tail).

Main contraction, per output block of 128 features:
 - base channel x*(512*Wb) in bf16 (1 cycle/row on the PE);
 - spline channels in fp8e4 with DoubleRow perf mode (2 contraction rows per
   partition, 0.5 cycles/row). The grid-3 quadratic-spline basis is
   reformulated as bn_p = h(clamp(x,0,1)) - step(x>2), bn_m = h(clamp(-x,0,1))
   - step(x<-2), sigma = 1 - step+ - step-, with h(c) = 0.5 + u/(2u^2+0.5),
   u = c - 0.5 (exactly the eps-regularized normalized basis away from a
   ~1e-3-wide band at |x|=2). Channels: v_d*(Wm-W0)[fp8 hi+lo], v_c*(Wp-W0)
   [fp8 hi+lo], step-*(-Wm), step+*(-Wp); the 0.5 offsets of h and the W0
   channel fold into a per-feature constant c0 added at the PSUM descale.
   Steps are ACT sigmoids with slope 4400 evaluated on an fp32 copy of x
   (the bf16 ulp at |x|=2 is wider than the band being approximated).
   Weights are loaded into SBUF once; PSUM accumulates everything at 512x.

Elementwise engine placement (balanced ~80us each): ACT does the two
h-squares, the 2u^2+0.5 Copy-biases and the step sigmoids; DVE does clamps,
reciprocals (18-bit approx), PSUM descales, LN apply and stats chain; the
gpsimd/Pool engine does the v=u*inv fp8 writes and out^2 squares.

PSUM (8 banks): main accumulation is split into o-groups of 6+2 banks plus
one stats/broadcast bank (shared-tag rotation: psAB -> zb -> mrb -> psS-even)
and one SE bank, so tile t's LayerNorm/SE aux runs concurrently with tile
t+1's accumulation; aux stages are emitted interleaved into tile t+1's chunk
loop, and tile t+1's basis is produced during tile t's mains.

LayerNorm: per-token stats via ones-matmuls (mean folded into the ones
scale), fp32 int32-bit-hack rsqrt seed + one Newton step, then
y = out*(ones (x) z) + (ones (x) -mu*z); ln_w/ln_b are applied per-feature by
a 4x-rate tensor_scalar op (ln_w is also folded into the SE W1 on the host,
ln_b into the SE b1). SE: h = relu(W1'.y + b1'), se = sigmoid(W2.h + b2),
final = (ln_w*y + ln_b)*se, stored bf16 and shipped per-tile in one DMA.
"""

import os
from contextlib import ExitStack

import numpy as np

P = 128
D = 1024
NC_I = 8           # contraction chunks of 128 over D_IN
NPAIR = 4          # chunk pairs for fp8 DoubleRow spline matmuls
NSCH = 6           # spline DR channels: (bnm, bnp, sg) x (w_hi, w_lo)
NO = 8             # output-feature chunks of 128
NTOK = 1024        # tokens per core
N_CORES = 8
TILES = [(0, 512), (512, 384), (896, 128)]   # (tok0, T) per token tile
OGROUPS = [(0, 6), (6, 2)]                   # o-block groups (start, count)
KSLOPE = 4400.0    # step-channel sigmoid slope (matches the eps=1e-6
                   # rational transition width sqrt(eps)~1e-3 at |x|=2)
LN_EPS = 1e-5
RSQRT_MAGIC32 = 0x5F3759DF   # fp32 rsqrt bit-hack seed

_cache = {}


def _build_nc():
    import concourse.bass as bass
    import concourse.mybir as mybir
    import concourse.tile as tile
    from concourse import bacc

    f32 = mybir.dt.float32
    bf16 = mybir.dt.bfloat16
    f8e4 = mybir.dt.float8e4
    PM = mybir.MatmulPerfMode
    i32 = mybir.dt.int32
    AF = mybir.ActivationFunctionType
    OP = mybir.AluOpType
    ts = bass.ts

    nc = bacc.Bacc(
        "TRN2",
        target_bir_lowering=False,
        debug=False,
        enable_asserts=False,
        num_devices=N_CORES,
    )

    xt_d = nc.dram_tensor("xt", [NC_I, P, NTOK], bf16, kind="ExternalInput")
    xt32_d = nc.dram_tensor("xt32", [NC_I, P, NTOK], f32, kind="ExternalInput")
    c0_d = nc.dram_tensor("c0", [P, NO], f32, kind="ExternalInput")
    w_d = nc.dram_tensor("w", [NC_I, P, D], bf16, kind="ExternalInput")
    w8_d = nc.dram_tensor("w8", [NPAIR, P, NSCH * 2 * D], f8e4,
                          kind="ExternalInput")
    w1t_d = nc.dram_tensor("w1t", [NO, P, 32], bf16, kind="ExternalInput")
    w2t_d = nc.dram_tensor("w2t", [32, D], bf16, kind="ExternalInput")
    lnw_d = nc.dram_tensor("lnw", [P, NO], f32, kind="ExternalInput")
    lnb_d = nc.dram_tensor("lnb", [P, NO], f32, kind="ExternalInput")
    onesc_d = nc.dram_tensor("onesc", [P, 1], bf16, kind="ExternalInput")  # 1/D
    onesp_d = nc.dram_tensor("onesp", [1, P], bf16, kind="ExternalInput")  # 1.0
    b1_d = nc.dram_tensor("b1", [32, 1], f32, kind="ExternalInput")
    b2_d = nc.dram_tensor("b2", [P, NO], f32, kind="ExternalInput")
    out_d = nc.dram_tensor("outT", [NO, P, NTOK], bf16, kind="ExternalOutput")

    with tile.TileContext(nc) as tc, ExitStack() as ctx:
        cp = ctx.enter_context(tc.tile_pool(name="cp", bufs=1))
        wp = ctx.enter_context(tc.tile_pool(name="wp", bufs=1))
        xp = ctx.enter_context(tc.tile_pool(name="xp", bufs=2))
        bn8p = ctx.enter_context(tc.tile_pool(name="bn8p", bufs=2))
        bp = ctx.enter_context(tc.tile_pool(name="bp", bufs=2))
        op_pool = ctx.enter_context(tc.tile_pool(name="op", bufs=1))
        sqp = ctx.enter_context(tc.tile_pool(name="sqp", bufs=3))
        x32p = ctx.enter_context(tc.tile_pool(name="x32p", bufs=2))
        stp = ctx.enter_context(tc.tile_pool(name="stp", bufs=1))
        yp = ctx.enter_context(tc.tile_pool(name="yp", bufs=1))
        sep = ctx.enter_context(tc.tile_pool(name="sep", bufs=3))
        finp = ctx.enter_context(tc.tile_pool(name="finp", bufs=1))
        # PSUM: 6 (main) + 1 (stats/broadcast, shared tag) + 1 (SE) = 8 banks
        mps = ctx.enter_context(tc.tile_pool(name="mps", bufs=6, space="PSUM"))
        auxps = ctx.enter_context(tc.tile_pool(name="auxps", bufs=1, space="PSUM"))
        seps = ctx.enter_context(tc.tile_pool(name="seps", bufs=1, space="PSUM"))

        # warm the sigmoid_and_others ACT table at t=0 (overlaps initial DMA)
        warm_t = cp.tile([P, 1], f32, tag="warm")
        nc.scalar.activation(warm_t[:], nc.const_aps.tensor(1.0, (P, 1)), AF.Relu)

        bk_t = cp.tile([P, 1], f32, tag="bk")
        nc.gpsimd.memset(bk_t[:], -2.0 * KSLOPE)
        half_t = cp.tile([P, 512], bf16, tag="half")
        nc.gpsimd.memset(half_t[:], 0.5)

        C = {}  # constant tiles, DMA'd after the first chunk's x/w (startup)

        def emit_consts():
            C["w1t"] = cp.tile([P, NO, 32], bf16, tag="w1t", name="w1t_t")
            nc.gpsimd.dma_start(C["w1t"][:], w1t_d.ap().rearrange("c p j -> p c j"))
            C["w2t"] = cp.tile([32, D], bf16, tag="w2t", name="w2t_t")
            nc.gpsimd.dma_start(C["w2t"][:], w2t_d.ap())
            C["lnw"] = cp.tile([P, NO], f32, tag="lnw", name="lnw_t")
            nc.gpsimd.dma_start(C["lnw"][:], lnw_d.ap())
            C["lnb"] = cp.tile([P, NO], f32, tag="lnb", name="lnb_t")
            nc.gpsimd.dma_start(C["lnb"][:], lnb_d.ap())
            C["onesc"] = cp.tile([P, 1], bf16, tag="onesc", name="onesc_t")
            nc.gpsimd.dma_start(C["onesc"][:], onesc_d.ap())
            C["onesp"] = cp.tile([1, P], bf16, tag="onesp", name="onesp_t")
            nc.gpsimd.dma_start(C["onesp"][:], onesp_d.ap())
            C["b1"] = cp.tile([32, 1], f32, tag="b1", name="b1_t")
            nc.gpsimd.dma_start(C["b1"][:], b1_d.ap())
            C["b2"] = cp.tile([P, NO], f32, tag="b2", name="b2_t")
            nc.gpsimd.dma_start(C["b2"][:], b2_d.ap())
            C["c0"] = cp.tile([P, NO], f32, tag="c0", name="c0_t")
            nc.gpsimd.dma_start(C["c0"][:], c0_d.ap())

        w_tiles = [None] * NC_I
        w8_tiles = [None] * NPAIR

        def emit_wb_dma(c):
            w_t = wp.tile([P, D], bf16, tag=f"w{c}")
            nc.sync.dma_start(w_t[:], w_d.ap()[c])
            w_tiles[c] = w_t

        def emit_w8_dma(pair):
            w8_src = w8_d.ap()[pair].rearrange(
                "p (c8 j d) -> p c8 j d", c8=NSCH, j=2
            )
            tiles = []
            for c8 in range(NSCH):
                w8_t = wp.tile([P, 2, D], f8e4, tag=f"w8p{pair}c{c8}")
                nc.sync.dma_start(w8_t[:], w8_src[:, c8])
                tiles.append(w8_t)
            w8_tiles[pair] = tiles

        def emit_basis(ti, c, T, tok0, bn8_by_pair):
            """Basis channels for chunk c via the h/step reformulation.

            bn_p = h(clamp(x,0,1)) - step(x>2), bn_m = h(clamp(-x,0,1)) -
            step(x<-2), sigma = 1 - step+ - step-, with h(c) = c^2/(c^2 +
            (1-c)^2) = 0.5 + u/(2u^2+0.5), u = c-0.5. The 0.5 offsets and the
            W0 channel fold into a per-feature constant (c0) applied at the
            PSUM descale. v = u/(2u^2+0.5) and the steps are written as fp8
            into slot c%2 of the chunk-pair tiles for DoubleRow matmuls. The
            steps use the fp32 copy of x (the bf16 ulp at |x|=2 is 16x wider
            than the eps-rational transition band being approximated).
            """
            pair, j = c // 2, c % 2
            x_t = xp.tile([P, T], bf16, tag=f"x{c}")
            nc.sync.dma_start(x_t[:], xt_d.ap()[c, :, tok0:tok0 + T])
            x32_t = x32p.tile([P, T], f32, tag="x32")
            nc.sync.dma_start(x32_t[:], xt32_d.ap()[c, :, tok0:tok0 + T])
            if j == 0:
                bn8_by_pair[pair] = [
                    bn8p.tile([P, 2, T], f8e4, tag=f"bn8{k}{pair}",
                              name=f"bn8{k}{pair}_{ti}")
                    for k in ("vd", "vc", "km", "kp")
                ]
            vd8, vc8, km8, kp8 = bn8_by_pair[pair]
            uc_t = bp.tile([P, T], bf16, tag="uc")
            nc.vector.tensor_scalar(uc_t[:], x_t[:], 1.0, 0.0, OP.min, OP.max)
            nc.vector.tensor_scalar(uc_t[:], uc_t[:], -0.5, None, OP.add)
            ud_t = bp.tile([P, T], bf16, tag="ud")
            nc.vector.tensor_scalar(ud_t[:], x_t[:], 0.0, -1.0, OP.min, OP.mult)
            nc.vector.tensor_scalar(ud_t[:], ud_t[:], 1.0, -0.5, OP.min, OP.add)
            u2c = bp.tile([P, T], bf16, tag="u2c")
            nc.scalar.activation(u2c[:], uc_t[:], AF.Square, scale=2.0 ** 0.5)
            u2d = bp.tile([P, T], bf16, tag="u2d")
            nc.scalar.activation(u2d[:], ud_t[:], AF.Square, scale=2.0 ** 0.5)
            shc = bp.tile([P, T], f32, tag="shc")
            nc.scalar.activation(shc[:], u2c[:], AF.Copy, bias=0.5, scale=1.0)
            shd = bp.tile([P, T], f32, tag="shd")
            nc.scalar.activation(shd[:], u2d[:], AF.Copy, bias=0.5, scale=1.0)
            invc = bp.tile([P, T], f32, tag="invc")
            nc.vector.reciprocal_approx_fast(out=invc[:], in_=shc[:])
            invd = bp.tile([P, T], f32, tag="invd")
            nc.vector.reciprocal_approx_fast(out=invd[:], in_=shd[:])
            nc.gpsimd.tensor_tensor(vc8[:, j], uc_t[:], invc[:], OP.mult)
            nc.gpsimd.tensor_tensor(vd8[:, j], ud_t[:], invd[:], OP.mult)
            nc.scalar.activation(kp8[:, j], x32_t[:], AF.Sigmoid, bias=bk_t[:],
                                 scale=KSLOPE)
            nc.scalar.activation(km8[:, j], x32_t[:], AF.Sigmoid, bias=bk_t[:],
                                 scale=-KSLOPE)
            return x_t

        def emit_base_mains(ps, x_t, c, o0, no):
            """bf16 base-channel matmuls (x * 512Wb) for one chunk."""
            for oi in range(no):
                o = o0 + oi
                nc.tensor.matmul(
                    ps[oi][:],
                    lhsT=w_tiles[c][:, ts(o, P)],
                    rhs=x_t[:],
                    start=(c == 0),
                    stop=False,
                )

        def emit_spline_mains(ps, bn8, pair, o0, no, last, block_post, psl=None):
            """fp8 DoubleRow spline matmuls for one chunk pair (6 channels:
            3 bn tensors x hi/lo weights). On the final pair the emission is
            o-outer with per-block copies/stats chasing the stop."""
            w8 = w8_tiles[pair]
            # channels: vd*(Wm'-hi/lo), vc*(Wp'-hi/lo), km*(-Wm), kp*(-Wp)
            rhs = [bn8[0], bn8[0], bn8[1], bn8[1], bn8[2], bn8[3]]
            if not last:
                for c8 in range(NSCH):
                    for oi in range(no):
                        o = o0 + oi
                        nc.tensor.matmul(
                            ps[oi][:],
                            lhsT=w8[c8][:, :, ts(o, P)],
                            rhs=rhs[c8][:],
                            start=False,
                            stop=False,
                            perf_mode=PM.DoubleRow,
                        )
            else:
                for oi in range(no):
                    o = o0 + oi
                    for c8 in range(NSCH):
                        nc.tensor.matmul(
                            ps[oi][:],
                            lhsT=w8[c8][:, :, ts(o, P)],
                            rhs=rhs[c8][:],
                            start=False,
                            stop=(c8 == NSCH - 1),
                            perf_mode=PM.DoubleRow,
                        )
                    block_post(ps[oi], o)

        def emit_tile(ti, tok0, T, pending_aux, rhs_by_chunk, bn8_by_pair,
                      basis_next):
            """Emit one token tile's mains+stats; interleave prev tile's aux
            into group 0 and the NEXT tile's basis into group 1."""
            outs = [None] * NO
            sq = [None] * NO
            st = {}

            def get_psAB():
                # allocated lazily so the "aux" tag rotation matches runtime
                # order (after the previous tile's zb/mrb/psS allocations)
                if "psAB" not in st:
                    st["psAB"] = auxps.tile([33, T], f32, tag="aux",
                                            name=f"psAB_{ti}")
                return st["psAB"]

            def block_post(ps_ap, o):
                psAB = get_psAB()
                psA = psAB[0:1, :]
                psB = psAB[32:33, :]
                o_t = op_pool.tile([P, T], bf16, tag=f"out{o}", name=f"o_{ti}_{o}")
                nc.vector.tensor_scalar(o_t[:], ps_ap[:], 1.0 / 512.0,
                                        C["c0"][:, o:o + 1], OP.mult, OP.add)
                outs[o] = o_t
                sq_t = sqp.tile([P, T], bf16, tag="sq")
                nc.gpsimd.tensor_tensor(sq_t[:], o_t[:], o_t[:], OP.mult)
                sq[o] = sq_t
                nc.tensor.matmul(
                    psA, lhsT=C["onesc"][:], rhs=o_t[:],
                    start=(o == 0), stop=(o == NO - 1),
                )
                nc.tensor.matmul(
                    psB, lhsT=C["onesc"][:], rhs=sq_t[:],
                    start=(o == 0), stop=(o == NO - 1),
                )

            next_rhs = [None] * NC_I
            next_bn8 = [None] * NPAIR
            for gi, (o0, no) in enumerate(OGROUPS):
                ps = [
                    mps.tile([P, T], f32, tag="mps", name=f"ps_{ti}_{o0 + i}")
                    for i in range(no)
                ]
                if ti == 0 and gi == 0:
                    # startup: the base channel of every chunk only needs x +
                    # its weights, so run all 8 of those first while the basis
                    # pipeline fills; the spline pairs follow in a second pass
                    for c in range(NC_I):
                        rhs_by_chunk[c] = emit_basis(ti, c, T, tok0,
                                                     bn8_by_pair)
                        emit_wb_dma(c)
                        emit_base_mains(ps, rhs_by_chunk[c], c, o0, no)
                        if c == 0:
                            emit_consts()
                    for pair in range(NPAIR):
                        emit_w8_dma(pair)
                        emit_spline_mains(ps, bn8_by_pair[pair], pair, o0, no,
                                          last=(pair == NPAIR - 1),
                                          block_post=block_post)
                    continue
                for c in range(NC_I):
                    emit_base_mains(ps, rhs_by_chunk[c], c, o0, no)
                    if c % 2 == 1:
                        emit_spline_mains(ps, bn8_by_pair[c // 2], c // 2,
                                          o0, no, last=(c == NC_I - 1),
                                          block_post=block_post)
                    # interleave the previous tile's aux across both groups
                    if pending_aux is not None:
                        stage = pending_aux.get((gi, c))
                        if stage is not None:
                            stage()
                    # compute the NEXT tile's basis while this tile's mains
                    # run (this tile's basis was precomputed, so the basis
                    # engines are otherwise idle): chunks 0-3 late in group 0,
                    # chunks 4-7 early in group 1
                    if basis_next is not None and ti > 0:
                        ntok0, nT = basis_next
                        nc_ = None
                        if gi == 0 and c >= 3 and c < 7:
                            nc_ = c - 3
                        elif gi == 1 and c < 4:
                            nc_ = c + 4
                        elif gi == 0 and c == 7:
                            nc_ = None
                        if nc_ is not None:
                            next_rhs[nc_] = emit_basis(ti + 1, nc_, nT, ntok0,
                                                       next_bn8)
                    elif gi == 1 and basis_next is not None:
                        ntok0, nT = basis_next
                        next_rhs[c] = emit_basis(ti + 1, c, nT, ntok0,
                                                 next_bn8)

            return {"T": T, "tok0": tok0, "ti": ti, "psAB": st["psAB"],
                    "outs": outs, "sq": sq, "next_rhs": next_rhs,
                    "next_bn8": next_bn8}

        def make_aux(tile_st):
            """Aux stage emitters for a completed tile: stats chain -> LN -> SE."""
            T = tile_st["T"]
            ti = tile_st["ti"]
            tok0 = tile_st["tok0"]
            outs = tile_st["outs"]
            psAB = tile_st["psAB"]
            ctx_st = {}

            def chain():
                # negmu/e2 rows from psum, fp32 bit-hack rsqrt + 1 Newton step
                negmu = stp.tile([1, T], f32, tag="negmu")
                nc.vector.tensor_scalar(
                    negmu[:], psAB[0:1, :], -1.0, 0.0, OP.mult, OP.add
                )
                e2 = stp.tile([1, T], f32, tag="e2")
                nc.vector.tensor_scalar(
                    e2[:], psAB[32:33, :], 1.0, LN_EPS, OP.mult, OP.add
                )
                mu2 = stp.tile([1, T], f32, tag="mu2")
                nc.vector.tensor_tensor(mu2[:], negmu[:], negmu[:], OP.mult)
                var = stp.tile([1, T], f32, tag="var")
                nc.vector.tensor_tensor(var[:], e2[:], mu2[:], OP.subtract)
                zw = stp.tile([1, T], f32, tag="zw")
                nc.vector.tensor_scalar(
                    zw[:].bitcast(i32), var[:].bitcast(i32), 1, 0,
                    OP.arith_shift_right,
                )
                nc.vector.tensor_scalar(
                    zw[:].bitcast(i32), zw[:].bitcast(i32), -1, RSQRT_MAGIC32,
                    OP.mult, OP.add,
                )
                t1 = stp.tile([1, T], f32, tag="t1")
                nc.vector.tensor_tensor(t1[:], zw[:], zw[:], OP.mult)
                nc.vector.tensor_tensor(t1[:], t1[:], var[:], OP.mult)
                nc.vector.tensor_scalar(t1[:], t1[:], -0.5, 1.5, OP.mult, OP.add)
                z16 = stp.tile([1, T], bf16, tag="z16")
                nc.vector.tensor_tensor(z16[:], zw[:], t1[:], OP.mult)
                mr16 = stp.tile([1, T], bf16, tag="mr16")
                nc.vector.tensor_tensor(mr16[:], negmu[:], z16[:], OP.mult)
                # broadcast rows across partitions via K=1 outer products
                zbp = auxps.tile([P, T], f32, tag="aux", name=f"zb_{ti}")
                nc.tensor.matmul(zbp[:], lhsT=C["onesp"][:], rhs=z16[:],
                                 start=True, stop=True)
                zb16 = stp.tile([P, T], bf16, tag="zb16")
                nc.vector.tensor_copy(out=zb16[:], in_=zbp[:])
                mrp = auxps.tile([P, T], f32, tag="aux", name=f"mrb_{ti}")
                nc.tensor.matmul(mrp[:], lhsT=C["onesp"][:], rhs=mr16[:],
                                 start=True, stop=True)
                mrb16 = stp.tile([P, T], bf16, tag="mrb16")
                nc.scalar.activation(mrb16[:], mrp[:], AF.Copy)
                ctx_st["zb16"] = zb16
                ctx_st["mrb16"] = mrb16

            def ln():
                zb16, mrb16 = ctx_st["zb16"], ctx_st["mrb16"]
                psH = seps.tile([32, T], f32, tag="sps", name=f"psH_{ti}")
                yhat = []
                for o in range(NO):
                    q_t = sqp.tile([P, T], bf16, tag="q")
                    nc.vector.tensor_tensor(q_t[:], outs[o][:], zb16[:], OP.mult)
                    yh_t = yp.tile([P, T], bf16, tag=f"yh{o}")
                    nc.vector.tensor_tensor(yh_t[:], q_t[:], mrb16[:], OP.add)
                    yhat.append(yh_t)
                    nc.tensor.matmul(
                        psH[:], lhsT=C["w1t"][:, o, :], rhs=yh_t[:],
                        start=(o == 0), stop=(o == NO - 1),
                    )
                hr = sep.tile([32, T], bf16, tag="hr")
                nc.scalar.activation(hr[:], psH[:], AF.Relu, bias=C["b1"][:], scale=1.0)
                ctx_st["yhat"] = yhat
                ctx_st["hr"] = hr

            def se_blocks(olist):
                yhat, hr = ctx_st["yhat"], ctx_st["hr"]
                if "fin" not in ctx_st:
                    # one wide tile so all 8 out blocks leave in a single DMA
                    # (the descriptor engine processes DMAs serially)
                    ctx_st["fin"] = finp.tile([P, NO, T], bf16, tag="fin",
                                             name=f"fin_{ti}")
                fin_t = ctx_st["fin"]
                for o in olist:
                    # alternate psS between the two non-main PSUM banks so the
                    # W2 matmul of block o+1 overlaps the sigmoid of block o
                    pool, tg = (seps, "sps") if o % 2 else (auxps, "aux")
                    psS = pool.tile([P, T], f32, tag=tg, name=f"psS_{ti}_{o}")
                    nc.tensor.matmul(
                        psS[:], lhsT=C["w2t"][:, ts(o, P)], rhs=hr[:],
                        start=True, stop=True,
                    )
                    yf_t = sep.tile([P, T], bf16, tag="yf")
                    nc.vector.tensor_scalar(
                        yf_t[:], yhat[o][:], C["lnw"][:, o:o + 1], C["lnb"][:, o:o + 1],
                        OP.mult, OP.add,
                    )
                    se_t = sep.tile([P, T], bf16, tag="se")
                    nc.scalar.activation(
                        se_t[:], psS[:], AF.Sigmoid, bias=C["b2"][:, o:o + 1], scale=1.0
                    )
                    nc.vector.tensor_tensor(fin_t[:, o], yf_t[:], se_t[:], OP.mult)
                h0, h1 = olist[0], olist[-1] + 1
                nc.sync.dma_start(
                    out_d.ap()[h0:h1, :, tok0:tok0 + T].rearrange("o p t -> p o t"),
                    fin_t[:, h0:h1],
                )

            return {"chain": chain, "ln": ln,
                    "se_a": lambda: se_blocks(range(0, 4)),
                    "se_b": lambda: se_blocks(range(4, NO))}

        AUX_SLOTS = {"chain": (0, 0), "ln": (0, 2), "se_a": (0, 5), "se_b": (1, 2)}

        pending = None
        rhs_cur = [None] * NC_I
        bn8_cur = [None] * NPAIR
        for ti, (tok0, T) in enumerate(TILES):
            basis_next = TILES[ti + 1] if ti + 1 < len(TILES) else None
            aux_by_slot = (
                {slot: pending[name] for name, slot in AUX_SLOTS.items()}
                if pending else None
            )
            tile_st = emit_tile(ti, tok0, T, aux_by_slot, rhs_cur, bn8_cur,
                                basis_next)
            rhs_cur = tile_st["next_rhs"]
            bn8_cur = tile_st["next_bn8"]
            pending = make_aux(tile_st)
        # final tile's aux runs at the end (smallest tile -> short tail)
        pending["chain"]()
        pending["ln"]()
        pending["se_a"]()
        pending["se_b"]()

    nc.compile()
    return nc


def _get_nc():
    if "nc" not in _cache:
        _cache["nc"] = _build_nc()
    return _cache["nc"]


def _prep_host(inputs):
    import ml_dtypes

    f = np.float32
    bf = ml_dtypes.bfloat16
    x = np.asarray(inputs["x"], f)
    base_weight = np.asarray(inputs["base_weight"], f)
    spline_weight = np.asarray(inputs["spline_weight"], f)
    ln_w = np.asarray(inputs["ln_w"], f)
    ln_b = np.asarray(inputs["ln_b"], f)
    se_w1 = np.asarray(inputs["se_w1"], f)
    se_b1 = np.asarray(inputs["se_b1"], f)
    se_w2 = np.asarray(inputs["se_w2"], f)
    se_b2 = np.asarray(inputs["se_b2"], f)

    f8 = ml_dtypes.float8_e4m3
    xt_all = x.reshape(N_CORES, NTOK, D).transpose(0, 2, 1)  # [core, D, ntok]

    # base channel: 512*Wb in bf16 (2^9 scale is exact); the 512 factor
    # matches the fp8 spline product scale so both share one PSUM bank
    w_base = (512.0 * base_weight.T).reshape(NC_I, P, D).astype(bf)

    wsT = spline_weight.transpose(1, 2, 0)  # [i, g, o]
    wm = wsT[:, 0, :].astype(f)
    w0 = wsT[:, 1, :].astype(f)
    wp = wsT[:, 2, :].astype(f)
    # h/step channels (x512): vd*(Wm-W0) [hi+lo], vc*(Wp-W0) [hi+lo],
    # km*(-Wm), kp*(-Wp); the 0.5 offsets of h and the W0 channel fold into
    # the per-feature constant c0 applied at the PSUM descale
    wmp = (wm - w0) * 512.0
    wpp = (wp - w0) * 512.0
    wmp_hi = wmp.astype(f8)
    wpp_hi = wpp.astype(f8)
    chans = [
        wmp_hi,
        (wmp - wmp_hi.astype(f)).astype(f8),
        wpp_hi,
        (wpp - wpp_hi.astype(f)).astype(f8),
        (-512.0 * wm).astype(f8),
        (-512.0 * wp).astype(f8),
    ]
    w8_all = np.empty((NPAIR, P, NSCH, 2, D), f8)
    for c8, wsrc in enumerate(chans):
        w8_all[:, :, c8, :, :] = wsrc.reshape(NPAIR, 2, P, D).transpose(0, 2, 1, 3)
    w8_all = np.ascontiguousarray(w8_all.reshape(NPAIR, P, NSCH * 2 * D))
    c0 = (w0.sum(axis=0) + 0.5 * (wm - w0).sum(axis=0)
          + 0.5 * (wp - w0).sum(axis=0)).astype(f)              # [o]

    w1p = (se_w1 * ln_w[None, :]).astype(f)          # ln_w folded into W1
    b1p = (se_b1 + se_w1 @ ln_b).astype(f)           # ln_b folded into b1

    shared = {
        "w": w_base,
        "w8": w8_all,
        "c0": np.ascontiguousarray(c0.reshape(NO, P).T),
        "w1t": np.ascontiguousarray(w1p.T.reshape(NO, P, 32)).astype(bf),
        "w2t": np.ascontiguousarray(se_w2.T).astype(bf),
        "lnw": np.ascontiguousarray(ln_w.reshape(NO, P).T),
        "lnb": np.ascontiguousarray(ln_b.reshape(NO, P).T),
        "onesc": np.full((P, 1), 1.0 / D, bf),
        "onesp": np.ones((1, P), bf),
        "b1": np.ascontiguousarray(b1p.reshape(32, 1)),
        "b2": np.ascontiguousarray(se_b2.reshape(NO, P).T),
    }
    in_maps = []
    for k in range(N_CORES):
        m = dict(shared)
        xk = np.ascontiguousarray(xt_all[k].reshape(NC_I, P, NTOK))
        m["xt"] = xk.astype(bf)
        m["xt32"] = xk
        in_maps.append(m)
    return in_maps


def kernel(**inputs) -> np.ndarray:
    from concourse.bass_utils import run_bass_kernel_spmd

    nc = _get_nc()
    in_maps = _prep_host(inputs)
    trace = bool(int(os.environ.get("KERNEL_TRACE", "0")))
    res = run_bass_kernel_spmd(
        nc, in_maps, core_ids=list(range(N_CORES)), trace=trace
    )
    _cache["last_result"] = res
    outs = []
    for k in range(N_CORES):
        outT = np.asarray(res.results[k]["outT"], dtype=np.float32)  # [NO, P, NTOK]
        outs.append(outT.reshape(D, NTOK).T)                          # [ntok, o]
    out = np.concatenate(outs, axis=0).reshape(8, 1024, 1024)
    return np.ascontiguousarray(out.astype(np.float32))
